# revision 84
# baseline (speedup 1.0000x reference)
"""HGNNPConv Trainium2 kernel (8 NeuronCores, SPMD).

Math (equivalent reformulation of the reference):
  Xe_raw[e] = mean_{i: e_idx[i]=e} X[v_idx[i]]              (v2e, softmax of ones = 1/deg)
  Xe_p      = Xe_raw @ W.T + b                              (GEMM on 4000 edges, not 20000 verts)
  Xv[v]     = sum_i wn_i * Xe_p[e_idx[i]],  wn_i = exp(w_i)/sum_{v} exp(w)
              (wn precomputed on host -> no on-chip denominator pass)
  out       = relu(Xv)
Empty edges get a spurious +b in Xe_p but are never referenced downstream
(an edge appearing in phase 2 has >=1 incidence, hence deg>=1 in phase 1).

Sharding: phase 1 by destination edge (500/core), edge-level GEMM per core,
AllGather of the projected edge table (1MB/core), phase 2 by destination
vertex (2500/core). Per-destination-window weighted one-hot selection
matrices (built on DVE from iota) reduce gathered rows on the PE into PSUM.
Phase-1 gather table is fp8-e3m4 (X pre-scaled by 2), phase-2 table bf16,
output bf16 (upcast on host).
"""

import os
from contextlib import ExitStack

import numpy as np
import ml_dtypes

# ---------------------------------------------------------------- config ---
NCORES = 8
NV, NE, NNZ, CH = 20000, 4000, 160000, 512
GATHER_BF16 = os.environ.get("KERNEL_F32", "") == ""  # bf16 tables+matmuls by default
P1_DT = os.environ.get("KERNEL_P1_DT", "f8")   # phase-1 gather table dtype
P2_DT = os.environ.get("KERNEL_P2_DT", "bf16")  # phase-2 gather table dtype
OUT_DT = os.environ.get("KERNEL_OUT_DT", "bf16")
FUSE = os.environ.get("KERNEL_FUSE", "pre")    # "post": GEMM after p2 agg
P1_SCALE = 2.0   # X pre-scale for fp8-e3m4 range use (exact power of 2)
P2_SCALE = 8.0   # Xe_raw pre-scale for the fp8 edge table (post mode)
GRP = 5          # gather chunks (of 128 idxs) per dma_gather call
AG_SLICED = os.environ.get("KERNEL_AG_SLICED", "1") != ""  # per-window AllGather
PRE_PER_B = int(os.environ.get("KERNEL_PRE_PER_B", "0"))  # p2 prefetches per p1 window
CW_MAJOR = os.environ.get("KERNEL_CW_MAJOR", "") != ""  # cw-major p2 table layout
TRACE = os.environ.get("BASS_TRACE", "") != ""


def _mydt(mybir, name):
    return {"f8": mybir.dt.float8e3, "bf16": mybir.dt.bfloat16,
            "f32": mybir.dt.float32}[name]


def _npdt(name):
    return {"f8": ml_dtypes.float8_e3m4, "bf16": ml_dtypes.bfloat16,
            "f32": np.float32}[name]

_last_results = None   # BassKernelResults of the most recent run (for test.py)


# ------------------------------------------------------------------- plan ---
class Plan:
    pass


def _binpack(ids, degs, nbins, cap=128):
    """Pack `ids` into `nbins` bins of <=cap items, balancing sum(degs)."""
    import heapq

    order = np.argsort(-degs, kind="stable")
    bins = [[] for _ in range(nbins)]
    loads = [0] * nbins
    heap = [(0, b) for b in range(nbins)]
    heapq.heapify(heap)
    for t in order:
        popped = []
        while True:
            load, b = heapq.heappop(heap)
            if len(bins[b]) < cap:
                break
            popped.append((load, b))
        for p in popped:
            heapq.heappush(heap, p)
        bins[b].append(int(ids[t]))
        loads[b] = load + int(degs[t])
        heapq.heappush(heap, (loads[b], b))
    return bins, loads


def _csr(idx, n):
    order = np.argsort(idx, kind="stable").astype(np.int64)
    deg = np.bincount(idx, minlength=n).astype(np.int64)
    starts = np.zeros(n + 1, np.int64)
    np.cumsum(deg, out=starts[1:])
    return order, deg, starts


def _pair_window(src, locs, ws):
    """Greedy within-window dedup: incidences sharing a source row become one
    gathered slot with two (loc, w) hots.  Returns (src', loc_a, w_a, loc_b,
    w_b) with pair slots FIRST; singles have loc_b = -1 / w_b = 0."""
    order = np.argsort(src, kind="stable")
    src, locs, ws = src[order], locs[order], ws[order]
    pa, pb, sg = [], [], []
    i, n = 0, len(src)
    while i < n:
        j = i
        while j < n and src[j] == src[i]:
            j += 1
        k = i
        while k + 1 < j:
            pa.append(k); pb.append(k + 1); k += 2
        if k < j:
            sg.append(k)
        i = j
    pa, pb, sg = np.array(pa, np.int64), np.array(pb, np.int64), np.array(sg, np.int64)
    src2 = np.concatenate([src[pa], src[sg]]) if len(pa) else src[sg]
    la = np.concatenate([locs[pa], locs[sg]]) if len(pa) else locs[sg]
    wa = np.concatenate([ws[pa], ws[sg]]) if len(pa) else ws[sg]
    lb = np.concatenate([locs[pb], np.full(len(sg), -1.0, locs.dtype)]) if len(pa) \
        else np.full(len(sg), -1.0, locs.dtype)
    wb = np.concatenate([ws[pb], np.zeros(len(sg), ws.dtype)]) if len(pa) \
        else np.zeros(len(sg), ws.dtype)
    return src2, la, wa, lb, wb, len(pa)


def _phase_windows(bins_per_core, order, starts, idx_of_inc, w_of_inc,
                   loc_dtype=np.float32, pair=True, piece_order=False):
    """Per-window slot lists for one core of one phase, after source dedup.

    Returns (wins, wmax): wins[w] = (src, loc_a, w_a, loc_b, w_b).  By
    default pair slots (loc_b >= 0) come first within each window; with
    piece_order=True, singles come first sorted by source id (so early
    chunks only reference low table pieces) and pair slots go last.
    """
    wins = []
    wmax = 0
    for bin_ids in bins_per_core:
        incs = []
        locs = []
        for j, d in enumerate(bin_ids):
            seg = order[starts[d]:starts[d + 1]]
            incs.append(seg)
            locs.append(np.full(len(seg), j, loc_dtype))
        incs = np.concatenate(incs) if incs else np.zeros(0, np.int64)
        locs = np.concatenate(locs) if locs else np.zeros(0, loc_dtype)
        src = idx_of_inc[incs]
        ws = w_of_inc[incs].astype(np.float32)
        if pair and len(src):
            src, la, wa, lb, wb, npair = _pair_window(src, locs, ws)
            if piece_order:
                ps = np.argsort(src[:npair], kind="stable")
                ss = npair + np.argsort(src[npair:], kind="stable")
                perm = np.concatenate([ss, ps])
                src, la, wa, lb, wb = (src[perm], la[perm], wa[perm],
                                       lb[perm], wb[perm])
        else:
            la, wa = locs, ws
            lb = np.full(len(src), -1.0, loc_dtype)
            wb = np.zeros(len(src), np.float32)
        wins.append((src, la, wa, lb, wb))
        wmax = max(wmax, len(src))
    return wins, wmax


def _layout(wins, W, nw, cw_major=False):
    """Flat slot arrays.  Table column of window w's cw-th chunk is
    cw*nw + w when cw_major (so the cw-th chunks of all windows are a
    consecutive gather range) else w*W + cw."""
    L = nw * W * 128
    gidx = np.zeros(L, np.int16)
    loc = np.full(L, -1.0, np.float32)
    wsel = np.zeros(L, np.float32)
    locb = np.full(L, -1.0, np.float32)
    wselb = np.zeros(L, np.float32)
    for w, (src, la, wa, lb, wb) in enumerate(wins):
        for cw in range(-(-len(src) // 128) if len(src) else 0):
            t = cw * nw + w if cw_major else w * W + cw
            i0, i1 = cw * 128, min((cw + 1) * 128, len(src))
            o = t * 128
            n = i1 - i0
            gidx[o:o + n] = src[i0:i1]
            loc[o:o + n] = la[i0:i1]
            wsel[o:o + n] = wa[i0:i1]
            locb[o:o + n] = lb[i0:i1]
            wselb[o:o + n] = wb[i0:i1]
    return gidx, loc, wsel, locb, wselb


def _wrap_idx(flat):
    """int16 flat[i] -> [128, len/16] with value i at [i%16, i//16], replicated."""
    a = flat.reshape(-1, 16).T  # [16, L/16]
    return np.ascontiguousarray(np.tile(a, (8, 1)))


def _pack(flat, C):
    """flat[c*128+p] -> [128, C]"""
    return np.ascontiguousarray(flat.reshape(C, 128).T)


def _dedup_slots(srcs):
    """#gather slots for a window's source list after pairing."""
    if not len(srcs):
        return 0
    _, cnt = np.unique(srcs, return_counts=True)
    return int(((cnt + 1) // 2).sum())


def _repair_bins(bins, order, starts, idx_of_inc, cap=128, iters=400):
    """Greedy rebalance: move members out of the window with the most
    post-dedup slots into the one with the fewest (respecting the member
    cap), to minimize max slots per window."""
    srcs = [
        [idx_of_inc[order[starts[d]:starts[d + 1]]] for d in b] for b in bins
    ]

    def slots(w):
        return _dedup_slots(np.concatenate(srcs[w]) if srcs[w] else
                            np.zeros(0, np.int64))

    cur = [slots(w) for w in range(len(bins))]
    for _ in range(iters):
        hot = int(np.argmax(cur))
        order_cold = np.argsort(cur)
        moved = False
        for cold in order_cold:
            if cold == hot or len(bins[cold]) >= cap:
                continue
            # move the member with the smallest segment out of `hot`
            j = int(np.argmin([len(s) for s in srcs[hot]]))
            bins[cold].append(bins[hot].pop(j))
            srcs[cold].append(srcs[hot].pop(j))
            new_hot, new_cold = slots(hot), slots(cold)
            if max(new_hot, new_cold) >= cur[hot]:
                # revert: no improvement
                bins[hot].append(bins[cold].pop())
                srcs[hot].append(srcs[cold].pop())
                continue
            cur[hot], cur[cold] = new_hot, new_cold
            moved = True
            break
        if not moved:
            break
    return bins


def make_plan(v_idx, e_idx, e2v_weight, nv=NV, ne=NE, ch=CH, ncores=NCORES):
    P = Plan()
    P.nv, P.ne, P.ch, P.ncores = nv, ne, ch, ncores
    epc, vpc = ne // ncores, nv // ncores
    P.epc, P.vpc = epc, vpc

    order_e, deg_e, starts_e = _csr(e_idx, ne)
    order_v, deg_v, starts_v = _csr(v_idx, nv)
    inv_deg = np.zeros(ne, np.float32)
    nz = deg_e > 0
    inv_deg[nz] = (np.float32(1.0) / deg_e[nz].astype(np.float32))

    nb1 = -(-epc // 128)
    nb2 = -(-vpc // 128)
    v_of_inc = v_idx.astype(np.int64)
    e_of_inc = e_idx.astype(np.int64)
    # balance destinations across cores globally (assignment is free — pos /
    # vmap carry it), then binpack windows within each core and rebalance for
    # post-dedup slot counts.
    cores_e, _ = _binpack(np.arange(ne), deg_e, ncores, cap=nb1 * 128)
    bins1 = []
    for k in range(ncores):
        eids = np.asarray(cores_e[k])
        b, _ = _binpack(eids, deg_e[eids], nb1)
        bins1.append(_repair_bins(b, order_e, starts_e, v_of_inc))

    # p2 window count: an extra window can admit a smaller W2 (less gather
    # padding) once dedup shrinks the per-window loads — pick the best.
    best = None
    for nb2c in (nb2, nb2 + 1):
        cores_v, _ = _binpack(np.arange(nv), deg_v, ncores, cap=nb2c * 128)
        cand = []
        wmax = 0
        for k in range(ncores):
            vids = np.asarray(cores_v[k])
            b, _ = _binpack(vids, deg_v[vids], nb2c)
            b = _repair_bins(b, order_v, starts_v, e_of_inc)
            cand.append(b)
            for bb in b:
                wmax = max(wmax, _dedup_slots(np.concatenate(
                    [e_of_inc[order_v[starts_v[d]:starts_v[d + 1]]]
                     for d in bb]) if bb else np.zeros(0, np.int64)))
        W2c = -(-wmax // 128)
        if best is None or nb2c * W2c < best[0] * best[1]:
            best = (nb2c, W2c, cand)
    nb2, _, bins2 = best
    P.NW1, P.NW2 = nb1, nb2

    # phase-1 windows (dedup within window) + edge position map.  Positions
    # are window-major (w, core, row) so each window's table slice can be
    # AllGathered independently as soon as its GEMM finishes.
    pos = np.zeros(ne, np.int64)
    wins1 = []
    w1max = 0
    for k in range(ncores):
        wins, wmax = _phase_windows(
            bins1[k], order_e, starts_e, v_idx.astype(np.int64),
            inv_deg[e_idx.astype(np.int64)])
        wins1.append(wins)
        w1max = max(w1max, wmax)
        for w, bin_ids in enumerate(bins1[k]):
            for j, e in enumerate(bin_ids):
                if AG_SLICED:   # window-major: (w, core, row)
                    pos[e] = (w * ncores + k) * 128 + j
                else:           # core-major: AllGather rank concatenation
                    pos[e] = (k * nb1 + w) * 128 + j
    assert pos.max() < 32768
    P.W1 = -(-w1max // 128)
    P.C1 = P.NW1 * P.W1
    P.p1 = [_layout(wins, P.W1, P.NW1) for wins in wins1]

    # phase-2 windows + output row map. Softmax weights are fully normalized
    # on the host (exp / per-vertex sum), so the kernel needs no denominator
    # pass.
    expw = np.exp(e2v_weight.astype(np.float64))
    den = np.zeros(nv, np.float64)
    np.add.at(den, v_idx, expw)
    wnorm = (expw / den[v_idx.astype(np.int64)]).astype(np.float32)
    wins2 = []
    w2max = 0
    P.vmap = []
    for k in range(ncores):
        wins, wmax = _phase_windows(
            bins2[k], order_v, starts_v, pos[e_idx.astype(np.int64)], wnorm,
            piece_order=AG_SLICED)
        wins2.append(wins)
        w2max = max(w2max, wmax)
        vm = np.full(P.NW2 * 128, -1, np.int64)
        for w, bin_ids in enumerate(bins2[k]):
            vm[w * 128:w * 128 + len(bin_ids)] = bin_ids
        # deg-0 vertices never receive contributions; drop them from the
        # output map so any on-chip garbage (e.g. a stray +b) is discarded.
        vme = vm[vm >= 0]
        vm[vm >= 0] = np.where(deg_v[vme] > 0, vme, -1)
        P.vmap.append(vm)
    P.W2 = -(-w2max // 128)
    P.C2 = P.NW2 * P.W2
    P.p2 = [_layout(wins, P.W2, P.NW2, cw_major=CW_MAJOR) for wins in wins2]
    return P


# ---------------------------------------------------------------- builder ---
def build_nc(P, bf16=GATHER_BF16, spmd=True, reps=1, grp=GRP, gbufs=6,
             nqueues=1, p1_dt=P1_DT, p2_dt=P2_DT, out_dt=OUT_DT, fuse=FUSE):
    import concourse.bacc as bacc
    import concourse.mybir as mybir
    import concourse.tile as tile

    f32 = mybir.dt.float32
    dt_g = mybir.dt.bfloat16 if bf16 else f32   # sel matrices + GEMM operands
    dt_p1 = _mydt(mybir, p1_dt)
    dt_p2 = _mydt(mybir, p2_dt)
    dt_out = _mydt(mybir, out_dt)
    eq, mul, mx, add = (mybir.AluOpType.is_equal, mybir.AluOpType.mult,
                        mybir.AluOpType.max, mybir.AluOpType.add)
    ch, KT = P.ch, P.ch // 128
    post = fuse == "post"

    nc = bacc.Bacc("TRN2", target_bir_lowering=False, debug=False,
                   num_devices=P.ncores if spmd else 1,
                   num_swdge_queues=nqueues)

    XT = nc.dram_tensor("xt", [P.nv, ch], dt_p1, kind="ExternalInput")
    WT = nc.dram_tensor("wt", [128, KT, ch], dt_g, kind="ExternalInput")
    BT = nc.dram_tensor("bt", [1, ch], dt_g, kind="ExternalInput")
    IOTA = nc.dram_tensor("iota", [128, 128], dt_g, kind="ExternalInput")
    IDENT = nc.dram_tensor("ident", [128, 128], dt_g, kind="ExternalInput")
    G1IDX = nc.dram_tensor("g1idx", [128, P.C1 * 8], mybir.dt.int16, kind="ExternalInput")
    P1TAB = nc.dram_tensor("p1tab", [128, 4, P.C1], f32, kind="ExternalInput")
    G2IDX = nc.dram_tensor("g2idx", [128, P.C2 * 8], mybir.dt.int16, kind="ExternalInput")
    P2TAB = nc.dram_tensor("p2tab", [128, 4, P.C2], f32, kind="ExternalInput")

    ner1 = P.NW1 * 128
    CCIN = nc.dram_tensor("ccin", [ner1, ch], dt_p2)
    CCOUT = nc.dram_tensor("ccout", [P.ncores * ner1, ch], dt_p2, addr_space="Shared")
    OUT = nc.dram_tensor("out", [P.NW2 * 128, ch], dt_out, kind="ExternalOutput")

    with tile.TileContext(nc) as tc, ExitStack() as ctx:
        const = ctx.enter_context(tc.tile_pool(name="const", bufs=1))
        gpool = ctx.enter_context(tc.tile_pool(name="g", bufs=gbufs))
        g2pool = ctx.enter_context(tc.tile_pool(name="g2", bufs=gbufs))
        prep = ctx.enter_context(tc.tile_pool(name="prep", bufs=1))
        selp = ctx.enter_context(tc.tile_pool(name="selp", bufs=6))
        psum = ctx.enter_context(tc.tile_pool(name="ps", bufs=2, space="PSUM"))
        sbp = ctx.enter_context(tc.tile_pool(name="sbp", bufs=2))
        pre_tiles = {}

        def cload(dram, shape, dt, tag, eng=None):
            t = const.tile(shape, dt, tag=tag)
            (eng or nc.sync).dma_start(t[:], dram[:])
            return t

        # p1-critical tables first (SP ring); the rest go on the ACT ring so
        # they never delay the first gather.
        iota_t = cload(IOTA, [128, 128], dt_g, "iota")
        g1idx_t = cload(G1IDX, [128, P.C1 * 8], mybir.dt.int16, "g1idx")
        p1tab_t = cload(P1TAB, [128, 4, P.C1], f32, "p1tab")
        eloc1_t, wsel1_t = p1tab_t[:, 0, :], p1tab_t[:, 1, :]
        eloc1b_t, wsel1b_t = p1tab_t[:, 2, :], p1tab_t[:, 3, :]
        wt_t = cload(WT, [128, KT, ch], dt_g, "wt", eng=nc.scalar)
        bt_t = cload(BT, [1, ch], dt_g, "bt", eng=nc.scalar)
        ident_t = cload(IDENT, [128, 128], dt_g, "ident", eng=nc.scalar)
        g2idx_t = cload(G2IDX, [128, P.C2 * 8], mybir.dt.int16, "g2idx",
                        eng=nc.scalar)
        p2tab_t = cload(P2TAB, [128, 4, P.C2], f32, "p2tab", eng=nc.scalar)
        vloc2_t, w2raw_t = p2tab_t[:, 0, :], p2tab_t[:, 1, :]
        vloc2b_t, w2rawb_t = p2tab_t[:, 2, :], p2tab_t[:, 3, :]
        ones1_t = const.tile([1, 128], dt_g, tag="ones1")
        nc.vector.memset(ones1_t[:], 1.0)

        # ---------------- gather + one/two-hot reduce ----------------------
        def agg_phase(src_ap, gidx_t, loc_t, w_t, locb_t, wb_t, is2, C, W,
                      gtag, chunk_cb, win_cb, dt_tab, src_sel=None,
                      pre_tiles=None, cw_major=False, pool=None):
            """Consumption iterates (window, cw); table column t of a chunk is
            cw*NW + w when cw_major else the position itself.  Gather calls
            cover consecutive table columns; with cw_major a call spans the
            cw-th chunks of `grp` windows, so its source-prefix (src_sel)
            dependency stays low for early cw ranks."""
            pre_tiles = pre_tiles or {}
            pool = pool or gpool
            NW = C // W
            tcol = (lambda w, cw: cw * NW + w) if cw_major \
                else (lambda w, cw: w * W + cw)
            # calls = runs of consecutive non-prefetched table columns, never
            # crossing a cw-group boundary in cw_major mode
            calls = []
            run = []
            bounds = set(cw * NW for cw in range(W)) if cw_major else set()
            for t in range(C):
                if t in pre_tiles or len(run) == grp or (run and t in bounds):
                    if run:
                        calls.append((run[0], len(run)))
                    run = []
                if t not in pre_tiles:
                    run.append(t)
            if run:
                calls.append((run[0], len(run)))
            call_of = {}
            for g0, n in calls:
                for j in range(n):
                    call_of[g0 + j] = (g0, n)
            # issue each call right before its first-consumed chunk
            first_use = {}
            for g0, n in calls:
                p0 = min((t % NW) * W + t // NW if cw_major else t
                         for t in range(g0, g0 + n))
                first_use.setdefault(p0, []).append((g0, n))
            tiles = {}
            pw = None
            for p in range(C):
                for g0, n in first_use.get(p, []):
                    gt_new = pool.tile([128, grp, ch], dt_tab, tag=gtag)
                    src = src_sel(g0, n) if src_sel is not None else src_ap
                    nc.gpsimd.dma_gather(
                        gt_new[:, 0:n, :], src,
                        gidx_t[:, g0 * 8:(g0 + n) * 8],
                        n * 128, n * 128, ch,
                        queue_num=(g0 // grp) % nqueues)
                    tiles[g0] = gt_new
                w, cw = divmod(p, W)
                t = tcol(w, cw)
                if t in pre_tiles:
                    gt, j = pre_tiles[t]
                else:
                    g0, n = call_of[t]
                    gt, j = tiles[g0], t - g0
                sel = selp.tile([128, 128], dt_g, tag="sel")
                nc.vector.tensor_scalar(
                    sel[:], iota_t[:], loc_t[:, t:t + 1], w_t[:, t:t + 1],
                    op0=eq, op1=mul)
                if is2[t]:  # dedup chunk: add the second hot
                    selb = selp.tile([128, 128], dt_g, tag="selb")
                    nc.vector.tensor_scalar(
                        selb[:], iota_t[:], locb_t[:, t:t + 1],
                        wb_t[:, t:t + 1], op0=eq, op1=mul)
                    sel2 = selp.tile([128, 128], dt_g, tag="sel2")
                    nc.vector.tensor_tensor(sel2[:], sel[:], selb[:], op=add)
                    sel = sel2
                if cw == 0:
                    pw = psum.tile([128, ch], f32, tag="win")
                chunk_cb(pw, sel, gt, j, w, cw, cw == W - 1)
                if cw == W - 1:
                    win_cb(pw, w)

        def p1_chunk(pw, sel, gt, j, w, cw, last):
            nc.tensor.matmul(pw[:], sel[:], gt[:, j, :],
                             start=(cw == 0), stop=last)

        def gemm_bias(src_t, dst_psum):
            """dst[v/e, co] = src^T blocks @ W.T + 1^T b (K=1 bias matmul)."""
            for k in range(KT):
                nc.tensor.matmul(dst_psum[:], src_t[:, k, :], wt_t[:, k, :],
                                 start=(k == 0), stop=False)
            nc.tensor.matmul(dst_psum[:], ones1_t[:], bt_t[:],
                             start=False, stop=True)

        def transpose_blocks(pw, tag):
            """psum [128, ch] f32 -> sbuf [128, KT, 128] dt_g transposed."""
            t_w = sbp.tile([128, ch], dt_g, tag=tag + "f", name=tag + "f")
            nc.vector.tensor_copy(t_w[:], pw[:])
            tT_w = sbp.tile([128, KT, 128], dt_g, tag=tag + "T", name=tag + "T")
            for k in range(KT):
                pt = psum.tile([128, 128], dt_g, tag="aux", name="pt")
                nc.tensor.transpose(pt[:], t_w[:, k * 128:(k + 1) * 128],
                                    ident_t[:])
                nc.vector.tensor_copy(tT_w[:, k, :], pt[:])
            return tT_w

        def p1_win(pw, w):
            # window w's edge rows are complete: ship its CCIN slice and
            # immediately AllGather that window's table piece, overlapping
            # the collective with the remaining p1 windows.
            xep = sbp.tile([128, ch], dt_p2, tag="xep", name="xep")
            if post:
                # raw table, scaled for fp8 range; GEMM happens after p2 agg
                nc.vector.tensor_scalar(xep[:], pw[:], float(P2_SCALE), None,
                                        op0=mul)
            else:
                xeT_w = transpose_blocks(pw, "xe")
                pg = psum.tile([128, ch], f32, tag="gemm", name="pg")
                gemm_bias(xeT_w, pg)
                nc.vector.tensor_copy(xep[:], pg[:])
            nc.sync.dma_start(CCIN[w * 128:(w + 1) * 128, :], xep[:])
            if AG_SLICED:
                o = w * P.ncores * 128
                if spmd:
                    nc.gpsimd.collective_compute(
                        "AllGather", mybir.AluOpType.bypass,
                        replica_groups=[list(range(P.ncores))],
                        ins=[CCIN[w * 128:(w + 1) * 128, :]],
                        outs=[CCOUT[o:o + P.ncores * 128, :]])
                else:  # single-core stand-in for the window AllGather
                    nc.sync.dma_start(CCOUT[o:o + 128, :],
                                      CCIN[w * 128:(w + 1) * 128, :])
            elif w == P.NW1 - 1:
                if spmd:
                    nc.gpsimd.collective_compute(
                        "AllGather", mybir.AluOpType.bypass,
                        replica_groups=[list(range(P.ncores))],
                        ins=[CCIN[:]], outs=[CCOUT[:]])
                else:
                    nc.sync.dma_start(CCOUT[0:ner1, :], CCIN[:])
            # prefetch p2 chunks whose table pieces are already gathered,
            # soaking p1's spare DMA bandwidth
            for c in pre_sched.get(w, []):
                gt = prep.tile([128, 1, ch], dt_p2, tag=f"pre{c}")
                nc.gpsimd.dma_gather(
                    gt[:], p2_src(c, 1), g2idx_t[:, c * 8:(c + 1) * 8],
                    128, 128, ch, queue_num=0)
                pre_tiles[c] = (gt, 0)

        def p2_chunk(pw, sel, gt, j, w, cw, last):
            nc.tensor.matmul(pw[:], sel[:], gt[:, j, :],
                             start=(cw == 0), stop=last)

        def p2_win(pw, w):
            if post:
                awT = transpose_blocks(pw, "aw")
                po = psum.tile([128, ch], f32, tag="gemm", name="po")
                gemm_bias(awT, po)
                pw = po
            # weights pre-normalized on host: just relu + store
            ow = sbp.tile([128, ch], dt_out, tag="ow", name="ow")
            nc.vector.tensor_scalar(ow[:], pw[:], 1.0, 0.0, op0=mul, op1=mx)
            nc.sync.dma_start(OUT[w * 128:(w + 1) * 128, :], ow[:])

        # chunks that contain any dedup pair need the second sel pass; the
        # union over cores keeps the SPMD program identical on every core.
        is2_1 = np.zeros(P.C1, bool)
        is2_2 = np.zeros(P.C2, bool)
        maxrow2 = np.zeros(P.C2, np.int64)
        for k in range(P.ncores):
            is2_1 |= (_pack(P.p1[k][3], P.C1) >= 0).any(axis=0)
            is2_2 |= (_pack(P.p2[k][3], P.C2) >= 0).any(axis=0)
            maxrow2 = np.maximum(
                maxrow2, _pack(P.p2[k][0], P.C2).astype(np.int64).max(axis=0))

        piece = P.ncores * 128
        def p2_src(g0, n):
            # prefix slice of the edge table covering every row this gather
            # call touches, so it only waits on the AllGather pieces it needs
            pieces = int(maxrow2[g0:g0 + n].max()) // piece + 1
            if not AG_SLICED or pieces >= P.NW1:
                return CCOUT[:]
            return CCOUT[0:pieces * piece, :]

        # p2 prefetch schedule: at p1 window boundary w we may issue gathers
        # for p2 chunks that only need table pieces < w (their AllGather was
        # triggered a full window earlier).  Earliest-consumed chunks first.
        bound = (maxrow2 // piece + 1).astype(int)   # pieces needed per chunk
        pre_sched = {w: [] for w in range(1, P.NW1)}
        if AG_SLICED and PRE_PER_B > 0:
            taken = set()
            for w in range(1, P.NW1):
                for c in range(P.C2):
                    if len(pre_sched[w]) >= PRE_PER_B:
                        break
                    if c not in taken and bound[c] <= w:
                        pre_sched[w].append(c)
                        taken.add(c)

        for _rep in range(reps):
            pre_tiles.clear()
            agg_phase(XT[:], g1idx_t, eloc1_t, wsel1_t, eloc1b_t, wsel1b_t,
                      is2_1, P.C1, P.W1, "g1", p1_chunk, p1_win, dt_p1)

            # phase 2: e2v aggregation (sel weights pre-normalized on host)
            agg_phase(CCOUT[:], g2idx_t, vloc2_t, w2raw_t, vloc2b_t, w2rawb_t,
                      is2_2, P.C2, P.W2, "g2", p2_chunk, p2_win, dt_p2,
                      src_sel=p2_src, pre_tiles=pre_tiles,
                      cw_major=CW_MAJOR, pool=g2pool)

    nc.compile()
    return nc


# ------------------------------------------------------------------ runner ---
def make_in_maps(P, X, W, b, bf16=GATHER_BF16, p1_dt=P1_DT, fuse=FUSE):
    npdt = ml_dtypes.bfloat16 if bf16 else np.float32
    np_p1 = _npdt(p1_dt)
    s1 = P1_SCALE if p1_dt == "f8" else 1.0
    s2 = P2_SCALE if fuse == "post" else 1.0
    KT = P.ch // 128
    xt = np.ascontiguousarray((X * s1).astype(np_p1))
    wt = np.ascontiguousarray(
        W.T.reshape(KT, 128, P.ch).transpose(1, 0, 2).astype(npdt))
    bt = np.ascontiguousarray(b.astype(npdt).reshape(1, P.ch))
    iota = np.ascontiguousarray(
        np.broadcast_to(np.arange(128, dtype=npdt), (128, 128)))
    ident = np.eye(128, dtype=npdt)

    def tb(flat, C, s=1.0):
        return _pack(flat, C) / np.float32(s)

    in_maps = []
    for k in range(P.ncores):
        g1, l1, w1, l1b, w1b = P.p1[k]
        g2, l2, w2, l2b, w2b = P.p2[k]
        p1tab = np.ascontiguousarray(np.stack(
            [tb(l1, P.C1), tb(w1, P.C1, s1), tb(l1b, P.C1), tb(w1b, P.C1, s1)],
            axis=1))
        p2tab = np.ascontiguousarray(np.stack(
            [tb(l2, P.C2), tb(w2, P.C2, s2), tb(l2b, P.C2), tb(w2b, P.C2, s2)],
            axis=1))
        in_maps.append({
            "xt": xt, "wt": wt, "bt": bt, "iota": iota, "ident": ident,
            "g1idx": _wrap_idx(g1), "p1tab": p1tab,
            "g2idx": _wrap_idx(g2), "p2tab": p2tab,
        })
    return in_maps


def assemble(P, shards):
    out = np.zeros((P.nv, P.ch), np.float32)
    for k in range(P.ncores):
        vm = P.vmap[k]
        m = vm >= 0
        out[vm[m]] = shards[k][m].astype(np.float32)
    return out


_nc_cache = {}


def kernel(X, W, b, e2v_weight, v_idx, e_idx):
    global _last_results
    from concourse.bass_utils import run_bass_kernel_spmd

    P = make_plan(v_idx, e_idx, e2v_weight)
    key = (P.C1, P.C2, P.W1, P.W2, GATHER_BF16, P1_DT, P2_DT, OUT_DT, FUSE,
           AG_SLICED)
    if key not in _nc_cache:
        _nc_cache[key] = build_nc(P)
    nc = _nc_cache[key]
    in_maps = make_in_maps(P, X, W, b)
    res = run_bass_kernel_spmd(nc, in_maps, list(range(P.ncores)), trace=TRACE)
    _last_results = res
    shards = [res.results[k]["out"] for k in range(P.ncores)]
    return assemble(P, shards)



# revision 88
# speedup vs baseline: 1.0092x; 1.0092x over previous
"""HGNNPConv Trainium2 kernel (8 NeuronCores, SPMD).

Math (equivalent reformulation of the reference):
  Xe_raw[e] = mean_{i: e_idx[i]=e} X[v_idx[i]]              (v2e, softmax of ones = 1/deg)
  Xe_p      = Xe_raw @ W.T + b                              (GEMM on 4000 edges, not 20000 verts)
  Xv[v]     = sum_i wn_i * Xe_p[e_idx[i]],  wn_i = exp(w_i)/sum_{v} exp(w)
              (wn precomputed on host -> no on-chip denominator pass)
  out       = relu(Xv)
Deg-0 vertices are masked host-side in assemble(); empty edges get a spurious
+b in Xe_p but are never referenced downstream.

Sharding: edges and vertices are binpacked across the 8 cores (balancing
post-dedup gather slots).  Phase 1 aggregates by destination edge, runs the
edge-level GEMM per 128-edge window (bias folded in as a K=1 matmul), and
AllGathers each window's table slice as soon as it is ready (overlapping the
collective with the remaining phase-1 work).  Phase 2 aggregates by
destination vertex; each gather call's source AP is a prefix slice of the
table covering only the AllGather pieces it needs.

Per-destination-window weighted one-hot selection matrices (built on DVE
from a bf16 iota) reduce gathered rows on the PE into PSUM.  Incidences
sharing a source row within a window are deduplicated into one gathered slot
with a two-hot sel column (~11% fewer gather bytes; window binpacking is
rebalanced for post-dedup slot counts, and phase 2 uses 21 windows x 7
chunks instead of 20 x 8).

Dtypes: phase-1 gather table fp8-e3m4 (X pre-scaled by 2: worst-case quant
error halves vs e4m3 and the GEMM averages it out), phase-2 table bf16
(absmax-norm tolerance rules out fp8 post-GEMM), output bf16 (upcast on
host).  End-to-end rel err 1.35e-2 vs the 2e-2 gate.
"""

import os
from contextlib import ExitStack

import numpy as np
import ml_dtypes

# ---------------------------------------------------------------- config ---
NCORES = 8
NV, NE, NNZ, CH = 20000, 4000, 160000, 512
GATHER_BF16 = os.environ.get("KERNEL_F32", "") == ""  # bf16 tables+matmuls by default
P1_DT = os.environ.get("KERNEL_P1_DT", "f8")   # phase-1 gather table dtype
P2_DT = os.environ.get("KERNEL_P2_DT", "bf16")  # phase-2 gather table dtype
OUT_DT = os.environ.get("KERNEL_OUT_DT", "bf16")
FUSE = os.environ.get("KERNEL_FUSE", "pre")    # "post": GEMM after p2 agg
P1_SCALE = 2.0   # X pre-scale for fp8-e3m4 range use (exact power of 2)
P2_SCALE = 8.0   # Xe_raw pre-scale for the fp8 edge table (post mode)
GRP = 5          # gather chunks (of 128 idxs) per dma_gather call
AG_SLICED = os.environ.get("KERNEL_AG_SLICED", "1") != ""  # per-window AllGather
PRE_PER_B = int(os.environ.get("KERNEL_PRE_PER_B", "0"))  # p2 prefetches per p1 window
CW_MAJOR = os.environ.get("KERNEL_CW_MAJOR", "") != ""  # cw-major p2 table layout
TRACE = os.environ.get("BASS_TRACE", "") != ""


def _mydt(mybir, name):
    return {"f8": mybir.dt.float8e3, "bf16": mybir.dt.bfloat16,
            "f32": mybir.dt.float32}[name]


def _npdt(name):
    return {"f8": ml_dtypes.float8_e3m4, "bf16": ml_dtypes.bfloat16,
            "f32": np.float32}[name]

_last_results = None   # BassKernelResults of the most recent run (for test.py)


# ------------------------------------------------------------------- plan ---
class Plan:
    pass


def _binpack(ids, degs, nbins, cap=128):
    """Pack `ids` into `nbins` bins of <=cap items, balancing sum(degs)."""
    import heapq

    order = np.argsort(-degs, kind="stable")
    bins = [[] for _ in range(nbins)]
    loads = [0] * nbins
    heap = [(0, b) for b in range(nbins)]
    heapq.heapify(heap)
    for t in order:
        popped = []
        while True:
            load, b = heapq.heappop(heap)
            if len(bins[b]) < cap:
                break
            popped.append((load, b))
        for p in popped:
            heapq.heappush(heap, p)
        bins[b].append(int(ids[t]))
        loads[b] = load + int(degs[t])
        heapq.heappush(heap, (loads[b], b))
    return bins, loads


def _csr(idx, n):
    order = np.argsort(idx, kind="stable").astype(np.int64)
    deg = np.bincount(idx, minlength=n).astype(np.int64)
    starts = np.zeros(n + 1, np.int64)
    np.cumsum(deg, out=starts[1:])
    return order, deg, starts


def _pair_window(src, locs, ws):
    """Greedy within-window dedup: incidences sharing a source row become one
    gathered slot with two (loc, w) hots.  Returns (src', loc_a, w_a, loc_b,
    w_b) with pair slots FIRST; singles have loc_b = -1 / w_b = 0."""
    order = np.argsort(src, kind="stable")
    src, locs, ws = src[order], locs[order], ws[order]
    pa, pb, sg = [], [], []
    i, n = 0, len(src)
    while i < n:
        j = i
        while j < n and src[j] == src[i]:
            j += 1
        k = i
        while k + 1 < j:
            pa.append(k); pb.append(k + 1); k += 2
        if k < j:
            sg.append(k)
        i = j
    pa, pb, sg = np.array(pa, np.int64), np.array(pb, np.int64), np.array(sg, np.int64)
    src2 = np.concatenate([src[pa], src[sg]]) if len(pa) else src[sg]
    la = np.concatenate([locs[pa], locs[sg]]) if len(pa) else locs[sg]
    wa = np.concatenate([ws[pa], ws[sg]]) if len(pa) else ws[sg]
    lb = np.concatenate([locs[pb], np.full(len(sg), -1.0, locs.dtype)]) if len(pa) \
        else np.full(len(sg), -1.0, locs.dtype)
    wb = np.concatenate([ws[pb], np.zeros(len(sg), ws.dtype)]) if len(pa) \
        else np.zeros(len(sg), ws.dtype)
    return src2, la, wa, lb, wb, len(pa)


def _phase_windows(bins_per_core, order, starts, idx_of_inc, w_of_inc,
                   loc_dtype=np.float32, pair=True, piece_order=False):
    """Per-window slot lists for one core of one phase, after source dedup.

    Returns (wins, wmax): wins[w] = (src, loc_a, w_a, loc_b, w_b).  By
    default pair slots (loc_b >= 0) come first within each window; with
    piece_order=True, singles come first sorted by source id (so early
    chunks only reference low table pieces) and pair slots go last.
    """
    wins = []
    wmax = 0
    for bin_ids in bins_per_core:
        incs = []
        locs = []
        for j, d in enumerate(bin_ids):
            seg = order[starts[d]:starts[d + 1]]
            incs.append(seg)
            locs.append(np.full(len(seg), j, loc_dtype))
        incs = np.concatenate(incs) if incs else np.zeros(0, np.int64)
        locs = np.concatenate(locs) if locs else np.zeros(0, loc_dtype)
        src = idx_of_inc[incs]
        ws = w_of_inc[incs].astype(np.float32)
        if pair and len(src):
            src, la, wa, lb, wb, npair = _pair_window(src, locs, ws)
            if piece_order:
                ps = np.argsort(src[:npair], kind="stable")
                ss = npair + np.argsort(src[npair:], kind="stable")
                perm = np.concatenate([ss, ps])
                src, la, wa, lb, wb = (src[perm], la[perm], wa[perm],
                                       lb[perm], wb[perm])
        else:
            la, wa = locs, ws
            lb = np.full(len(src), -1.0, loc_dtype)
            wb = np.zeros(len(src), np.float32)
        wins.append((src, la, wa, lb, wb))
        wmax = max(wmax, len(src))
    return wins, wmax


def _layout(wins, W, nw, cw_major=False):
    """Flat slot arrays.  Table column of window w's cw-th chunk is
    cw*nw + w when cw_major (so the cw-th chunks of all windows are a
    consecutive gather range) else w*W + cw."""
    L = nw * W * 128
    gidx = np.zeros(L, np.int16)
    loc = np.full(L, -1.0, np.float32)
    wsel = np.zeros(L, np.float32)
    locb = np.full(L, -1.0, np.float32)
    wselb = np.zeros(L, np.float32)
    for w, (src, la, wa, lb, wb) in enumerate(wins):
        for cw in range(-(-len(src) // 128) if len(src) else 0):
            t = cw * nw + w if cw_major else w * W + cw
            i0, i1 = cw * 128, min((cw + 1) * 128, len(src))
            o = t * 128
            n = i1 - i0
            gidx[o:o + n] = src[i0:i1]
            loc[o:o + n] = la[i0:i1]
            wsel[o:o + n] = wa[i0:i1]
            locb[o:o + n] = lb[i0:i1]
            wselb[o:o + n] = wb[i0:i1]
    return gidx, loc, wsel, locb, wselb


def _wrap_idx(flat):
    """int16 flat[i] -> [128, len/16] with value i at [i%16, i//16], replicated."""
    a = flat.reshape(-1, 16).T  # [16, L/16]
    return np.ascontiguousarray(np.tile(a, (8, 1)))


def _pack(flat, C):
    """flat[c*128+p] -> [128, C]"""
    return np.ascontiguousarray(flat.reshape(C, 128).T)


def _dedup_slots(srcs):
    """#gather slots for a window's source list after pairing."""
    if not len(srcs):
        return 0
    _, cnt = np.unique(srcs, return_counts=True)
    return int(((cnt + 1) // 2).sum())


def _repair_bins(bins, order, starts, idx_of_inc, cap=128, iters=400):
    """Greedy rebalance: move members out of the window with the most
    post-dedup slots into the one with the fewest (respecting the member
    cap), to minimize max slots per window."""
    srcs = [
        [idx_of_inc[order[starts[d]:starts[d + 1]]] for d in b] for b in bins
    ]

    def slots(w):
        return _dedup_slots(np.concatenate(srcs[w]) if srcs[w] else
                            np.zeros(0, np.int64))

    cur = [slots(w) for w in range(len(bins))]
    for _ in range(iters):
        hot = int(np.argmax(cur))
        order_cold = np.argsort(cur)
        moved = False
        for cold in order_cold:
            if cold == hot or len(bins[cold]) >= cap:
                continue
            # move the member with the smallest segment out of `hot`
            j = int(np.argmin([len(s) for s in srcs[hot]]))
            bins[cold].append(bins[hot].pop(j))
            srcs[cold].append(srcs[hot].pop(j))
            new_hot, new_cold = slots(hot), slots(cold)
            if max(new_hot, new_cold) >= cur[hot]:
                # revert: no improvement
                bins[hot].append(bins[cold].pop())
                srcs[hot].append(srcs[cold].pop())
                continue
            cur[hot], cur[cold] = new_hot, new_cold
            moved = True
            break
        if not moved:
            break
    return bins


def make_plan(v_idx, e_idx, e2v_weight, nv=NV, ne=NE, ch=CH, ncores=NCORES):
    P = Plan()
    P.nv, P.ne, P.ch, P.ncores = nv, ne, ch, ncores
    epc, vpc = ne // ncores, nv // ncores
    P.epc, P.vpc = epc, vpc

    order_e, deg_e, starts_e = _csr(e_idx, ne)
    order_v, deg_v, starts_v = _csr(v_idx, nv)
    inv_deg = np.zeros(ne, np.float32)
    nz = deg_e > 0
    inv_deg[nz] = (np.float32(1.0) / deg_e[nz].astype(np.float32))

    nb1 = -(-epc // 128)
    nb2 = -(-vpc // 128)
    v_of_inc = v_idx.astype(np.int64)
    e_of_inc = e_idx.astype(np.int64)
    # balance destinations across cores globally (assignment is free — pos /
    # vmap carry it), then binpack windows within each core and rebalance for
    # post-dedup slot counts.
    cores_e, _ = _binpack(np.arange(ne), deg_e, ncores, cap=nb1 * 128)
    bins1 = []
    for k in range(ncores):
        eids = np.asarray(cores_e[k])
        b, _ = _binpack(eids, deg_e[eids], nb1)
        bins1.append(_repair_bins(b, order_e, starts_e, v_of_inc))

    # p2 window count: an extra window can admit a smaller W2 (less gather
    # padding) once dedup shrinks the per-window loads — pick the best.
    best = None
    for nb2c in (nb2, nb2 + 1):
        cores_v, _ = _binpack(np.arange(nv), deg_v, ncores, cap=nb2c * 128)
        cand = []
        wmax = 0
        for k in range(ncores):
            vids = np.asarray(cores_v[k])
            b, _ = _binpack(vids, deg_v[vids], nb2c)
            b = _repair_bins(b, order_v, starts_v, e_of_inc)
            cand.append(b)
            for bb in b:
                wmax = max(wmax, _dedup_slots(np.concatenate(
                    [e_of_inc[order_v[starts_v[d]:starts_v[d + 1]]]
                     for d in bb]) if bb else np.zeros(0, np.int64)))
        W2c = -(-wmax // 128)
        if best is None or nb2c * W2c < best[0] * best[1]:
            best = (nb2c, W2c, cand)
    nb2, _, bins2 = best
    P.NW1, P.NW2 = nb1, nb2

    # phase-1 windows (dedup within window) + edge position map.  Positions
    # are window-major (w, core, row) so each window's table slice can be
    # AllGathered independently as soon as its GEMM finishes.
    pos = np.zeros(ne, np.int64)
    wins1 = []
    w1max = 0
    for k in range(ncores):
        wins, wmax = _phase_windows(
            bins1[k], order_e, starts_e, v_idx.astype(np.int64),
            inv_deg[e_idx.astype(np.int64)])
        wins1.append(wins)
        w1max = max(w1max, wmax)
        for w, bin_ids in enumerate(bins1[k]):
            for j, e in enumerate(bin_ids):
                if AG_SLICED:   # window-major: (w, core, row)
                    pos[e] = (w * ncores + k) * 128 + j
                else:           # core-major: AllGather rank concatenation
                    pos[e] = (k * nb1 + w) * 128 + j
    assert pos.max() < 32768
    P.W1 = -(-w1max // 128)
    P.C1 = P.NW1 * P.W1
    P.p1 = [_layout(wins, P.W1, P.NW1) for wins in wins1]

    # phase-2 windows + output row map. Softmax weights are fully normalized
    # on the host (exp / per-vertex sum), so the kernel needs no denominator
    # pass.
    expw = np.exp(e2v_weight.astype(np.float64))
    den = np.zeros(nv, np.float64)
    np.add.at(den, v_idx, expw)
    wnorm = (expw / den[v_idx.astype(np.int64)]).astype(np.float32)
    wins2 = []
    w2max = 0
    P.vmap = []
    for k in range(ncores):
        wins, wmax = _phase_windows(
            bins2[k], order_v, starts_v, pos[e_idx.astype(np.int64)], wnorm,
            piece_order=AG_SLICED)
        wins2.append(wins)
        w2max = max(w2max, wmax)
        vm = np.full(P.NW2 * 128, -1, np.int64)
        for w, bin_ids in enumerate(bins2[k]):
            vm[w * 128:w * 128 + len(bin_ids)] = bin_ids
        # deg-0 vertices never receive contributions; drop them from the
        # output map so any on-chip garbage (e.g. a stray +b) is discarded.
        vme = vm[vm >= 0]
        vm[vm >= 0] = np.where(deg_v[vme] > 0, vme, -1)
        P.vmap.append(vm)
    P.W2 = -(-w2max // 128)
    P.C2 = P.NW2 * P.W2
    P.p2 = [_layout(wins, P.W2, P.NW2, cw_major=CW_MAJOR) for wins in wins2]
    return P


# ---------------------------------------------------------------- builder ---
def build_nc(P, bf16=GATHER_BF16, spmd=True, reps=1, grp=GRP, gbufs=5,
             nqueues=1, p1_dt=P1_DT, p2_dt=P2_DT, out_dt=OUT_DT, fuse=FUSE):
    import concourse.bacc as bacc
    import concourse.mybir as mybir
    import concourse.tile as tile

    f32 = mybir.dt.float32
    dt_g = mybir.dt.bfloat16 if bf16 else f32   # sel matrices + GEMM operands
    dt_p1 = _mydt(mybir, p1_dt)
    dt_p2 = _mydt(mybir, p2_dt)
    dt_out = _mydt(mybir, out_dt)
    eq, mul, mx, add = (mybir.AluOpType.is_equal, mybir.AluOpType.mult,
                        mybir.AluOpType.max, mybir.AluOpType.add)
    ch, KT = P.ch, P.ch // 128
    post = fuse == "post"

    nc = bacc.Bacc("TRN2", target_bir_lowering=False, debug=False,
                   num_devices=P.ncores if spmd else 1,
                   num_swdge_queues=nqueues)

    XT = nc.dram_tensor("xt", [P.nv, ch], dt_p1, kind="ExternalInput")
    WT = nc.dram_tensor("wt", [128, KT, ch], dt_g, kind="ExternalInput")
    BT = nc.dram_tensor("bt", [1, ch], dt_g, kind="ExternalInput")
    IOTA = nc.dram_tensor("iota", [128, 128], dt_g, kind="ExternalInput")
    IDENT = nc.dram_tensor("ident", [128, 128], dt_g, kind="ExternalInput")
    G1IDX = nc.dram_tensor("g1idx", [128, P.C1 * 8], mybir.dt.int16, kind="ExternalInput")
    P1TAB = nc.dram_tensor("p1tab", [128, 4, P.C1], f32, kind="ExternalInput")
    G2IDX = nc.dram_tensor("g2idx", [128, P.C2 * 8], mybir.dt.int16, kind="ExternalInput")
    P2TAB = nc.dram_tensor("p2tab", [128, 4, P.C2], f32, kind="ExternalInput")

    ner1 = P.NW1 * 128
    CCIN = nc.dram_tensor("ccin", [ner1, ch], dt_p2)
    CCOUT = nc.dram_tensor("ccout", [P.ncores * ner1, ch], dt_p2, addr_space="Shared")
    OUT = nc.dram_tensor("out", [P.NW2 * 128, ch], dt_out, kind="ExternalOutput")

    with tile.TileContext(nc) as tc, ExitStack() as ctx:
        const = ctx.enter_context(tc.tile_pool(name="const", bufs=1))
        gpool = ctx.enter_context(tc.tile_pool(name="g", bufs=gbufs))
        g2pool = ctx.enter_context(tc.tile_pool(name="g2", bufs=gbufs))
        prep = ctx.enter_context(tc.tile_pool(name="prep", bufs=1))
        selp = ctx.enter_context(tc.tile_pool(name="selp", bufs=8))
        psum = ctx.enter_context(tc.tile_pool(name="ps", bufs=2, space="PSUM"))
        sbp = ctx.enter_context(tc.tile_pool(name="sbp", bufs=2))
        pre_tiles = {}

        def cload(dram, shape, dt, tag, eng=None):
            t = const.tile(shape, dt, tag=tag)
            (eng or nc.sync).dma_start(t[:], dram[:])
            return t

        # p1-critical tables first (SP ring); the rest go on the ACT ring so
        # they never delay the first gather.
        iota_t = cload(IOTA, [128, 128], dt_g, "iota")
        g1idx_t = cload(G1IDX, [128, P.C1 * 8], mybir.dt.int16, "g1idx")
        p1tab_t = cload(P1TAB, [128, 4, P.C1], f32, "p1tab")
        eloc1_t, wsel1_t = p1tab_t[:, 0, :], p1tab_t[:, 1, :]
        eloc1b_t, wsel1b_t = p1tab_t[:, 2, :], p1tab_t[:, 3, :]
        wt_t = cload(WT, [128, KT, ch], dt_g, "wt", eng=nc.scalar)
        bt_t = cload(BT, [1, ch], dt_g, "bt", eng=nc.scalar)
        ident_t = cload(IDENT, [128, 128], dt_g, "ident", eng=nc.scalar)
        g2idx_t = cload(G2IDX, [128, P.C2 * 8], mybir.dt.int16, "g2idx",
                        eng=nc.scalar)
        p2tab_t = cload(P2TAB, [128, 4, P.C2], f32, "p2tab", eng=nc.scalar)
        vloc2_t, w2raw_t = p2tab_t[:, 0, :], p2tab_t[:, 1, :]
        vloc2b_t, w2rawb_t = p2tab_t[:, 2, :], p2tab_t[:, 3, :]
        ones1_t = const.tile([1, 128], dt_g, tag="ones1")
        nc.vector.memset(ones1_t[:], 1.0)

        # ---------------- gather + one/two-hot reduce ----------------------
        def agg_phase(src_ap, gidx_t, loc_t, w_t, locb_t, wb_t, is2, C, W,
                      gtag, chunk_cb, win_cb, dt_tab, src_sel=None,
                      pre_tiles=None, cw_major=False, pool=None):
            """Consumption iterates (window, cw); table column t of a chunk is
            cw*NW + w when cw_major else the position itself.  Gather calls
            cover consecutive table columns; with cw_major a call spans the
            cw-th chunks of `grp` windows, so its source-prefix (src_sel)
            dependency stays low for early cw ranks."""
            pre_tiles = pre_tiles or {}
            pool = pool or gpool
            NW = C // W
            tcol = (lambda w, cw: cw * NW + w) if cw_major \
                else (lambda w, cw: w * W + cw)
            # calls = runs of consecutive non-prefetched table columns, never
            # crossing a cw-group boundary in cw_major mode
            calls = []
            run = []
            bounds = set(cw * NW for cw in range(W)) if cw_major else set()
            for t in range(C):
                if t in pre_tiles or len(run) == grp or (run and t in bounds):
                    if run:
                        calls.append((run[0], len(run)))
                    run = []
                if t not in pre_tiles:
                    run.append(t)
            if run:
                calls.append((run[0], len(run)))
            call_of = {}
            for g0, n in calls:
                for j in range(n):
                    call_of[g0 + j] = (g0, n)
            # issue each call right before its first-consumed chunk
            first_use = {}
            for g0, n in calls:
                p0 = min((t % NW) * W + t // NW if cw_major else t
                         for t in range(g0, g0 + n))
                first_use.setdefault(p0, []).append((g0, n))
            tiles = {}
            pw = None
            for p in range(C):
                for g0, n in first_use.get(p, []):
                    gt_new = pool.tile([128, grp, ch], dt_tab, tag=gtag)
                    src = src_sel(g0, n) if src_sel is not None else src_ap
                    nc.gpsimd.dma_gather(
                        gt_new[:, 0:n, :], src,
                        gidx_t[:, g0 * 8:(g0 + n) * 8],
                        n * 128, n * 128, ch,
                        queue_num=(g0 // grp) % nqueues)
                    tiles[g0] = gt_new
                w, cw = divmod(p, W)
                t = tcol(w, cw)
                if t in pre_tiles:
                    gt, j = pre_tiles[t]
                else:
                    g0, n = call_of[t]
                    gt, j = tiles[g0], t - g0
                sel = selp.tile([128, 128], dt_g, tag="sel")
                nc.vector.tensor_scalar(
                    sel[:], iota_t[:], loc_t[:, t:t + 1], w_t[:, t:t + 1],
                    op0=eq, op1=mul)
                if is2[t]:  # dedup chunk: add the second hot
                    selb = selp.tile([128, 128], dt_g, tag="selb")
                    nc.vector.tensor_scalar(
                        selb[:], iota_t[:], locb_t[:, t:t + 1],
                        wb_t[:, t:t + 1], op0=eq, op1=mul)
                    sel2 = selp.tile([128, 128], dt_g, tag="sel2")
                    nc.vector.tensor_tensor(sel2[:], sel[:], selb[:], op=add)
                    sel = sel2
                if cw == 0:
                    pw = psum.tile([128, ch], f32, tag="win")
                chunk_cb(pw, sel, gt, j, w, cw, cw == W - 1)
                if cw == W - 1:
                    win_cb(pw, w)

        def p1_chunk(pw, sel, gt, j, w, cw, last):
            nc.tensor.matmul(pw[:], sel[:], gt[:, j, :],
                             start=(cw == 0), stop=last)

        def gemm_bias(src_t, dst_psum):
            """dst[v/e, co] = src^T blocks @ W.T + 1^T b (K=1 bias matmul)."""
            for k in range(KT):
                nc.tensor.matmul(dst_psum[:], src_t[:, k, :], wt_t[:, k, :],
                                 start=(k == 0), stop=False)
            nc.tensor.matmul(dst_psum[:], ones1_t[:], bt_t[:],
                             start=False, stop=True)

        def transpose_blocks(pw, tag):
            """psum [128, ch] f32 -> sbuf [128, KT, 128] dt_g transposed.
            Copies are per-128-block so transpose k pipelines with copy k+1."""
            tT_w = sbp.tile([128, KT, 128], dt_g, tag=tag + "T", name=tag + "T")
            for k in range(KT):
                twk = sbp.tile([128, 128], dt_g, tag=tag + "f", name=tag + "f")
                nc.vector.tensor_copy(twk[:], pw[:, k * 128:(k + 1) * 128])
                pt = psum.tile([128, 128], dt_g, tag="aux", name="pt")
                nc.tensor.transpose(pt[:], twk[:], ident_t[:])
                nc.vector.tensor_copy(tT_w[:, k, :], pt[:])
            return tT_w

        def p1_win(pw, w):
            # window w's edge rows are complete: ship its CCIN slice and
            # immediately AllGather that window's table piece, overlapping
            # the collective with the remaining p1 windows.
            xep = sbp.tile([128, ch], dt_p2, tag="xep", name="xep")
            if post:
                # raw table, scaled for fp8 range; GEMM happens after p2 agg
                nc.vector.tensor_scalar(xep[:], pw[:], float(P2_SCALE), None,
                                        op0=mul)
            else:
                xeT_w = transpose_blocks(pw, "xe")
                pg = psum.tile([128, ch], f32, tag="gemm", name="pg")
                gemm_bias(xeT_w, pg)
                nc.vector.tensor_copy(xep[:], pg[:])
            nc.sync.dma_start(CCIN[w * 128:(w + 1) * 128, :], xep[:])
            if AG_SLICED:
                o = w * P.ncores * 128
                if spmd:
                    nc.gpsimd.collective_compute(
                        "AllGather", mybir.AluOpType.bypass,
                        replica_groups=[list(range(P.ncores))],
                        ins=[CCIN[w * 128:(w + 1) * 128, :]],
                        outs=[CCOUT[o:o + P.ncores * 128, :]])
                else:  # single-core stand-in for the window AllGather
                    nc.sync.dma_start(CCOUT[o:o + 128, :],
                                      CCIN[w * 128:(w + 1) * 128, :])
            elif w == P.NW1 - 1:
                if spmd:
                    nc.gpsimd.collective_compute(
                        "AllGather", mybir.AluOpType.bypass,
                        replica_groups=[list(range(P.ncores))],
                        ins=[CCIN[:]], outs=[CCOUT[:]])
                else:
                    nc.sync.dma_start(CCOUT[0:ner1, :], CCIN[:])
            # prefetch p2 chunks whose table pieces are already gathered,
            # soaking p1's spare DMA bandwidth
            for c in pre_sched.get(w, []):
                gt = prep.tile([128, 1, ch], dt_p2, tag=f"pre{c}")
                nc.gpsimd.dma_gather(
                    gt[:], p2_src(c, 1), g2idx_t[:, c * 8:(c + 1) * 8],
                    128, 128, ch, queue_num=0)
                pre_tiles[c] = (gt, 0)

        def p2_chunk(pw, sel, gt, j, w, cw, last):
            nc.tensor.matmul(pw[:], sel[:], gt[:, j, :],
                             start=(cw == 0), stop=last)

        def p2_win(pw, w):
            if post:
                awT = transpose_blocks(pw, "aw")
                po = psum.tile([128, ch], f32, tag="gemm", name="po")
                gemm_bias(awT, po)
                pw = po
            # weights pre-normalized on host: just relu + store
            ow = sbp.tile([128, ch], dt_out, tag="ow", name="ow")
            nc.vector.tensor_scalar(ow[:], pw[:], 1.0, 0.0, op0=mul, op1=mx)
            nc.sync.dma_start(OUT[w * 128:(w + 1) * 128, :], ow[:])

        # chunks that contain any dedup pair need the second sel pass; the
        # union over cores keeps the SPMD program identical on every core.
        is2_1 = np.zeros(P.C1, bool)
        is2_2 = np.zeros(P.C2, bool)
        maxrow2 = np.zeros(P.C2, np.int64)
        for k in range(P.ncores):
            is2_1 |= (_pack(P.p1[k][3], P.C1) >= 0).any(axis=0)
            is2_2 |= (_pack(P.p2[k][3], P.C2) >= 0).any(axis=0)
            maxrow2 = np.maximum(
                maxrow2, _pack(P.p2[k][0], P.C2).astype(np.int64).max(axis=0))

        piece = P.ncores * 128
        def p2_src(g0, n):
            # prefix slice of the edge table covering every row this gather
            # call touches, so it only waits on the AllGather pieces it needs
            pieces = int(maxrow2[g0:g0 + n].max()) // piece + 1
            if not AG_SLICED or pieces >= P.NW1:
                return CCOUT[:]
            return CCOUT[0:pieces * piece, :]

        # p2 prefetch schedule: at p1 window boundary w we may issue gathers
        # for p2 chunks that only need table pieces < w (their AllGather was
        # triggered a full window earlier).  Earliest-consumed chunks first.
        bound = (maxrow2 // piece + 1).astype(int)   # pieces needed per chunk
        pre_sched = {w: [] for w in range(1, P.NW1)}
        if AG_SLICED and PRE_PER_B > 0:
            taken = set()
            for w in range(1, P.NW1):
                for c in range(P.C2):
                    if len(pre_sched[w]) >= PRE_PER_B:
                        break
                    if c not in taken and bound[c] <= w:
                        pre_sched[w].append(c)
                        taken.add(c)

        for _rep in range(reps):
            pre_tiles.clear()
            agg_phase(XT[:], g1idx_t, eloc1_t, wsel1_t, eloc1b_t, wsel1b_t,
                      is2_1, P.C1, P.W1, "g1", p1_chunk, p1_win, dt_p1)

            # phase 2: e2v aggregation (sel weights pre-normalized on host)
            agg_phase(CCOUT[:], g2idx_t, vloc2_t, w2raw_t, vloc2b_t, w2rawb_t,
                      is2_2, P.C2, P.W2, "g2", p2_chunk, p2_win, dt_p2,
                      src_sel=p2_src, pre_tiles=pre_tiles,
                      cw_major=CW_MAJOR, pool=g2pool)

    nc.compile()
    return nc


# ------------------------------------------------------------------ runner ---
def make_in_maps(P, X, W, b, bf16=GATHER_BF16, p1_dt=P1_DT, fuse=FUSE):
    npdt = ml_dtypes.bfloat16 if bf16 else np.float32
    np_p1 = _npdt(p1_dt)
    s1 = P1_SCALE if p1_dt == "f8" else 1.0
    s2 = P2_SCALE if fuse == "post" else 1.0
    KT = P.ch // 128
    xt = np.ascontiguousarray((X * s1).astype(np_p1))
    wt = np.ascontiguousarray(
        W.T.reshape(KT, 128, P.ch).transpose(1, 0, 2).astype(npdt))
    bt = np.ascontiguousarray(b.astype(npdt).reshape(1, P.ch))
    iota = np.ascontiguousarray(
        np.broadcast_to(np.arange(128, dtype=npdt), (128, 128)))
    ident = np.eye(128, dtype=npdt)

    def tb(flat, C, s=1.0):
        return _pack(flat, C) / np.float32(s)

    in_maps = []
    for k in range(P.ncores):
        g1, l1, w1, l1b, w1b = P.p1[k]
        g2, l2, w2, l2b, w2b = P.p2[k]
        p1tab = np.ascontiguousarray(np.stack(
            [tb(l1, P.C1), tb(w1, P.C1, s1), tb(l1b, P.C1), tb(w1b, P.C1, s1)],
            axis=1))
        p2tab = np.ascontiguousarray(np.stack(
            [tb(l2, P.C2), tb(w2, P.C2, s2), tb(l2b, P.C2), tb(w2b, P.C2, s2)],
            axis=1))
        in_maps.append({
            "xt": xt, "wt": wt, "bt": bt, "iota": iota, "ident": ident,
            "g1idx": _wrap_idx(g1), "p1tab": p1tab,
            "g2idx": _wrap_idx(g2), "p2tab": p2tab,
        })
    return in_maps


def assemble(P, shards):
    out = np.zeros((P.nv, P.ch), np.float32)
    for k in range(P.ncores):
        vm = P.vmap[k]
        m = vm >= 0
        out[vm[m]] = shards[k][m].astype(np.float32)
    return out


_nc_cache = {}


def kernel(X, W, b, e2v_weight, v_idx, e_idx):
    global _last_results
    from concourse.bass_utils import run_bass_kernel_spmd

    P = make_plan(v_idx, e_idx, e2v_weight)
    key = (P.C1, P.C2, P.W1, P.W2, GATHER_BF16, P1_DT, P2_DT, OUT_DT, FUSE,
           AG_SLICED)
    if key not in _nc_cache:
        _nc_cache[key] = build_nc(P)
    nc = _nc_cache[key]
    in_maps = make_in_maps(P, X, W, b)
    res = run_bass_kernel_spmd(nc, in_maps, list(range(P.ncores)), trace=TRACE)
    _last_results = res
    shards = [res.results[k]["out"] for k in range(P.ncores)]
    return assemble(P, shards)



# revision 89
# speedup vs baseline: 1.0248x; 1.0155x over previous
"""HGNNPConv Trainium2 kernel (8 NeuronCores, SPMD).

Math (equivalent reformulation of the reference):
  Xe_raw[e] = mean_{i: e_idx[i]=e} X[v_idx[i]]              (v2e, softmax of ones = 1/deg)
  Xe_p      = Xe_raw @ W.T + b                              (GEMM on 4000 edges, not 20000 verts)
  Xv[v]     = sum_i wn_i * Xe_p[e_idx[i]],  wn_i = exp(w_i)/sum_{v} exp(w)
              (wn precomputed on host -> no on-chip denominator pass)
  out       = relu(Xv)
Deg-0 vertices are masked host-side in assemble(); empty edges get a spurious
+b in Xe_p but are never referenced downstream.

Sharding: edges and vertices are binpacked across the 8 cores (balancing
post-dedup gather slots).  Phase 1 aggregates by destination edge, runs the
edge-level GEMM per 128-edge window (bias folded in as a K=1 matmul), and
AllGathers each window's table slice as soon as it is ready (overlapping the
collective with the remaining phase-1 work).  Phase 2 aggregates by
destination vertex; each gather call's source AP is a prefix slice of the
table covering only the AllGather pieces it needs.

Per-destination-window weighted one-hot selection matrices (built on DVE
from a bf16 iota) reduce gathered rows on the PE into PSUM.  Incidences
sharing a source row within a window are deduplicated into one gathered slot
with a two-hot sel column (~11% fewer gather bytes; window binpacking is
rebalanced for post-dedup slot counts, and phase 2 uses 21 windows x 7
chunks instead of 20 x 8).

Dtypes: phase-1 gather table fp8-e3m4 (X pre-scaled by 2: worst-case quant
error halves vs e4m3 and the GEMM averages it out), phase-2 table bf16
(absmax-norm tolerance rules out fp8 post-GEMM), output bf16 (upcast on
host).  End-to-end rel err 1.35e-2 vs the 2e-2 gate.
"""

import os
from contextlib import ExitStack

import numpy as np
import ml_dtypes

# ---------------------------------------------------------------- config ---
NCORES = 8
NV, NE, NNZ, CH = 20000, 4000, 160000, 512
GATHER_BF16 = os.environ.get("KERNEL_F32", "") == ""  # bf16 tables+matmuls by default
P1_DT = os.environ.get("KERNEL_P1_DT", "f8")   # phase-1 gather table dtype
P2_DT = os.environ.get("KERNEL_P2_DT", "bf16")  # phase-2 gather table dtype
OUT_DT = os.environ.get("KERNEL_OUT_DT", "bf16")
FUSE = os.environ.get("KERNEL_FUSE", "pre")    # "post": GEMM after p2 agg
P1_SCALE = 2.0   # X pre-scale for fp8-e3m4 range use (exact power of 2)
P2_SCALE = 8.0   # Xe_raw pre-scale for the fp8 edge table (post mode)
GRP = 5          # gather chunks (of 128 idxs) per dma_gather call
AG_SLICED = os.environ.get("KERNEL_AG_SLICED", "1") != ""  # per-window AllGather
PRE_PER_B = int(os.environ.get("KERNEL_PRE_PER_B", "0"))  # p2 prefetches per p1 window
CW_MAJOR = os.environ.get("KERNEL_CW_MAJOR", "") != ""  # cw-major p2 table layout
TRACE = os.environ.get("BASS_TRACE", "") != ""


def _mydt(mybir, name):
    return {"f8": mybir.dt.float8e3, "bf16": mybir.dt.bfloat16,
            "f32": mybir.dt.float32}[name]


def _npdt(name):
    return {"f8": ml_dtypes.float8_e3m4, "bf16": ml_dtypes.bfloat16,
            "f32": np.float32}[name]

_last_results = None   # BassKernelResults of the most recent run (for test.py)


# ------------------------------------------------------------------- plan ---
class Plan:
    pass


def _binpack(ids, degs, nbins, cap=128):
    """Pack `ids` into `nbins` bins of <=cap items, balancing sum(degs)."""
    import heapq

    order = np.argsort(-degs, kind="stable")
    bins = [[] for _ in range(nbins)]
    loads = [0] * nbins
    heap = [(0, b) for b in range(nbins)]
    heapq.heapify(heap)
    for t in order:
        popped = []
        while True:
            load, b = heapq.heappop(heap)
            if len(bins[b]) < cap:
                break
            popped.append((load, b))
        for p in popped:
            heapq.heappush(heap, p)
        bins[b].append(int(ids[t]))
        loads[b] = load + int(degs[t])
        heapq.heappush(heap, (loads[b], b))
    return bins, loads


def _csr(idx, n):
    order = np.argsort(idx, kind="stable").astype(np.int64)
    deg = np.bincount(idx, minlength=n).astype(np.int64)
    starts = np.zeros(n + 1, np.int64)
    np.cumsum(deg, out=starts[1:])
    return order, deg, starts


def _pair_window(src, locs, ws):
    """Greedy within-window dedup: incidences sharing a source row become one
    gathered slot with two (loc, w) hots.  Returns (src', loc_a, w_a, loc_b,
    w_b) with pair slots FIRST; singles have loc_b = -1 / w_b = 0."""
    order = np.argsort(src, kind="stable")
    src, locs, ws = src[order], locs[order], ws[order]
    pa, pb, sg = [], [], []
    i, n = 0, len(src)
    while i < n:
        j = i
        while j < n and src[j] == src[i]:
            j += 1
        k = i
        while k + 1 < j:
            pa.append(k); pb.append(k + 1); k += 2
        if k < j:
            sg.append(k)
        i = j
    pa, pb, sg = np.array(pa, np.int64), np.array(pb, np.int64), np.array(sg, np.int64)
    src2 = np.concatenate([src[pa], src[sg]]) if len(pa) else src[sg]
    la = np.concatenate([locs[pa], locs[sg]]) if len(pa) else locs[sg]
    wa = np.concatenate([ws[pa], ws[sg]]) if len(pa) else ws[sg]
    lb = np.concatenate([locs[pb], np.full(len(sg), -1.0, locs.dtype)]) if len(pa) \
        else np.full(len(sg), -1.0, locs.dtype)
    wb = np.concatenate([ws[pb], np.zeros(len(sg), ws.dtype)]) if len(pa) \
        else np.zeros(len(sg), ws.dtype)
    return src2, la, wa, lb, wb, len(pa)


def _phase_windows(bins_per_core, order, starts, idx_of_inc, w_of_inc,
                   loc_dtype=np.float32, pair=True, piece_order=False):
    """Per-window slot lists for one core of one phase, after source dedup.

    Returns (wins, wmax): wins[w] = (src, loc_a, w_a, loc_b, w_b).  By
    default pair slots (loc_b >= 0) come first within each window; with
    piece_order=True, singles come first sorted by source id (so early
    chunks only reference low table pieces) and pair slots go last.
    """
    wins = []
    wmax = 0
    for bin_ids in bins_per_core:
        incs = []
        locs = []
        for j, d in enumerate(bin_ids):
            seg = order[starts[d]:starts[d + 1]]
            incs.append(seg)
            locs.append(np.full(len(seg), j, loc_dtype))
        incs = np.concatenate(incs) if incs else np.zeros(0, np.int64)
        locs = np.concatenate(locs) if locs else np.zeros(0, loc_dtype)
        src = idx_of_inc[incs]
        ws = w_of_inc[incs].astype(np.float32)
        if pair and len(src):
            src, la, wa, lb, wb, npair = _pair_window(src, locs, ws)
            if piece_order:
                ps = np.argsort(src[:npair], kind="stable")
                ss = npair + np.argsort(src[npair:], kind="stable")
                perm = np.concatenate([ss, ps])
                src, la, wa, lb, wb = (src[perm], la[perm], wa[perm],
                                       lb[perm], wb[perm])
        else:
            la, wa = locs, ws
            lb = np.full(len(src), -1.0, loc_dtype)
            wb = np.zeros(len(src), np.float32)
        wins.append((src, la, wa, lb, wb))
        wmax = max(wmax, len(src))
    return wins, wmax


def _layout(wins, W, nw, cw_major=False):
    """Flat slot arrays.  Table column of window w's cw-th chunk is
    cw*nw + w when cw_major (so the cw-th chunks of all windows are a
    consecutive gather range) else w*W + cw."""
    L = nw * W * 128
    gidx = np.zeros(L, np.int16)
    loc = np.full(L, -1.0, np.float32)
    wsel = np.zeros(L, np.float32)
    locb = np.full(L, -1.0, np.float32)
    wselb = np.zeros(L, np.float32)
    for w, (src, la, wa, lb, wb) in enumerate(wins):
        for cw in range(-(-len(src) // 128) if len(src) else 0):
            t = cw * nw + w if cw_major else w * W + cw
            i0, i1 = cw * 128, min((cw + 1) * 128, len(src))
            o = t * 128
            n = i1 - i0
            gidx[o:o + n] = src[i0:i1]
            loc[o:o + n] = la[i0:i1]
            wsel[o:o + n] = wa[i0:i1]
            locb[o:o + n] = lb[i0:i1]
            wselb[o:o + n] = wb[i0:i1]
    return gidx, loc, wsel, locb, wselb


def _wrap_idx(flat):
    """int16 flat[i] -> [128, len/16] with value i at [i%16, i//16], replicated."""
    a = flat.reshape(-1, 16).T  # [16, L/16]
    return np.ascontiguousarray(np.tile(a, (8, 1)))


def _pack(flat, C):
    """flat[c*128+p] -> [128, C]"""
    return np.ascontiguousarray(flat.reshape(C, 128).T)


def _dedup_slots(srcs):
    """#gather slots for a window's source list after pairing."""
    if not len(srcs):
        return 0
    _, cnt = np.unique(srcs, return_counts=True)
    return int(((cnt + 1) // 2).sum())


def _repair_bins(bins, order, starts, idx_of_inc, cap=128, iters=400):
    """Greedy rebalance: move members out of the window with the most
    post-dedup slots into the one with the fewest (respecting the member
    cap), to minimize max slots per window."""
    srcs = [
        [idx_of_inc[order[starts[d]:starts[d + 1]]] for d in b] for b in bins
    ]

    def slots(w):
        return _dedup_slots(np.concatenate(srcs[w]) if srcs[w] else
                            np.zeros(0, np.int64))

    cur = [slots(w) for w in range(len(bins))]
    for _ in range(iters):
        hot = int(np.argmax(cur))
        order_cold = np.argsort(cur)
        moved = False
        for cold in order_cold:
            if cold == hot or len(bins[cold]) >= cap:
                continue
            # move the member with the smallest segment out of `hot`
            j = int(np.argmin([len(s) for s in srcs[hot]]))
            bins[cold].append(bins[hot].pop(j))
            srcs[cold].append(srcs[hot].pop(j))
            new_hot, new_cold = slots(hot), slots(cold)
            if max(new_hot, new_cold) >= cur[hot]:
                # revert: no improvement
                bins[hot].append(bins[cold].pop())
                srcs[hot].append(srcs[cold].pop())
                continue
            cur[hot], cur[cold] = new_hot, new_cold
            moved = True
            break
        if not moved:
            break
    return bins


def make_plan(v_idx, e_idx, e2v_weight, nv=NV, ne=NE, ch=CH, ncores=NCORES):
    P = Plan()
    P.nv, P.ne, P.ch, P.ncores = nv, ne, ch, ncores
    epc, vpc = ne // ncores, nv // ncores
    P.epc, P.vpc = epc, vpc

    order_e, deg_e, starts_e = _csr(e_idx, ne)
    order_v, deg_v, starts_v = _csr(v_idx, nv)
    inv_deg = np.zeros(ne, np.float32)
    nz = deg_e > 0
    inv_deg[nz] = (np.float32(1.0) / deg_e[nz].astype(np.float32))

    nb1 = -(-epc // 128)
    nb2 = -(-vpc // 128)
    v_of_inc = v_idx.astype(np.int64)
    e_of_inc = e_idx.astype(np.int64)
    # balance destinations across cores globally (assignment is free — pos /
    # vmap carry it), then binpack windows within each core and rebalance for
    # post-dedup slot counts.
    cores_e, _ = _binpack(np.arange(ne), deg_e, ncores, cap=nb1 * 128)
    bins1 = []
    for k in range(ncores):
        eids = np.asarray(cores_e[k])
        b, _ = _binpack(eids, deg_e[eids], nb1)
        bins1.append(_repair_bins(b, order_e, starts_e, v_of_inc))

    # p2 window count: an extra window can admit a smaller W2 (less gather
    # padding) once dedup shrinks the per-window loads — pick the best.
    best = None
    for nb2c in (nb2, nb2 + 1):
        cores_v, _ = _binpack(np.arange(nv), deg_v, ncores, cap=nb2c * 128)
        cand = []
        wmax = 0
        for k in range(ncores):
            vids = np.asarray(cores_v[k])
            b, _ = _binpack(vids, deg_v[vids], nb2c)
            b = _repair_bins(b, order_v, starts_v, e_of_inc)
            cand.append(b)
            for bb in b:
                wmax = max(wmax, _dedup_slots(np.concatenate(
                    [e_of_inc[order_v[starts_v[d]:starts_v[d + 1]]]
                     for d in bb]) if bb else np.zeros(0, np.int64)))
        W2c = -(-wmax // 128)
        if best is None or nb2c * W2c < best[0] * best[1]:
            best = (nb2c, W2c, cand)
    nb2, _, bins2 = best
    P.NW1, P.NW2 = nb1, nb2

    # phase-1 windows (dedup within window) + edge position map.  Positions
    # are window-major (w, core, row) so each window's table slice can be
    # AllGathered independently as soon as its GEMM finishes.
    pos = np.zeros(ne, np.int64)
    wins1 = []
    w1max = 0
    for k in range(ncores):
        wins, wmax = _phase_windows(
            bins1[k], order_e, starts_e, v_idx.astype(np.int64),
            inv_deg[e_idx.astype(np.int64)])
        wins1.append(wins)
        w1max = max(w1max, wmax)
        for w, bin_ids in enumerate(bins1[k]):
            for j, e in enumerate(bin_ids):
                if AG_SLICED:   # window-major: (w, core, row)
                    pos[e] = (w * ncores + k) * 128 + j
                else:           # core-major: AllGather rank concatenation
                    pos[e] = (k * nb1 + w) * 128 + j
    assert pos.max() < 32768
    P.W1 = -(-w1max // 128)
    P.C1 = P.NW1 * P.W1
    P.p1 = [_layout(wins, P.W1, P.NW1) for wins in wins1]

    # phase-2 windows + output row map. Softmax weights are fully normalized
    # on the host (exp / per-vertex sum), so the kernel needs no denominator
    # pass.
    expw = np.exp(e2v_weight.astype(np.float64))
    den = np.zeros(nv, np.float64)
    np.add.at(den, v_idx, expw)
    wnorm = (expw / den[v_idx.astype(np.int64)]).astype(np.float32)
    wins2 = []
    w2max = 0
    P.vmap = []
    for k in range(ncores):
        wins, wmax = _phase_windows(
            bins2[k], order_v, starts_v, pos[e_idx.astype(np.int64)], wnorm,
            piece_order=AG_SLICED)
        wins2.append(wins)
        w2max = max(w2max, wmax)
        vm = np.full(P.NW2 * 128, -1, np.int64)
        for w, bin_ids in enumerate(bins2[k]):
            vm[w * 128:w * 128 + len(bin_ids)] = bin_ids
        # deg-0 vertices never receive contributions; drop them from the
        # output map so any on-chip garbage (e.g. a stray +b) is discarded.
        vme = vm[vm >= 0]
        vm[vm >= 0] = np.where(deg_v[vme] > 0, vme, -1)
        P.vmap.append(vm)
    P.W2 = -(-w2max // 128)
    P.C2 = P.NW2 * P.W2
    P.p2 = [_layout(wins, P.W2, P.NW2, cw_major=CW_MAJOR) for wins in wins2]
    return P


# ---------------------------------------------------------------- builder ---
def build_nc(P, bf16=GATHER_BF16, spmd=True, reps=1, grp=GRP, gbufs=5,
             nqueues=1, p1_dt=P1_DT, p2_dt=P2_DT, out_dt=OUT_DT, fuse=FUSE):
    import concourse.bacc as bacc
    import concourse.mybir as mybir
    import concourse.tile as tile

    f32 = mybir.dt.float32
    dt_g = mybir.dt.bfloat16 if bf16 else f32   # sel matrices + GEMM operands
    dt_p1 = _mydt(mybir, p1_dt)
    dt_p2 = _mydt(mybir, p2_dt)
    dt_out = _mydt(mybir, out_dt)
    eq, mul, mx, add = (mybir.AluOpType.is_equal, mybir.AluOpType.mult,
                        mybir.AluOpType.max, mybir.AluOpType.add)
    ch, KT = P.ch, P.ch // 128
    post = fuse == "post"

    nc = bacc.Bacc("TRN2", target_bir_lowering=False, debug=False,
                   num_devices=P.ncores if spmd else 1,
                   num_swdge_queues=nqueues)

    XT = nc.dram_tensor("xt", [P.nv, ch], dt_p1, kind="ExternalInput")
    WT = nc.dram_tensor("wt", [128, KT, ch], dt_g, kind="ExternalInput")
    BT = nc.dram_tensor("bt", [1, ch], dt_g, kind="ExternalInput")
    IOTA = nc.dram_tensor("iota", [128, 128], dt_g, kind="ExternalInput")
    IDENT = nc.dram_tensor("ident", [128, 128], dt_g, kind="ExternalInput")
    G1IDX = nc.dram_tensor("g1idx", [128, P.C1 * 8], mybir.dt.int16, kind="ExternalInput")
    P1TAB = nc.dram_tensor("p1tab", [128, 4, P.C1], f32, kind="ExternalInput")
    G2IDX = nc.dram_tensor("g2idx", [128, P.C2 * 8], mybir.dt.int16, kind="ExternalInput")
    P2TAB = nc.dram_tensor("p2tab", [128, 4, P.C2], f32, kind="ExternalInput")

    ner1 = P.NW1 * 128
    CCIN = nc.dram_tensor("ccin", [ner1, ch], dt_p2)
    CCOUT = nc.dram_tensor("ccout", [P.ncores * ner1, ch], dt_p2, addr_space="Shared")
    OUT = nc.dram_tensor("out", [P.NW2 * 128, ch], dt_out, kind="ExternalOutput")

    with tile.TileContext(nc) as tc, ExitStack() as ctx:
        const = ctx.enter_context(tc.tile_pool(name="const", bufs=1))
        gpool = ctx.enter_context(tc.tile_pool(name="g", bufs=gbufs))
        g2pool = ctx.enter_context(tc.tile_pool(name="g2", bufs=gbufs))
        prep = ctx.enter_context(tc.tile_pool(name="prep", bufs=1))
        selp = ctx.enter_context(tc.tile_pool(name="selp", bufs=8))
        psum = ctx.enter_context(tc.tile_pool(name="ps", bufs=2, space="PSUM"))
        sbp = ctx.enter_context(tc.tile_pool(name="sbp", bufs=2))
        pre_tiles = {}

        def cload(dram, shape, dt, tag, eng=None):
            t = const.tile(shape, dt, tag=tag)
            (eng or nc.sync).dma_start(t[:], dram[:])
            return t

        # p1-critical tables first (SP ring); the rest go on the ACT ring so
        # they never delay the first gather.  g1idx leads: the first gather
        # needs only it, iota/p1tab are for the (later) first sel build.
        g1idx_t = cload(G1IDX, [128, P.C1 * 8], mybir.dt.int16, "g1idx")
        iota_t = cload(IOTA, [128, 128], dt_g, "iota")
        p1tab_t = cload(P1TAB, [128, 4, P.C1], f32, "p1tab")
        eloc1_t, wsel1_t = p1tab_t[:, 0, :], p1tab_t[:, 1, :]
        eloc1b_t, wsel1b_t = p1tab_t[:, 2, :], p1tab_t[:, 3, :]
        wt_t = cload(WT, [128, KT, ch], dt_g, "wt", eng=nc.scalar)
        bt_t = cload(BT, [1, ch], dt_g, "bt", eng=nc.scalar)
        ident_t = cload(IDENT, [128, 128], dt_g, "ident", eng=nc.scalar)
        g2idx_t = cload(G2IDX, [128, P.C2 * 8], mybir.dt.int16, "g2idx",
                        eng=nc.scalar)
        p2tab_t = cload(P2TAB, [128, 4, P.C2], f32, "p2tab", eng=nc.scalar)
        vloc2_t, w2raw_t = p2tab_t[:, 0, :], p2tab_t[:, 1, :]
        vloc2b_t, w2rawb_t = p2tab_t[:, 2, :], p2tab_t[:, 3, :]
        ones1_t = const.tile([1, 128], dt_g, tag="ones1")
        nc.vector.memset(ones1_t[:], 1.0)

        # ---------------- gather + one/two-hot reduce ----------------------
        def agg_phase(src_ap, gidx_t, loc_t, w_t, locb_t, wb_t, is2, C, W,
                      gtag, chunk_cb, win_cb, dt_tab, src_sel=None,
                      pre_tiles=None, cw_major=False, pool=None):
            """Consumption iterates (window, cw); table column t of a chunk is
            cw*NW + w when cw_major else the position itself.  Gather calls
            cover consecutive table columns; with cw_major a call spans the
            cw-th chunks of `grp` windows, so its source-prefix (src_sel)
            dependency stays low for early cw ranks."""
            pre_tiles = pre_tiles or {}
            pool = pool or gpool
            NW = C // W
            tcol = (lambda w, cw: cw * NW + w) if cw_major \
                else (lambda w, cw: w * W + cw)
            # calls = runs of consecutive non-prefetched table columns, never
            # crossing a cw-group boundary in cw_major mode
            calls = []
            run = []
            bounds = set(cw * NW for cw in range(W)) if cw_major else set()
            for t in range(C):
                if t in pre_tiles or len(run) == grp or (run and t in bounds):
                    if run:
                        calls.append((run[0], len(run)))
                    run = []
                if t not in pre_tiles:
                    run.append(t)
            if run:
                calls.append((run[0], len(run)))
            call_of = {}
            for g0, n in calls:
                for j in range(n):
                    call_of[g0 + j] = (g0, n)
            # issue each call right before its first-consumed chunk
            first_use = {}
            for g0, n in calls:
                p0 = min((t % NW) * W + t // NW if cw_major else t
                         for t in range(g0, g0 + n))
                first_use.setdefault(p0, []).append((g0, n))
            tiles = {}
            pw = None
            for p in range(C):
                for g0, n in first_use.get(p, []):
                    gt_new = pool.tile([128, grp, ch], dt_tab, tag=gtag)
                    src = src_sel(g0, n) if src_sel is not None else src_ap
                    nc.gpsimd.dma_gather(
                        gt_new[:, 0:n, :], src,
                        gidx_t[:, g0 * 8:(g0 + n) * 8],
                        n * 128, n * 128, ch,
                        queue_num=(g0 // grp) % nqueues)
                    tiles[g0] = gt_new
                w, cw = divmod(p, W)
                t = tcol(w, cw)
                if t in pre_tiles:
                    gt, j = pre_tiles[t]
                else:
                    g0, n = call_of[t]
                    gt, j = tiles[g0], t - g0
                sel = selp.tile([128, 128], dt_g, tag="sel")
                nc.vector.tensor_scalar(
                    sel[:], iota_t[:], loc_t[:, t:t + 1], w_t[:, t:t + 1],
                    op0=eq, op1=mul)
                if is2[t]:  # dedup chunk: add the second hot
                    selb = selp.tile([128, 128], dt_g, tag="selb")
                    nc.vector.tensor_scalar(
                        selb[:], iota_t[:], locb_t[:, t:t + 1],
                        wb_t[:, t:t + 1], op0=eq, op1=mul)
                    sel2 = selp.tile([128, 128], dt_g, tag="sel2")
                    nc.vector.tensor_tensor(sel2[:], sel[:], selb[:], op=add)
                    sel = sel2
                if cw == 0:
                    pw = psum.tile([128, ch], f32, tag="win")
                chunk_cb(pw, sel, gt, j, w, cw, cw == W - 1)
                if cw == W - 1:
                    win_cb(pw, w)

        def p1_chunk(pw, sel, gt, j, w, cw, last):
            nc.tensor.matmul(pw[:], sel[:], gt[:, j, :],
                             start=(cw == 0), stop=last)

        def gemm_bias(src_t, dst_psum):
            """dst[v/e, co] = src^T blocks @ W.T + 1^T b (K=1 bias matmul)."""
            for k in range(KT):
                nc.tensor.matmul(dst_psum[:], src_t[:, k, :], wt_t[:, k, :],
                                 start=(k == 0), stop=False)
            nc.tensor.matmul(dst_psum[:], ones1_t[:], bt_t[:],
                             start=False, stop=True)

        def transpose_blocks(pw, tag):
            """psum [128, ch] f32 -> sbuf [128, KT, 128] dt_g transposed.
            Copies are per-128-block so transpose k pipelines with copy k+1."""
            tT_w = sbp.tile([128, KT, 128], dt_g, tag=tag + "T", name=tag + "T")
            for k in range(KT):
                twk = sbp.tile([128, 128], dt_g, tag=tag + "f", name=tag + "f")
                nc.vector.tensor_copy(twk[:], pw[:, k * 128:(k + 1) * 128])
                pt = psum.tile([128, 128], dt_g, tag="aux", name="pt")
                nc.tensor.transpose(pt[:], twk[:], ident_t[:])
                nc.vector.tensor_copy(tT_w[:, k, :], pt[:])
            return tT_w

        def p1_win(pw, w):
            # window w's edge rows are complete: ship its CCIN slice and
            # immediately AllGather that window's table piece, overlapping
            # the collective with the remaining p1 windows.
            xep = sbp.tile([128, ch], dt_p2, tag="xep", name="xep")
            if post:
                # raw table, scaled for fp8 range; GEMM happens after p2 agg
                nc.vector.tensor_scalar(xep[:], pw[:], float(P2_SCALE), None,
                                        op0=mul)
            else:
                xeT_w = transpose_blocks(pw, "xe")
                pg = psum.tile([128, ch], f32, tag="gemm", name="pg")
                gemm_bias(xeT_w, pg)
                nc.vector.tensor_copy(xep[:], pg[:])
            nc.sync.dma_start(CCIN[w * 128:(w + 1) * 128, :], xep[:])
            if AG_SLICED:
                o = w * P.ncores * 128
                if spmd:
                    nc.gpsimd.collective_compute(
                        "AllGather", mybir.AluOpType.bypass,
                        replica_groups=[list(range(P.ncores))],
                        ins=[CCIN[w * 128:(w + 1) * 128, :]],
                        outs=[CCOUT[o:o + P.ncores * 128, :]])
                else:  # single-core stand-in for the window AllGather
                    nc.sync.dma_start(CCOUT[o:o + 128, :],
                                      CCIN[w * 128:(w + 1) * 128, :])
            elif w == P.NW1 - 1:
                if spmd:
                    nc.gpsimd.collective_compute(
                        "AllGather", mybir.AluOpType.bypass,
                        replica_groups=[list(range(P.ncores))],
                        ins=[CCIN[:]], outs=[CCOUT[:]])
                else:
                    nc.sync.dma_start(CCOUT[0:ner1, :], CCIN[:])
            # prefetch p2 chunks whose table pieces are already gathered,
            # soaking p1's spare DMA bandwidth
            for c in pre_sched.get(w, []):
                gt = prep.tile([128, 1, ch], dt_p2, tag=f"pre{c}")
                nc.gpsimd.dma_gather(
                    gt[:], p2_src(c, 1), g2idx_t[:, c * 8:(c + 1) * 8],
                    128, 128, ch, queue_num=0)
                pre_tiles[c] = (gt, 0)

        def p2_chunk(pw, sel, gt, j, w, cw, last):
            nc.tensor.matmul(pw[:], sel[:], gt[:, j, :],
                             start=(cw == 0), stop=last)

        def p2_win(pw, w):
            if post:
                awT = transpose_blocks(pw, "aw")
                po = psum.tile([128, ch], f32, tag="gemm", name="po")
                gemm_bias(awT, po)
                pw = po
            # weights pre-normalized on host: just relu + store
            ow = sbp.tile([128, ch], dt_out, tag="ow", name="ow")
            nc.vector.tensor_scalar(ow[:], pw[:], 1.0, 0.0, op0=mul, op1=mx)
            nc.sync.dma_start(OUT[w * 128:(w + 1) * 128, :], ow[:])

        # chunks that contain any dedup pair need the second sel pass; the
        # union over cores keeps the SPMD program identical on every core.
        is2_1 = np.zeros(P.C1, bool)
        is2_2 = np.zeros(P.C2, bool)
        maxrow2 = np.zeros(P.C2, np.int64)
        for k in range(P.ncores):
            is2_1 |= (_pack(P.p1[k][3], P.C1) >= 0).any(axis=0)
            is2_2 |= (_pack(P.p2[k][3], P.C2) >= 0).any(axis=0)
            maxrow2 = np.maximum(
                maxrow2, _pack(P.p2[k][0], P.C2).astype(np.int64).max(axis=0))

        piece = P.ncores * 128
        def p2_src(g0, n):
            # prefix slice of the edge table covering every row this gather
            # call touches, so it only waits on the AllGather pieces it needs
            pieces = int(maxrow2[g0:g0 + n].max()) // piece + 1
            if not AG_SLICED or pieces >= P.NW1:
                return CCOUT[:]
            return CCOUT[0:pieces * piece, :]

        # p2 prefetch schedule: at p1 window boundary w we may issue gathers
        # for p2 chunks that only need table pieces < w (their AllGather was
        # triggered a full window earlier).  Earliest-consumed chunks first.
        bound = (maxrow2 // piece + 1).astype(int)   # pieces needed per chunk
        pre_sched = {w: [] for w in range(1, P.NW1)}
        if AG_SLICED and PRE_PER_B > 0:
            taken = set()
            for w in range(1, P.NW1):
                for c in range(P.C2):
                    if len(pre_sched[w]) >= PRE_PER_B:
                        break
                    if c not in taken and bound[c] <= w:
                        pre_sched[w].append(c)
                        taken.add(c)

        for _rep in range(reps):
            pre_tiles.clear()
            agg_phase(XT[:], g1idx_t, eloc1_t, wsel1_t, eloc1b_t, wsel1b_t,
                      is2_1, P.C1, P.W1, "g1", p1_chunk, p1_win, dt_p1)

            # phase 2: e2v aggregation (sel weights pre-normalized on host)
            agg_phase(CCOUT[:], g2idx_t, vloc2_t, w2raw_t, vloc2b_t, w2rawb_t,
                      is2_2, P.C2, P.W2, "g2", p2_chunk, p2_win, dt_p2,
                      src_sel=p2_src, pre_tiles=pre_tiles,
                      cw_major=CW_MAJOR, pool=g2pool)

    nc.compile()
    return nc


# ------------------------------------------------------------------ runner ---
def make_in_maps(P, X, W, b, bf16=GATHER_BF16, p1_dt=P1_DT, fuse=FUSE):
    npdt = ml_dtypes.bfloat16 if bf16 else np.float32
    np_p1 = _npdt(p1_dt)
    s1 = P1_SCALE if p1_dt == "f8" else 1.0
    s2 = P2_SCALE if fuse == "post" else 1.0
    KT = P.ch // 128
    xt = np.ascontiguousarray((X * s1).astype(np_p1))
    wt = np.ascontiguousarray(
        W.T.reshape(KT, 128, P.ch).transpose(1, 0, 2).astype(npdt))
    bt = np.ascontiguousarray(b.astype(npdt).reshape(1, P.ch))
    iota = np.ascontiguousarray(
        np.broadcast_to(np.arange(128, dtype=npdt), (128, 128)))
    ident = np.eye(128, dtype=npdt)

    def tb(flat, C, s=1.0):
        return _pack(flat, C) / np.float32(s)

    in_maps = []
    for k in range(P.ncores):
        g1, l1, w1, l1b, w1b = P.p1[k]
        g2, l2, w2, l2b, w2b = P.p2[k]
        p1tab = np.ascontiguousarray(np.stack(
            [tb(l1, P.C1), tb(w1, P.C1, s1), tb(l1b, P.C1), tb(w1b, P.C1, s1)],
            axis=1))
        p2tab = np.ascontiguousarray(np.stack(
            [tb(l2, P.C2), tb(w2, P.C2, s2), tb(l2b, P.C2), tb(w2b, P.C2, s2)],
            axis=1))
        in_maps.append({
            "xt": xt, "wt": wt, "bt": bt, "iota": iota, "ident": ident,
            "g1idx": _wrap_idx(g1), "p1tab": p1tab,
            "g2idx": _wrap_idx(g2), "p2tab": p2tab,
        })
    return in_maps


def assemble(P, shards):
    out = np.zeros((P.nv, P.ch), np.float32)
    for k in range(P.ncores):
        vm = P.vmap[k]
        m = vm >= 0
        out[vm[m]] = shards[k][m].astype(np.float32)
    return out


_nc_cache = {}


def kernel(X, W, b, e2v_weight, v_idx, e_idx):
    global _last_results
    from concourse.bass_utils import run_bass_kernel_spmd

    P = make_plan(v_idx, e_idx, e2v_weight)
    key = (P.C1, P.C2, P.W1, P.W2, GATHER_BF16, P1_DT, P2_DT, OUT_DT, FUSE,
           AG_SLICED)
    if key not in _nc_cache:
        _nc_cache[key] = build_nc(P)
    nc = _nc_cache[key]
    in_maps = make_in_maps(P, X, W, b)
    res = run_bass_kernel_spmd(nc, in_maps, list(range(P.ncores)), trace=TRACE)
    _last_results = res
    shards = [res.results[k]["out"] for k in range(P.ncores)]
    return assemble(P, shards)



# revision 96
# speedup vs baseline: 1.0375x; 1.0124x over previous
"""HGNNPConv Trainium2 kernel (8 NeuronCores, SPMD).

Math (equivalent reformulation of the reference):
  Xe_raw[e] = mean_{i: e_idx[i]=e} X[v_idx[i]]              (v2e, softmax of ones = 1/deg)
  Xe_p      = Xe_raw @ W.T + b                              (GEMM on 4000 edges, not 20000 verts)
  Xv[v]     = sum_i wn_i * Xe_p[e_idx[i]],  wn_i = exp(w_i)/sum_{v} exp(w)
              (wn precomputed on host -> no on-chip denominator pass)
  out       = relu(Xv)
Deg-0 vertices are masked host-side in assemble(); empty edges get a spurious
+b in Xe_p but are never referenced downstream.

Sharding: edges and vertices are binpacked across the 8 cores (balancing
post-dedup gather slots).  Phase 1 aggregates by destination edge, runs the
edge-level GEMM per 128-edge window (bias folded in as a K=1 matmul), and
AllGathers each window's table slice as soon as it is ready (overlapping the
collective with the remaining phase-1 work).  Phase 2 aggregates by
destination vertex; each gather call's source AP is a prefix slice of the
table covering only the AllGather pieces it needs.

Per-destination-window weighted one-hot selection matrices (built on DVE
from a bf16 iota) reduce gathered rows on the PE into PSUM.  Incidences
sharing a source row within a window are deduplicated into one gathered slot
with a two-hot sel column (~11% fewer gather bytes; window binpacking is
rebalanced for post-dedup slot counts, phase 2 uses 21 windows x 7 chunks
instead of 20 x 8, and per-window chunk counts are variable — windows are
sorted largest-first so cross-core padding aligns and each phase ends on its
smallest window).

Dtypes: phase-1 gather table fp8-e3m4 (X pre-scaled by 2: worst-case quant
error halves vs e4m3 and the GEMM averages it out), phase-2 table bf16
(absmax-norm tolerance rules out fp8 post-GEMM), output bf16 (upcast on
host).  End-to-end rel err 1.35e-2 vs the 2e-2 gate.
"""

import os
from contextlib import ExitStack

import numpy as np
import ml_dtypes

# ---------------------------------------------------------------- config ---
NCORES = 8
NV, NE, NNZ, CH = 20000, 4000, 160000, 512
GATHER_BF16 = os.environ.get("KERNEL_F32", "") == ""  # bf16 tables+matmuls by default
P1_DT = os.environ.get("KERNEL_P1_DT", "f8")   # phase-1 gather table dtype
P2_DT = os.environ.get("KERNEL_P2_DT", "bf16")  # phase-2 gather table dtype
OUT_DT = os.environ.get("KERNEL_OUT_DT", "bf16")
FUSE = os.environ.get("KERNEL_FUSE", "pre")    # "post": GEMM after p2 agg
P1_SCALE = 2.0   # X pre-scale for fp8-e3m4 range use (exact power of 2)
P2_SCALE = 8.0   # Xe_raw pre-scale for the fp8 edge table (post mode)
GRP = 5          # gather chunks (of 128 idxs) per dma_gather call
AG_SLICED = os.environ.get("KERNEL_AG_SLICED", "1") != ""  # per-window AllGather
PRE_PER_B = int(os.environ.get("KERNEL_PRE_PER_B", "0"))  # p2 prefetches per p1 window
CW_MAJOR = os.environ.get("KERNEL_CW_MAJOR", "") != ""  # cw-major p2 table layout
TRACE = os.environ.get("BASS_TRACE", "") != ""


def _mydt(mybir, name):
    return {"f8": mybir.dt.float8e3, "bf16": mybir.dt.bfloat16,
            "f32": mybir.dt.float32}[name]


def _npdt(name):
    return {"f8": ml_dtypes.float8_e3m4, "bf16": ml_dtypes.bfloat16,
            "f32": np.float32}[name]

_last_results = None   # BassKernelResults of the most recent run (for test.py)


# ------------------------------------------------------------------- plan ---
class Plan:
    pass


def _binpack(ids, degs, nbins, cap=128):
    """Pack `ids` into `nbins` bins of <=cap items, balancing sum(degs)."""
    import heapq

    order = np.argsort(-degs, kind="stable")
    bins = [[] for _ in range(nbins)]
    loads = [0] * nbins
    heap = [(0, b) for b in range(nbins)]
    heapq.heapify(heap)
    for t in order:
        popped = []
        while True:
            load, b = heapq.heappop(heap)
            if len(bins[b]) < cap:
                break
            popped.append((load, b))
        for p in popped:
            heapq.heappush(heap, p)
        bins[b].append(int(ids[t]))
        loads[b] = load + int(degs[t])
        heapq.heappush(heap, (loads[b], b))
    return bins, loads


def _csr(idx, n):
    order = np.argsort(idx, kind="stable").astype(np.int64)
    deg = np.bincount(idx, minlength=n).astype(np.int64)
    starts = np.zeros(n + 1, np.int64)
    np.cumsum(deg, out=starts[1:])
    return order, deg, starts


def _pair_window(src, locs, ws):
    """Greedy within-window dedup: incidences sharing a source row become one
    gathered slot with two (loc, w) hots.  Returns (src', loc_a, w_a, loc_b,
    w_b) with pair slots FIRST; singles have loc_b = -1 / w_b = 0."""
    order = np.argsort(src, kind="stable")
    src, locs, ws = src[order], locs[order], ws[order]
    pa, pb, sg = [], [], []
    i, n = 0, len(src)
    while i < n:
        j = i
        while j < n and src[j] == src[i]:
            j += 1
        k = i
        while k + 1 < j:
            pa.append(k); pb.append(k + 1); k += 2
        if k < j:
            sg.append(k)
        i = j
    pa, pb, sg = np.array(pa, np.int64), np.array(pb, np.int64), np.array(sg, np.int64)
    src2 = np.concatenate([src[pa], src[sg]]) if len(pa) else src[sg]
    la = np.concatenate([locs[pa], locs[sg]]) if len(pa) else locs[sg]
    wa = np.concatenate([ws[pa], ws[sg]]) if len(pa) else ws[sg]
    lb = np.concatenate([locs[pb], np.full(len(sg), -1.0, locs.dtype)]) if len(pa) \
        else np.full(len(sg), -1.0, locs.dtype)
    wb = np.concatenate([ws[pb], np.zeros(len(sg), ws.dtype)]) if len(pa) \
        else np.zeros(len(sg), ws.dtype)
    return src2, la, wa, lb, wb, len(pa)


def _phase_windows(bins_per_core, order, starts, idx_of_inc, w_of_inc,
                   loc_dtype=np.float32, pair=True, piece_order=False):
    """Per-window slot lists for one core of one phase, after source dedup.

    Returns (wins, wmax): wins[w] = (src, loc_a, w_a, loc_b, w_b).  By
    default pair slots (loc_b >= 0) come first within each window; with
    piece_order=True, singles come first sorted by source id (so early
    chunks only reference low table pieces) and pair slots go last.
    """
    wins = []
    wmax = 0
    for bin_ids in bins_per_core:
        incs = []
        locs = []
        for j, d in enumerate(bin_ids):
            seg = order[starts[d]:starts[d + 1]]
            incs.append(seg)
            locs.append(np.full(len(seg), j, loc_dtype))
        incs = np.concatenate(incs) if incs else np.zeros(0, np.int64)
        locs = np.concatenate(locs) if locs else np.zeros(0, loc_dtype)
        src = idx_of_inc[incs]
        ws = w_of_inc[incs].astype(np.float32)
        if pair and len(src):
            src, la, wa, lb, wb, npair = _pair_window(src, locs, ws)
            if piece_order:
                ps = np.argsort(src[:npair], kind="stable")
                ss = npair + np.argsort(src[npair:], kind="stable")
                perm = np.concatenate([ss, ps])
                src, la, wa, lb, wb = (src[perm], la[perm], wa[perm],
                                       lb[perm], wb[perm])
        else:
            la, wa = locs, ws
            lb = np.full(len(src), -1.0, loc_dtype)
            wb = np.zeros(len(src), np.float32)
        wins.append((src, la, wa, lb, wb))
        wmax = max(wmax, len(src))
    return wins, wmax


def _layout(wins, W_list):
    """Flat slot arrays with per-window chunk counts.  Window w's cw-th
    chunk sits at table column offs[w] + cw."""
    offs = np.concatenate([[0], np.cumsum(W_list)])
    L = int(offs[-1]) * 128
    gidx = np.zeros(L, np.int16)
    loc = np.full(L, -1.0, np.float32)
    wsel = np.zeros(L, np.float32)
    locb = np.full(L, -1.0, np.float32)
    wselb = np.zeros(L, np.float32)
    for w, (src, la, wa, lb, wb) in enumerate(wins):
        n = len(src)
        assert n <= W_list[w] * 128
        o = int(offs[w]) * 128
        gidx[o:o + n] = src
        loc[o:o + n] = la
        wsel[o:o + n] = wa
        locb[o:o + n] = lb
        wselb[o:o + n] = wb
    return gidx, loc, wsel, locb, wselb


def _wrap_idx(flat):
    """int16 flat[i] -> [128, len/16] with value i at [i%16, i//16], replicated."""
    a = flat.reshape(-1, 16).T  # [16, L/16]
    return np.ascontiguousarray(np.tile(a, (8, 1)))


def _pack(flat, C):
    """flat[c*128+p] -> [128, C]"""
    return np.ascontiguousarray(flat.reshape(C, 128).T)


def _dedup_slots(srcs):
    """#gather slots for a window's source list after pairing."""
    if not len(srcs):
        return 0
    _, cnt = np.unique(srcs, return_counts=True)
    return int(((cnt + 1) // 2).sum())


def _repair_bins(bins, order, starts, idx_of_inc, cap=128, iters=400):
    """Greedy rebalance: move members out of the window with the most
    post-dedup slots into the one with the fewest (respecting the member
    cap), to minimize max slots per window."""
    srcs = [
        [idx_of_inc[order[starts[d]:starts[d + 1]]] for d in b] for b in bins
    ]

    def slots(w):
        return _dedup_slots(np.concatenate(srcs[w]) if srcs[w] else
                            np.zeros(0, np.int64))

    cur = [slots(w) for w in range(len(bins))]
    for _ in range(iters):
        hot = int(np.argmax(cur))
        order_cold = np.argsort(cur)
        moved = False
        for cold in order_cold:
            if cold == hot or len(bins[cold]) >= cap:
                continue
            # move the member with the smallest segment out of `hot`
            j = int(np.argmin([len(s) for s in srcs[hot]]))
            bins[cold].append(bins[hot].pop(j))
            srcs[cold].append(srcs[hot].pop(j))
            new_hot, new_cold = slots(hot), slots(cold)
            if max(new_hot, new_cold) >= cur[hot]:
                # revert: no improvement
                bins[hot].append(bins[cold].pop())
                srcs[hot].append(srcs[cold].pop())
                continue
            cur[hot], cur[cold] = new_hot, new_cold
            moved = True
            break
        if not moved:
            break
    return bins


def make_plan(v_idx, e_idx, e2v_weight, nv=NV, ne=NE, ch=CH, ncores=NCORES):
    P = Plan()
    P.nv, P.ne, P.ch, P.ncores = nv, ne, ch, ncores
    epc, vpc = ne // ncores, nv // ncores
    P.epc, P.vpc = epc, vpc

    order_e, deg_e, starts_e = _csr(e_idx, ne)
    order_v, deg_v, starts_v = _csr(v_idx, nv)
    inv_deg = np.zeros(ne, np.float32)
    nz = deg_e > 0
    inv_deg[nz] = (np.float32(1.0) / deg_e[nz].astype(np.float32))

    nb1 = -(-epc // 128)
    nb2 = -(-vpc // 128)
    v_of_inc = v_idx.astype(np.int64)
    e_of_inc = e_idx.astype(np.int64)
    # balance destinations across cores globally (assignment is free — pos /
    # vmap carry it), then binpack windows within each core and rebalance for
    # post-dedup slot counts.
    cores_e, _ = _binpack(np.arange(ne), deg_e, ncores, cap=nb1 * 128)
    bins1 = []
    for k in range(ncores):
        eids = np.asarray(cores_e[k])
        b, _ = _binpack(eids, deg_e[eids], nb1)
        bins1.append(_repair_bins(b, order_e, starts_e, v_of_inc))

    # p2 window count: an extra window can admit a smaller W2 (less gather
    # padding) once dedup shrinks the per-window loads — pick the best.
    best = None
    for nb2c in (nb2, nb2 + 1):
        cores_v, _ = _binpack(np.arange(nv), deg_v, ncores, cap=nb2c * 128)
        cand = []
        sizes = []
        for k in range(ncores):
            vids = np.asarray(cores_v[k])
            b, _ = _binpack(vids, deg_v[vids], nb2c)
            b = _repair_bins(b, order_v, starts_v, e_of_inc)
            cand.append(b)
            sizes.append(sorted(
                (-(-_dedup_slots(np.concatenate(
                    [e_of_inc[order_v[starts_v[d]:starts_v[d + 1]]]
                     for d in bb]) if bb else np.zeros(0, np.int64)) // 128)
                 for bb in b), reverse=True))
        C2c = sum(max(sizes[k][w] for k in range(ncores))
                  for w in range(nb2c))
        if best is None or C2c < best[1]:
            best = (nb2c, C2c, cand)
    nb2, _, bins2 = best
    P.NW1, P.NW2 = nb1, nb2

    # phase-1 windows (dedup within window) + edge position map.  Positions
    # are window-major (w, core, row) so each window's table slice can be
    # AllGathered independently as soon as its GEMM finishes.  Windows are
    # sorted largest-first per core: cross-core maxes align (less padding)
    # and the smallest window lands last (shorter phase tail).
    pos = np.zeros(ne, np.int64)
    wins1 = []
    w1max = 0
    for k in range(ncores):
        wins, wmax = _phase_windows(
            bins1[k], order_e, starts_e, v_idx.astype(np.int64),
            inv_deg[e_idx.astype(np.int64)])
        order = np.argsort([-len(w[0]) for w in wins], kind="stable")
        wins = [wins[i] for i in order]
        bins1[k] = [bins1[k][i] for i in order]
        wins1.append(wins)
        w1max = max(w1max, wmax)
        for w, bin_ids in enumerate(bins1[k]):
            for j, e in enumerate(bin_ids):
                if AG_SLICED:   # window-major: (w, core, row)
                    pos[e] = (w * ncores + k) * 128 + j
                else:           # core-major: AllGather rank concatenation
                    pos[e] = (k * nb1 + w) * 128 + j
    assert pos.max() < 32768
    P.W1_list = [
        max(-(-len(wins1[k][w][0]) // 128) for k in range(ncores))
        for w in range(nb1)
    ]
    P.W1 = max(P.W1_list)
    P.C1 = int(sum(P.W1_list))
    P.p1 = [_layout(wins, P.W1_list) for wins in wins1]

    # phase-2 windows + output row map. Softmax weights are fully normalized
    # on the host (exp / per-vertex sum), so the kernel needs no denominator
    # pass.
    expw = np.exp(e2v_weight.astype(np.float64))
    den = np.zeros(nv, np.float64)
    np.add.at(den, v_idx, expw)
    wnorm = (expw / den[v_idx.astype(np.int64)]).astype(np.float32)
    wins2 = []
    P.vmap = []
    for k in range(ncores):
        wins, _ = _phase_windows(
            bins2[k], order_v, starts_v, pos[e_idx.astype(np.int64)], wnorm,
            piece_order=AG_SLICED)
        order = np.argsort([-len(w[0]) for w in wins], kind="stable")
        wins = [wins[i] for i in order]
        bins2[k] = [bins2[k][i] for i in order]
        wins2.append(wins)
        vm = np.full(P.NW2 * 128, -1, np.int64)
        for w, bin_ids in enumerate(bins2[k]):
            vm[w * 128:w * 128 + len(bin_ids)] = bin_ids
        # deg-0 vertices never receive contributions; drop them from the
        # output map so any on-chip garbage (e.g. a stray +b) is discarded.
        vme = vm[vm >= 0]
        vm[vm >= 0] = np.where(deg_v[vme] > 0, vme, -1)
        P.vmap.append(vm)
    P.W2_list = [
        max(-(-len(wins2[k][w][0]) // 128) for k in range(ncores))
        for w in range(nb2)
    ]
    P.W2 = max(P.W2_list)
    P.C2 = int(sum(P.W2_list))
    P.p2 = [_layout(wins, P.W2_list) for wins in wins2]
    return P


# ---------------------------------------------------------------- builder ---
def build_nc(P, bf16=GATHER_BF16, spmd=True, reps=1, grp=GRP, gbufs=5,
             nqueues=1, p1_dt=P1_DT, p2_dt=P2_DT, out_dt=OUT_DT, fuse=FUSE):
    import concourse.bacc as bacc
    import concourse.mybir as mybir
    import concourse.tile as tile

    f32 = mybir.dt.float32
    dt_g = mybir.dt.bfloat16 if bf16 else f32   # sel matrices + GEMM operands
    dt_p1 = _mydt(mybir, p1_dt)
    dt_p2 = _mydt(mybir, p2_dt)
    dt_out = _mydt(mybir, out_dt)
    eq, mul, mx, add = (mybir.AluOpType.is_equal, mybir.AluOpType.mult,
                        mybir.AluOpType.max, mybir.AluOpType.add)
    ch, KT = P.ch, P.ch // 128
    post = fuse == "post"

    nc = bacc.Bacc("TRN2", target_bir_lowering=False, debug=False,
                   num_devices=P.ncores if spmd else 1,
                   num_swdge_queues=nqueues)

    XT = nc.dram_tensor("xt", [P.nv, ch], dt_p1, kind="ExternalInput")
    WT = nc.dram_tensor("wt", [128, KT, ch], dt_g, kind="ExternalInput")
    BT = nc.dram_tensor("bt", [1, ch], dt_g, kind="ExternalInput")
    IOTA = nc.dram_tensor("iota", [128, 128], dt_g, kind="ExternalInput")
    IDENT = nc.dram_tensor("ident", [128, 128], dt_g, kind="ExternalInput")
    G1IDX = nc.dram_tensor("g1idx", [128, P.C1 * 8], mybir.dt.int16, kind="ExternalInput")
    P1TAB = nc.dram_tensor("p1tab", [128, 4, P.C1], f32, kind="ExternalInput")
    G2IDX = nc.dram_tensor("g2idx", [128, P.C2 * 8], mybir.dt.int16, kind="ExternalInput")
    P2TAB = nc.dram_tensor("p2tab", [128, 4, P.C2], f32, kind="ExternalInput")

    ner1 = P.NW1 * 128
    CCIN = nc.dram_tensor("ccin", [ner1, ch], dt_p2)
    CCOUT = nc.dram_tensor("ccout", [P.ncores * ner1, ch], dt_p2, addr_space="Shared")
    OUT = nc.dram_tensor("out", [P.NW2 * 128, ch], dt_out, kind="ExternalOutput")

    with tile.TileContext(nc) as tc, ExitStack() as ctx:
        const = ctx.enter_context(tc.tile_pool(name="const", bufs=1))
        gpool = ctx.enter_context(tc.tile_pool(name="g", bufs=gbufs))
        g2pool = ctx.enter_context(tc.tile_pool(name="g2", bufs=gbufs))
        prep = ctx.enter_context(tc.tile_pool(name="prep", bufs=1))
        selp = ctx.enter_context(tc.tile_pool(name="selp", bufs=8))
        psum = ctx.enter_context(tc.tile_pool(name="ps", bufs=2, space="PSUM"))
        sbp = ctx.enter_context(tc.tile_pool(name="sbp", bufs=2))
        pre_tiles = {}

        def cload(dram, shape, dt, tag, eng=None):
            t = const.tile(shape, dt, tag=tag)
            (eng or nc.sync).dma_start(t[:], dram[:])
            return t

        # p1-critical tables first (SP ring); the rest go on the ACT ring so
        # they never delay the first gather.  g1idx leads: the first gather
        # needs only it, iota/p1tab are for the (later) first sel build.
        g1idx_t = cload(G1IDX, [128, P.C1 * 8], mybir.dt.int16, "g1idx")
        iota_t = cload(IOTA, [128, 128], dt_g, "iota")
        p1tab_t = cload(P1TAB, [128, 4, P.C1], f32, "p1tab")
        eloc1_t, wsel1_t = p1tab_t[:, 0, :], p1tab_t[:, 1, :]
        eloc1b_t, wsel1b_t = p1tab_t[:, 2, :], p1tab_t[:, 3, :]
        wt_t = cload(WT, [128, KT, ch], dt_g, "wt", eng=nc.scalar)
        bt_t = cload(BT, [1, ch], dt_g, "bt", eng=nc.scalar)
        ident_t = cload(IDENT, [128, 128], dt_g, "ident", eng=nc.scalar)
        g2idx_t = cload(G2IDX, [128, P.C2 * 8], mybir.dt.int16, "g2idx",
                        eng=nc.scalar)
        p2tab_t = cload(P2TAB, [128, 4, P.C2], f32, "p2tab", eng=nc.scalar)
        vloc2_t, w2raw_t = p2tab_t[:, 0, :], p2tab_t[:, 1, :]
        vloc2b_t, w2rawb_t = p2tab_t[:, 2, :], p2tab_t[:, 3, :]
        ones1_t = const.tile([1, 128], dt_g, tag="ones1")
        nc.vector.memset(ones1_t[:], 1.0)

        # ---------------- gather + one/two-hot reduce ----------------------
        def agg_phase(src_ap, gidx_t, loc_t, w_t, locb_t, wb_t, is2, W_list,
                      gtag, chunk_cb, win_cb, dt_tab, src_sel=None,
                      pre_tiles=None, pool=None):
            """Consumption iterates table columns; window w's chunks occupy
            columns offs[w]..offs[w+1)-1 (per-window chunk counts)."""
            pre_tiles = pre_tiles or {}
            pool = pool or gpool
            col_w, col_cw = [], []
            for w, Wx in enumerate(W_list):
                for cw in range(Wx):
                    col_w.append(w)
                    col_cw.append(cw)
            C = len(col_w)
            # calls = runs of consecutive non-prefetched table columns
            calls = []
            run = []
            for t in range(C):
                if t in pre_tiles or len(run) == grp:
                    if run:
                        calls.append((run[0], len(run)))
                    run = []
                if t not in pre_tiles:
                    run.append(t)
            if run:
                calls.append((run[0], len(run)))
            call_of = {}
            for g0, n in calls:
                for j in range(n):
                    call_of[g0 + j] = (g0, n)
            tiles = {}
            pw = None
            for t in range(C):
                if t in call_of and call_of[t][0] == t:
                    g0, n = call_of[t]
                    gt_new = pool.tile([128, grp, ch], dt_tab, tag=gtag)
                    src = src_sel(g0, n) if src_sel is not None else src_ap
                    nc.gpsimd.dma_gather(
                        gt_new[:, 0:n, :], src,
                        gidx_t[:, g0 * 8:(g0 + n) * 8],
                        n * 128, n * 128, ch,
                        queue_num=(g0 // grp) % nqueues)
                    tiles[g0] = gt_new
                w, cw = col_w[t], col_cw[t]
                if t in pre_tiles:
                    gt, j = pre_tiles[t]
                else:
                    g0, n = call_of[t]
                    gt, j = tiles[g0], t - g0
                sel = selp.tile([128, 128], dt_g, tag="sel")
                nc.vector.tensor_scalar(
                    sel[:], iota_t[:], loc_t[:, t:t + 1], w_t[:, t:t + 1],
                    op0=eq, op1=mul)
                if is2[t]:  # dedup chunk: add the second hot
                    selb = selp.tile([128, 128], dt_g, tag="selb")
                    nc.vector.tensor_scalar(
                        selb[:], iota_t[:], locb_t[:, t:t + 1],
                        wb_t[:, t:t + 1], op0=eq, op1=mul)
                    sel2 = selp.tile([128, 128], dt_g, tag="sel2")
                    nc.vector.tensor_tensor(sel2[:], sel[:], selb[:], op=add)
                    sel = sel2
                if cw == 0:
                    pw = psum.tile([128, ch], f32, tag="win")
                last = cw == W_list[w] - 1
                chunk_cb(pw, sel, gt, j, w, cw, last)
                if last:
                    win_cb(pw, w)

        def p1_chunk(pw, sel, gt, j, w, cw, last):
            nc.tensor.matmul(pw[:], sel[:], gt[:, j, :],
                             start=(cw == 0), stop=last)

        def gemm_bias(src_t, dst_psum):
            """dst[v/e, co] = src^T blocks @ W.T + 1^T b (K=1 bias matmul)."""
            for k in range(KT):
                nc.tensor.matmul(dst_psum[:], src_t[:, k, :], wt_t[:, k, :],
                                 start=(k == 0), stop=False)
            nc.tensor.matmul(dst_psum[:], ones1_t[:], bt_t[:],
                             start=False, stop=True)

        def transpose_blocks(pw, tag):
            """psum [128, ch] f32 -> sbuf [128, KT, 128] dt_g transposed.
            Copies are per-128-block so transpose k pipelines with copy k+1."""
            tT_w = sbp.tile([128, KT, 128], dt_g, tag=tag + "T", name=tag + "T")
            for k in range(KT):
                twk = sbp.tile([128, 128], dt_g, tag=tag + "f", name=tag + "f")
                nc.vector.tensor_copy(twk[:], pw[:, k * 128:(k + 1) * 128])
                pt = psum.tile([128, 128], dt_g, tag="aux", name="pt")
                nc.tensor.transpose(pt[:], twk[:], ident_t[:])
                nc.vector.tensor_copy(tT_w[:, k, :], pt[:])
            return tT_w

        def p1_win(pw, w):
            # window w's edge rows are complete: ship its CCIN slice and
            # immediately AllGather that window's table piece, overlapping
            # the collective with the remaining p1 windows.
            xep = sbp.tile([128, ch], dt_p2, tag="xep", name="xep")
            if post:
                # raw table, scaled for fp8 range; GEMM happens after p2 agg
                nc.vector.tensor_scalar(xep[:], pw[:], float(P2_SCALE), None,
                                        op0=mul)
            else:
                xeT_w = transpose_blocks(pw, "xe")
                pg = psum.tile([128, ch], f32, tag="gemm", name="pg")
                gemm_bias(xeT_w, pg)
                nc.vector.tensor_copy(xep[:], pg[:])
            nc.sync.dma_start(CCIN[w * 128:(w + 1) * 128, :], xep[:])
            if AG_SLICED:
                o = w * P.ncores * 128
                if spmd:
                    nc.gpsimd.collective_compute(
                        "AllGather", mybir.AluOpType.bypass,
                        replica_groups=[list(range(P.ncores))],
                        ins=[CCIN[w * 128:(w + 1) * 128, :]],
                        outs=[CCOUT[o:o + P.ncores * 128, :]])
                else:  # single-core stand-in for the window AllGather
                    nc.sync.dma_start(CCOUT[o:o + 128, :],
                                      CCIN[w * 128:(w + 1) * 128, :])
            elif w == P.NW1 - 1:
                if spmd:
                    nc.gpsimd.collective_compute(
                        "AllGather", mybir.AluOpType.bypass,
                        replica_groups=[list(range(P.ncores))],
                        ins=[CCIN[:]], outs=[CCOUT[:]])
                else:
                    nc.sync.dma_start(CCOUT[0:ner1, :], CCIN[:])
            # prefetch p2 chunks whose table pieces are already gathered,
            # soaking p1's spare DMA bandwidth
            for c in pre_sched.get(w, []):
                gt = prep.tile([128, 1, ch], dt_p2, tag=f"pre{c}")
                nc.gpsimd.dma_gather(
                    gt[:], p2_src(c, 1), g2idx_t[:, c * 8:(c + 1) * 8],
                    128, 128, ch, queue_num=0)
                pre_tiles[c] = (gt, 0)

        def p2_chunk(pw, sel, gt, j, w, cw, last):
            nc.tensor.matmul(pw[:], sel[:], gt[:, j, :],
                             start=(cw == 0), stop=last)

        def p2_win(pw, w):
            if post:
                awT = transpose_blocks(pw, "aw")
                po = psum.tile([128, ch], f32, tag="gemm", name="po")
                gemm_bias(awT, po)
                pw = po
            # weights pre-normalized on host: just relu + store
            ow = sbp.tile([128, ch], dt_out, tag="ow", name="ow")
            nc.vector.tensor_scalar(ow[:], pw[:], 1.0, 0.0, op0=mul, op1=mx)
            nc.sync.dma_start(OUT[w * 128:(w + 1) * 128, :], ow[:])

        # chunks that contain any dedup pair need the second sel pass; the
        # union over cores keeps the SPMD program identical on every core.
        is2_1 = np.zeros(P.C1, bool)
        is2_2 = np.zeros(P.C2, bool)
        maxrow2 = np.zeros(P.C2, np.int64)
        for k in range(P.ncores):
            is2_1 |= (_pack(P.p1[k][3], P.C1) >= 0).any(axis=0)
            is2_2 |= (_pack(P.p2[k][3], P.C2) >= 0).any(axis=0)
            maxrow2 = np.maximum(
                maxrow2, _pack(P.p2[k][0], P.C2).astype(np.int64).max(axis=0))

        piece = P.ncores * 128
        def p2_src(g0, n):
            # prefix slice of the edge table covering every row this gather
            # call touches, so it only waits on the AllGather pieces it needs
            pieces = int(maxrow2[g0:g0 + n].max()) // piece + 1
            if not AG_SLICED or pieces >= P.NW1:
                return CCOUT[:]
            return CCOUT[0:pieces * piece, :]

        # p2 prefetch schedule: at p1 window boundary w we may issue gathers
        # for p2 chunks that only need table pieces < w (their AllGather was
        # triggered a full window earlier).  Earliest-consumed chunks first.
        bound = (maxrow2 // piece + 1).astype(int)   # pieces needed per chunk
        pre_sched = {w: [] for w in range(1, P.NW1)}
        if AG_SLICED and PRE_PER_B > 0:
            taken = set()
            for w in range(1, P.NW1):
                for c in range(P.C2):
                    if len(pre_sched[w]) >= PRE_PER_B:
                        break
                    if c not in taken and bound[c] <= w:
                        pre_sched[w].append(c)
                        taken.add(c)

        for _rep in range(reps):
            pre_tiles.clear()
            agg_phase(XT[:], g1idx_t, eloc1_t, wsel1_t, eloc1b_t, wsel1b_t,
                      is2_1, P.W1_list, "g1", p1_chunk, p1_win, dt_p1)

            # phase 2: e2v aggregation (sel weights pre-normalized on host)
            agg_phase(CCOUT[:], g2idx_t, vloc2_t, w2raw_t, vloc2b_t, w2rawb_t,
                      is2_2, P.W2_list, "g2", p2_chunk, p2_win, dt_p2,
                      src_sel=p2_src, pre_tiles=pre_tiles, pool=g2pool)

    nc.compile()
    return nc


# ------------------------------------------------------------------ runner ---
def make_in_maps(P, X, W, b, bf16=GATHER_BF16, p1_dt=P1_DT, fuse=FUSE):
    npdt = ml_dtypes.bfloat16 if bf16 else np.float32
    np_p1 = _npdt(p1_dt)
    s1 = P1_SCALE if p1_dt == "f8" else 1.0
    s2 = P2_SCALE if fuse == "post" else 1.0
    KT = P.ch // 128
    xt = np.ascontiguousarray((X * s1).astype(np_p1))
    wt = np.ascontiguousarray(
        W.T.reshape(KT, 128, P.ch).transpose(1, 0, 2).astype(npdt))
    bt = np.ascontiguousarray(b.astype(npdt).reshape(1, P.ch))
    iota = np.ascontiguousarray(
        np.broadcast_to(np.arange(128, dtype=npdt), (128, 128)))
    ident = np.eye(128, dtype=npdt)

    def tb(flat, C, s=1.0):
        return _pack(flat, C) / np.float32(s)

    in_maps = []
    for k in range(P.ncores):
        g1, l1, w1, l1b, w1b = P.p1[k]
        g2, l2, w2, l2b, w2b = P.p2[k]
        p1tab = np.ascontiguousarray(np.stack(
            [tb(l1, P.C1), tb(w1, P.C1, s1), tb(l1b, P.C1), tb(w1b, P.C1, s1)],
            axis=1))
        p2tab = np.ascontiguousarray(np.stack(
            [tb(l2, P.C2), tb(w2, P.C2, s2), tb(l2b, P.C2), tb(w2b, P.C2, s2)],
            axis=1))
        in_maps.append({
            "xt": xt, "wt": wt, "bt": bt, "iota": iota, "ident": ident,
            "g1idx": _wrap_idx(g1), "p1tab": p1tab,
            "g2idx": _wrap_idx(g2), "p2tab": p2tab,
        })
    return in_maps


def assemble(P, shards):
    out = np.zeros((P.nv, P.ch), np.float32)
    for k in range(P.ncores):
        vm = P.vmap[k]
        m = vm >= 0
        out[vm[m]] = shards[k][m].astype(np.float32)
    return out


_nc_cache = {}


def kernel(X, W, b, e2v_weight, v_idx, e_idx):
    global _last_results
    from concourse.bass_utils import run_bass_kernel_spmd

    P = make_plan(v_idx, e_idx, e2v_weight)
    key = (P.C1, P.C2, P.W1, P.W2, GATHER_BF16, P1_DT, P2_DT, OUT_DT, FUSE,
           AG_SLICED)
    if key not in _nc_cache:
        _nc_cache[key] = build_nc(P)
    nc = _nc_cache[key]
    in_maps = make_in_maps(P, X, W, b)
    res = run_bass_kernel_spmd(nc, in_maps, list(range(P.ncores)), trace=TRACE)
    _last_results = res
    shards = [res.results[k]["out"] for k in range(P.ncores)]
    return assemble(P, shards)



# revision 106
# speedup vs baseline: 1.0506x; 1.0126x over previous
"""HGNNPConv Trainium2 kernel (8 NeuronCores, SPMD).

Math (equivalent reformulation of the reference):
  Xe_raw[e] = mean_{i: e_idx[i]=e} X[v_idx[i]]              (v2e, softmax of ones = 1/deg)
  Xe_p      = Xe_raw @ W.T + b                              (GEMM on 4000 edges, not 20000 verts)
  Xv[v]     = sum_i wn_i * Xe_p[e_idx[i]],  wn_i = exp(w_i)/sum_{v} exp(w)
              (wn precomputed on host -> no on-chip denominator pass)
  out       = relu(Xv)
Deg-0 vertices are masked host-side in assemble(); empty edges get a spurious
+b in Xe_p but are never referenced downstream.

Sharding: edges and vertices are binpacked across the 8 cores (balancing
post-dedup gather slots).  Phase 1 aggregates by destination edge, runs the
edge-level GEMM per 128-edge window (bias folded in as a K=1 matmul), and
AllGathers each window's table slice as soon as it is ready (overlapping the
collective with the remaining phase-1 work).  Phase 2 aggregates by
destination vertex; each gather call's source AP is a prefix slice of the
table covering only the AllGather pieces it needs.

Per-destination-window weighted one-hot selection matrices (built on DVE
from a bf16 iota) reduce gathered rows on the PE into PSUM.  Incidences
sharing a source row within a window are deduplicated into one gathered slot
with a two-hot sel column (~11% fewer gather bytes; window binpacking is
rebalanced for post-dedup slot counts, phase 2 uses 21 windows x 7 chunks
instead of 20 x 8, and per-window chunk counts are variable — windows are
sorted largest-first so cross-core padding aligns and each phase ends on its
smallest window).

Dtypes: phase-1 gather table fp8-e3m4 (X pre-scaled by 2: worst-case quant
error halves vs e4m3 and the GEMM averages it out), phase-2 table bf16
(absmax-norm tolerance rules out fp8 post-GEMM), output bf16 (upcast on
host).  End-to-end rel err 1.35e-2 vs the 2e-2 gate.
"""

import os
from contextlib import ExitStack

import numpy as np
import ml_dtypes

# ---------------------------------------------------------------- config ---
NCORES = 8
NV, NE, NNZ, CH = 20000, 4000, 160000, 512
GATHER_BF16 = os.environ.get("KERNEL_F32", "") == ""  # bf16 tables+matmuls by default
P1_DT = os.environ.get("KERNEL_P1_DT", "f8")   # phase-1 gather table dtype
P2_DT = os.environ.get("KERNEL_P2_DT", "bf16")  # phase-2 gather table dtype
OUT_DT = os.environ.get("KERNEL_OUT_DT", "bf16")
FUSE = os.environ.get("KERNEL_FUSE", "pre")    # "post": GEMM after p2 agg
P1_SCALE = 2.0   # X pre-scale for fp8-e3m4 range use (exact power of 2)
P2_SCALE = 8.0   # Xe_raw pre-scale for the fp8 edge table (post mode)
GRP = 5          # gather chunks (of 128 idxs) per dma_gather call
AG_SLICED = os.environ.get("KERNEL_AG_SLICED", "1") != ""  # per-window AllGather
PRE_PER_B = int(os.environ.get("KERNEL_PRE_PER_B", "0"))  # p2 prefetches per p1 window
CW_MAJOR = os.environ.get("KERNEL_CW_MAJOR", "") != ""  # cw-major p2 table layout
TRACE = os.environ.get("BASS_TRACE", "") != ""


def _mydt(mybir, name):
    return {"f8": mybir.dt.float8e3, "bf16": mybir.dt.bfloat16,
            "f32": mybir.dt.float32}[name]


def _npdt(name):
    return {"f8": ml_dtypes.float8_e3m4, "bf16": ml_dtypes.bfloat16,
            "f32": np.float32}[name]

_last_results = None   # BassKernelResults of the most recent run (for test.py)


# ------------------------------------------------------------------- plan ---
class Plan:
    pass


def _binpack(ids, degs, nbins, cap=128):
    """Pack `ids` into `nbins` bins of <=cap items, balancing sum(degs)."""
    import heapq

    order = np.argsort(-degs, kind="stable")
    bins = [[] for _ in range(nbins)]
    loads = [0] * nbins
    heap = [(0, b) for b in range(nbins)]
    heapq.heapify(heap)
    for t in order:
        popped = []
        while True:
            load, b = heapq.heappop(heap)
            if len(bins[b]) < cap:
                break
            popped.append((load, b))
        for p in popped:
            heapq.heappush(heap, p)
        bins[b].append(int(ids[t]))
        loads[b] = load + int(degs[t])
        heapq.heappush(heap, (loads[b], b))
    return bins, loads


def _csr(idx, n):
    order = np.argsort(idx, kind="stable").astype(np.int64)
    deg = np.bincount(idx, minlength=n).astype(np.int64)
    starts = np.zeros(n + 1, np.int64)
    np.cumsum(deg, out=starts[1:])
    return order, deg, starts


def _pair_window(src, locs, ws):
    """Greedy within-window dedup: incidences sharing a source row become one
    gathered slot with two (loc, w) hots.  Returns (src', loc_a, w_a, loc_b,
    w_b) with pair slots FIRST; singles have loc_b = -1 / w_b = 0."""
    order = np.argsort(src, kind="stable")
    src, locs, ws = src[order], locs[order], ws[order]
    pa, pb, sg = [], [], []
    i, n = 0, len(src)
    while i < n:
        j = i
        while j < n and src[j] == src[i]:
            j += 1
        k = i
        while k + 1 < j:
            pa.append(k); pb.append(k + 1); k += 2
        if k < j:
            sg.append(k)
        i = j
    pa, pb, sg = np.array(pa, np.int64), np.array(pb, np.int64), np.array(sg, np.int64)
    src2 = np.concatenate([src[pa], src[sg]]) if len(pa) else src[sg]
    la = np.concatenate([locs[pa], locs[sg]]) if len(pa) else locs[sg]
    wa = np.concatenate([ws[pa], ws[sg]]) if len(pa) else ws[sg]
    lb = np.concatenate([locs[pb], np.full(len(sg), -1.0, locs.dtype)]) if len(pa) \
        else np.full(len(sg), -1.0, locs.dtype)
    wb = np.concatenate([ws[pb], np.zeros(len(sg), ws.dtype)]) if len(pa) \
        else np.zeros(len(sg), ws.dtype)
    return src2, la, wa, lb, wb, len(pa)


def _phase_windows(bins_per_core, order, starts, idx_of_inc, w_of_inc,
                   loc_dtype=np.float32, pair=True, piece_order=False):
    """Per-window slot lists for one core of one phase, after source dedup.

    Returns (wins, wmax): wins[w] = (src, loc_a, w_a, loc_b, w_b).  By
    default pair slots (loc_b >= 0) come first within each window; with
    piece_order=True, singles come first sorted by source id (so early
    chunks only reference low table pieces) and pair slots go last.
    """
    wins = []
    wmax = 0
    for bin_ids in bins_per_core:
        incs = []
        locs = []
        for j, d in enumerate(bin_ids):
            seg = order[starts[d]:starts[d + 1]]
            incs.append(seg)
            locs.append(np.full(len(seg), j, loc_dtype))
        incs = np.concatenate(incs) if incs else np.zeros(0, np.int64)
        locs = np.concatenate(locs) if locs else np.zeros(0, loc_dtype)
        src = idx_of_inc[incs]
        ws = w_of_inc[incs].astype(np.float32)
        if pair and len(src):
            src, la, wa, lb, wb, npair = _pair_window(src, locs, ws)
            if piece_order:
                ps = np.argsort(src[:npair], kind="stable")
                ss = npair + np.argsort(src[npair:], kind="stable")
                perm = np.concatenate([ss, ps])
                src, la, wa, lb, wb = (src[perm], la[perm], wa[perm],
                                       lb[perm], wb[perm])
        else:
            la, wa = locs, ws
            lb = np.full(len(src), -1.0, loc_dtype)
            wb = np.zeros(len(src), np.float32)
        wins.append((src, la, wa, lb, wb))
        wmax = max(wmax, len(src))
    return wins, wmax


def _layout(wins, W_list):
    """Flat slot arrays with per-window chunk counts.  Window w's cw-th
    chunk sits at table column offs[w] + cw."""
    offs = np.concatenate([[0], np.cumsum(W_list)])
    cols = []
    for w, Wx in enumerate(W_list):
        cols.extend((w, cw) for cw in range(Wx))
    return _layout_cols(wins, cols)


def _layout_cols(wins, cols):
    """Flat slot arrays for an explicit column list of (window, cw)."""
    L = len(cols) * 128
    gidx = np.zeros(L, np.int16)
    loc = np.full(L, -1.0, np.float32)
    wsel = np.zeros(L, np.float32)
    locb = np.full(L, -1.0, np.float32)
    wselb = np.zeros(L, np.float32)
    for t, (w, cw) in enumerate(cols):
        src, la, wa, lb, wb = wins[w]
        i0, i1 = cw * 128, min((cw + 1) * 128, len(src))
        if i0 >= len(src):
            continue
        o, n = t * 128, i1 - i0
        gidx[o:o + n] = src[i0:i1]
        loc[o:o + n] = la[i0:i1]
        wsel[o:o + n] = wa[i0:i1]
        locb[o:o + n] = lb[i0:i1]
        wselb[o:o + n] = wb[i0:i1]
    return gidx, loc, wsel, locb, wselb


def _wrap_idx(flat):
    """int16 flat[i] -> [128, len/16] with value i at [i%16, i//16], replicated."""
    a = flat.reshape(-1, 16).T  # [16, L/16]
    return np.ascontiguousarray(np.tile(a, (8, 1)))


def _pack(flat, C):
    """flat[c*128+p] -> [128, C]"""
    return np.ascontiguousarray(flat.reshape(C, 128).T)


def _dedup_slots(srcs):
    """#gather slots for a window's source list after pairing."""
    if not len(srcs):
        return 0
    _, cnt = np.unique(srcs, return_counts=True)
    return int(((cnt + 1) // 2).sum())


def _repair_bins(bins, order, starts, idx_of_inc, cap=128, iters=400):
    """Greedy rebalance: move members out of the window with the most
    post-dedup slots into the one with the fewest (respecting the member
    cap), to minimize max slots per window."""
    srcs = [
        [idx_of_inc[order[starts[d]:starts[d + 1]]] for d in b] for b in bins
    ]

    def slots(w):
        return _dedup_slots(np.concatenate(srcs[w]) if srcs[w] else
                            np.zeros(0, np.int64))

    cur = [slots(w) for w in range(len(bins))]
    for _ in range(iters):
        hot = int(np.argmax(cur))
        order_cold = np.argsort(cur)
        moved = False
        for cold in order_cold:
            if cold == hot or len(bins[cold]) >= cap:
                continue
            # move the member with the smallest segment out of `hot`
            j = int(np.argmin([len(s) for s in srcs[hot]]))
            bins[cold].append(bins[hot].pop(j))
            srcs[cold].append(srcs[hot].pop(j))
            new_hot, new_cold = slots(hot), slots(cold)
            if max(new_hot, new_cold) >= cur[hot]:
                # revert: no improvement
                bins[hot].append(bins[cold].pop())
                srcs[hot].append(srcs[cold].pop())
                continue
            cur[hot], cur[cold] = new_hot, new_cold
            moved = True
            break
        if not moved:
            break
    return bins


def make_plan(v_idx, e_idx, e2v_weight, nv=NV, ne=NE, ch=CH, ncores=NCORES):
    P = Plan()
    P.nv, P.ne, P.ch, P.ncores = nv, ne, ch, ncores
    epc, vpc = ne // ncores, nv // ncores
    P.epc, P.vpc = epc, vpc

    order_e, deg_e, starts_e = _csr(e_idx, ne)
    order_v, deg_v, starts_v = _csr(v_idx, nv)
    inv_deg = np.zeros(ne, np.float32)
    nz = deg_e > 0
    inv_deg[nz] = (np.float32(1.0) / deg_e[nz].astype(np.float32))

    nb1 = -(-epc // 128)
    nb2 = -(-vpc // 128)
    v_of_inc = v_idx.astype(np.int64)
    e_of_inc = e_idx.astype(np.int64)
    # balance destinations across cores globally (assignment is free — pos /
    # vmap carry it), then binpack windows within each core and rebalance for
    # post-dedup slot counts.
    cores_e, _ = _binpack(np.arange(ne), deg_e, ncores, cap=nb1 * 128)
    bins1 = []
    for k in range(ncores):
        eids = np.asarray(cores_e[k])
        b, _ = _binpack(eids, deg_e[eids], nb1)
        bins1.append(_repair_bins(b, order_e, starts_e, v_of_inc))

    # p2 window count: an extra window can admit a smaller W2 (less gather
    # padding) once dedup shrinks the per-window loads — pick the best.
    best = None
    for nb2c in (nb2, nb2 + 1):
        cores_v, _ = _binpack(np.arange(nv), deg_v, ncores, cap=nb2c * 128)
        cand = []
        sizes = []
        for k in range(ncores):
            vids = np.asarray(cores_v[k])
            b, _ = _binpack(vids, deg_v[vids], nb2c)
            b = _repair_bins(b, order_v, starts_v, e_of_inc)
            cand.append(b)
            sizes.append(sorted(
                (-(-_dedup_slots(np.concatenate(
                    [e_of_inc[order_v[starts_v[d]:starts_v[d + 1]]]
                     for d in bb]) if bb else np.zeros(0, np.int64)) // 128)
                 for bb in b), reverse=True))
        C2c = sum(max(sizes[k][w] for k in range(ncores))
                  for w in range(nb2c))
        if best is None or C2c < best[1]:
            best = (nb2c, C2c, cand)
    nb2, _, bins2 = best
    P.NW1, P.NW2 = nb1, nb2

    # phase-1 windows (dedup within window) + edge position map.  Positions
    # are window-major (w, core, row) so each window's table slice can be
    # AllGathered independently as soon as its GEMM finishes.  Windows are
    # sorted largest-first per core: cross-core maxes align (less padding)
    # and the smallest window lands last (shorter phase tail).
    pos = np.zeros(ne, np.int64)
    wins1 = []
    w1max = 0
    for k in range(ncores):
        wins, wmax = _phase_windows(
            bins1[k], order_e, starts_e, v_idx.astype(np.int64),
            inv_deg[e_idx.astype(np.int64)])
        order = np.argsort([-len(w[0]) for w in wins], kind="stable")
        wins = [wins[i] for i in order]
        bins1[k] = [bins1[k][i] for i in order]
        wins1.append(wins)
        w1max = max(w1max, wmax)
        for w, bin_ids in enumerate(bins1[k]):
            for j, e in enumerate(bin_ids):
                if AG_SLICED:   # window-major: (w, core, row)
                    pos[e] = (w * ncores + k) * 128 + j
                else:           # core-major: AllGather rank concatenation
                    pos[e] = (k * nb1 + w) * 128 + j
    assert pos.max() < 32768
    P.W1_list = [
        max(-(-len(wins1[k][w][0]) // 128) for k in range(ncores))
        for w in range(nb1)
    ]
    P.W1 = max(P.W1_list)
    P.C1 = int(sum(P.W1_list))
    P.p1 = [_layout(wins, P.W1_list) for wins in wins1]

    # phase-2 windows + output row map. Softmax weights are fully normalized
    # on the host (exp / per-vertex sum), so the kernel needs no denominator
    # pass.
    expw = np.exp(e2v_weight.astype(np.float64))
    den = np.zeros(nv, np.float64)
    np.add.at(den, v_idx, expw)
    wnorm = (expw / den[v_idx.astype(np.int64)]).astype(np.float32)
    wins2 = []
    P.vmap = []
    for k in range(ncores):
        wins, _ = _phase_windows(
            bins2[k], order_v, starts_v, pos[e_idx.astype(np.int64)], wnorm,
            piece_order=AG_SLICED)
        order = np.argsort([-len(w[0]) for w in wins], kind="stable")
        wins = [wins[i] for i in order]
        bins2[k] = [bins2[k][i] for i in order]
        wins2.append(wins)
        vm = np.full(P.NW2 * 128, -1, np.int64)
        for w, bin_ids in enumerate(bins2[k]):
            vm[w * 128:w * 128 + len(bin_ids)] = bin_ids
        # deg-0 vertices never receive contributions; drop them from the
        # output map so any on-chip garbage (e.g. a stray +b) is discarded.
        vme = vm[vm >= 0]
        vm[vm >= 0] = np.where(deg_v[vme] > 0, vme, -1)
        P.vmap.append(vm)
    P.W2_list = [
        max(-(-len(wins2[k][w][0]) // 128) for k in range(ncores))
        for w in range(nb2)
    ]
    P.W2 = max(P.W2_list)
    P.C2 = int(sum(P.W2_list))
    # pre-block: chunk-0 of the first windows, but only if they reference
    # table piece 0 alone (so a gather over them can run during phase 1,
    # right after the first window's AllGather).  They form the leading
    # contiguous columns of the table layout.
    piece = ncores * 128
    npre = 0 if not AG_SLICED else min(15, nb2)
    npre_cap, npre = npre, 0
    for w in range(npre_cap):
        b0max = max(
            int(wins2[k][w][0][:128].max()) if len(wins2[k][w][0]) else 0
            for k in range(ncores))
        if b0max >= piece or P.W2_list[w] < 2:
            break
        npre += 1
    npre -= npre % 5            # whole grp-5 calls only
    P.npre2 = npre
    cols = [(w, 0) for w in range(npre)]
    for w in range(nb2):
        for cw in (range(1, P.W2_list[w]) if w < npre
                   else range(P.W2_list[w])):
            cols.append((w, cw))
    P.p2cols = cols
    P.p2 = [_layout_cols(wins, cols) for wins in wins2]
    return P


# ---------------------------------------------------------------- builder ---
def build_nc(P, bf16=GATHER_BF16, spmd=True, reps=1, grp=GRP, gbufs=5,
             nqueues=1, p1_dt=P1_DT, p2_dt=P2_DT, out_dt=OUT_DT, fuse=FUSE):
    import concourse.bacc as bacc
    import concourse.mybir as mybir
    import concourse.tile as tile

    f32 = mybir.dt.float32
    dt_g = mybir.dt.bfloat16 if bf16 else f32   # sel matrices + GEMM operands
    dt_p1 = _mydt(mybir, p1_dt)
    dt_p2 = _mydt(mybir, p2_dt)
    dt_out = _mydt(mybir, out_dt)
    eq, mul, mx, add = (mybir.AluOpType.is_equal, mybir.AluOpType.mult,
                        mybir.AluOpType.max, mybir.AluOpType.add)
    ch, KT = P.ch, P.ch // 128
    post = fuse == "post"

    nc = bacc.Bacc("TRN2", target_bir_lowering=False, debug=False,
                   num_devices=P.ncores if spmd else 1,
                   num_swdge_queues=nqueues)

    XT = nc.dram_tensor("xt", [P.nv, ch], dt_p1, kind="ExternalInput")
    WT = nc.dram_tensor("wt", [128, KT, ch], dt_g, kind="ExternalInput")
    BT = nc.dram_tensor("bt", [1, ch], dt_g, kind="ExternalInput")
    IOTA = nc.dram_tensor("iota", [128, 128], dt_g, kind="ExternalInput")
    IDENT = nc.dram_tensor("ident", [128, 128], dt_g, kind="ExternalInput")
    G1IDX = nc.dram_tensor("g1idx", [128, P.C1 * 8], mybir.dt.int16, kind="ExternalInput")
    P1TAB = nc.dram_tensor("p1tab", [128, 4, P.C1], f32, kind="ExternalInput")
    G2IDX = nc.dram_tensor("g2idx", [128, P.C2 * 8], mybir.dt.int16, kind="ExternalInput")
    P2TAB = nc.dram_tensor("p2tab", [128, 4, P.C2], f32, kind="ExternalInput")

    ner1 = P.NW1 * 128
    CCIN = nc.dram_tensor("ccin", [ner1, ch], dt_p2)
    CCOUT = nc.dram_tensor("ccout", [P.ncores * ner1, ch], dt_p2, addr_space="Shared")
    OUT = nc.dram_tensor("out", [P.NW2 * 128, ch], dt_out, kind="ExternalOutput")

    with tile.TileContext(nc) as tc, ExitStack() as ctx:
        const = ctx.enter_context(tc.tile_pool(name="const", bufs=1))
        gpool = ctx.enter_context(tc.tile_pool(name="g", bufs=gbufs))
        g2pool = ctx.enter_context(tc.tile_pool(name="g2", bufs=gbufs))
        prep = ctx.enter_context(tc.tile_pool(name="prep", bufs=1))
        selp = ctx.enter_context(tc.tile_pool(name="selp", bufs=8))
        psum = ctx.enter_context(tc.tile_pool(name="ps", bufs=2, space="PSUM"))
        sbp = ctx.enter_context(tc.tile_pool(name="sbp", bufs=2))
        pre_tiles = {}

        def cload(dram, shape, dt, tag, eng=None):
            t = const.tile(shape, dt, tag=tag)
            (eng or nc.sync).dma_start(t[:], dram[:])
            return t

        # p1-critical tables first (SP ring); the rest go on the ACT ring so
        # they never delay the first gather.  g1idx leads: the first gather
        # needs only it, iota/p1tab are for the (later) first sel build.
        g1idx_t = cload(G1IDX, [128, P.C1 * 8], mybir.dt.int16, "g1idx")
        iota_t = cload(IOTA, [128, 128], dt_g, "iota")
        p1tab_t = cload(P1TAB, [128, 4, P.C1], f32, "p1tab")
        eloc1_t, wsel1_t = p1tab_t[:, 0, :], p1tab_t[:, 1, :]
        eloc1b_t, wsel1b_t = p1tab_t[:, 2, :], p1tab_t[:, 3, :]
        wt_t = cload(WT, [128, KT, ch], dt_g, "wt", eng=nc.scalar)
        bt_t = cload(BT, [1, ch], dt_g, "bt", eng=nc.scalar)
        ident_t = cload(IDENT, [128, 128], dt_g, "ident", eng=nc.scalar)
        g2idx_t = cload(G2IDX, [128, P.C2 * 8], mybir.dt.int16, "g2idx",
                        eng=nc.scalar)
        p2tab_t = cload(P2TAB, [128, 4, P.C2], f32, "p2tab", eng=nc.scalar)
        vloc2_t, w2raw_t = p2tab_t[:, 0, :], p2tab_t[:, 1, :]
        vloc2b_t, w2rawb_t = p2tab_t[:, 2, :], p2tab_t[:, 3, :]
        ones1_t = const.tile([1, 128], dt_g, tag="ones1")
        nc.vector.memset(ones1_t[:], 1.0)

        # ---------------- gather + one/two-hot reduce ----------------------
        def agg_phase(src_ap, gidx_t, loc_t, w_t, locb_t, wb_t, is2, W_list,
                      gtag, chunk_cb, win_cb, dt_tab, src_sel=None,
                      pre_tiles=None, pool=None, cols=None):
            """Consumption iterates (window, cw) order; `cols` gives each
            table column's (window, cw) — prefetched pre-block columns may
            sit outside their window's run."""
            pre_tiles = pre_tiles or {}
            pool = pool or gpool
            if cols is None:
                cols = [(w, cw) for w, Wx in enumerate(W_list)
                        for cw in range(Wx)]
            C = len(cols)
            pos_order = sorted(range(C), key=lambda t: cols[t])
            # calls = runs of consecutive non-prefetched table columns
            calls = []
            run = []
            for t in range(C):
                if t in pre_tiles or len(run) == grp:
                    if run:
                        calls.append((run[0], len(run)))
                    run = []
                if t not in pre_tiles:
                    run.append(t)
            if run:
                calls.append((run[0], len(run)))
            call_of = {}
            for g0, n in calls:
                for j in range(n):
                    call_of[g0 + j] = (g0, n)
            tiles = {}
            pw = None
            for t in pos_order:
                if t in call_of and call_of[t][0] == t:
                    g0, n = call_of[t]
                    gt_new = pool.tile([128, grp, ch], dt_tab, tag=gtag)
                    src = src_sel(g0, n) if src_sel is not None else src_ap
                    nc.gpsimd.dma_gather(
                        gt_new[:, 0:n, :], src,
                        gidx_t[:, g0 * 8:(g0 + n) * 8],
                        n * 128, n * 128, ch,
                        queue_num=(g0 // grp) % nqueues)
                    tiles[g0] = gt_new
                w, cw = cols[t]
                if t in pre_tiles:
                    gt, j = pre_tiles[t]
                else:
                    g0, n = call_of[t]
                    gt, j = tiles[g0], t - g0
                sel = selp.tile([128, 128], dt_g, tag="sel")
                nc.vector.tensor_scalar(
                    sel[:], iota_t[:], loc_t[:, t:t + 1], w_t[:, t:t + 1],
                    op0=eq, op1=mul)
                if is2[t]:  # dedup chunk: add the second hot
                    selb = selp.tile([128, 128], dt_g, tag="selb")
                    nc.vector.tensor_scalar(
                        selb[:], iota_t[:], locb_t[:, t:t + 1],
                        wb_t[:, t:t + 1], op0=eq, op1=mul)
                    sel2 = selp.tile([128, 128], dt_g, tag="sel2")
                    nc.vector.tensor_tensor(sel2[:], sel[:], selb[:], op=add)
                    sel = sel2
                if cw == 0:
                    pw = psum.tile([128, ch], f32, tag="win")
                last = cw == W_list[w] - 1
                chunk_cb(pw, sel, gt, j, w, cw, last)
                if last:
                    win_cb(pw, w)

        def p1_chunk(pw, sel, gt, j, w, cw, last):
            nc.tensor.matmul(pw[:], sel[:], gt[:, j, :],
                             start=(cw == 0), stop=last)

        def gemm_bias(src_t, dst_psum):
            """dst[v/e, co] = src^T blocks @ W.T + 1^T b (K=1 bias matmul)."""
            for k in range(KT):
                nc.tensor.matmul(dst_psum[:], src_t[:, k, :], wt_t[:, k, :],
                                 start=(k == 0), stop=False)
            nc.tensor.matmul(dst_psum[:], ones1_t[:], bt_t[:],
                             start=False, stop=True)

        def transpose_blocks(pw, tag):
            """psum [128, ch] f32 -> sbuf [128, KT, 128] dt_g transposed.
            Copies are per-128-block so transpose k pipelines with copy k+1."""
            tT_w = sbp.tile([128, KT, 128], dt_g, tag=tag + "T", name=tag + "T")
            for k in range(KT):
                twk = sbp.tile([128, 128], dt_g, tag=tag + "f", name=tag + "f")
                nc.vector.tensor_copy(twk[:], pw[:, k * 128:(k + 1) * 128])
                pt = psum.tile([128, 128], dt_g, tag="aux", name="pt")
                nc.tensor.transpose(pt[:], twk[:], ident_t[:])
                nc.vector.tensor_copy(tT_w[:, k, :], pt[:])
            return tT_w

        def p1_win(pw, w):
            # window w's edge rows are complete: ship its CCIN slice and
            # immediately AllGather that window's table piece, overlapping
            # the collective with the remaining p1 windows.
            xep = sbp.tile([128, ch], dt_p2, tag="xep", name="xep")
            if post:
                # raw table, scaled for fp8 range; GEMM happens after p2 agg
                nc.vector.tensor_scalar(xep[:], pw[:], float(P2_SCALE), None,
                                        op0=mul)
            else:
                xeT_w = transpose_blocks(pw, "xe")
                pg = psum.tile([128, ch], f32, tag="gemm", name="pg")
                gemm_bias(xeT_w, pg)
                nc.vector.tensor_copy(xep[:], pg[:])
            nc.sync.dma_start(CCIN[w * 128:(w + 1) * 128, :], xep[:])
            if AG_SLICED:
                o = w * P.ncores * 128
                if spmd:
                    nc.gpsimd.collective_compute(
                        "AllGather", mybir.AluOpType.bypass,
                        replica_groups=[list(range(P.ncores))],
                        ins=[CCIN[w * 128:(w + 1) * 128, :]],
                        outs=[CCOUT[o:o + P.ncores * 128, :]])
                else:  # single-core stand-in for the window AllGather
                    nc.sync.dma_start(CCOUT[o:o + 128, :],
                                      CCIN[w * 128:(w + 1) * 128, :])
            elif w == P.NW1 - 1:
                if spmd:
                    nc.gpsimd.collective_compute(
                        "AllGather", mybir.AluOpType.bypass,
                        replica_groups=[list(range(P.ncores))],
                        ins=[CCIN[:]], outs=[CCOUT[:]])
                else:
                    nc.sync.dma_start(CCOUT[0:ner1, :], CCIN[:])
            # prefetch the p2 pre-block (piece-0-only chunk-0 columns) during
            # p1, one window after its AllGather piece was triggered — soaks
            # p1's spare DMA bandwidth and thins p2's DMA-bound span
            if w >= 1 and (w - 1) * grp < P.npre2:
                g0 = (w - 1) * grp
                n = min(grp, P.npre2 - g0)
                gt = prep.tile([128, grp, ch], dt_p2, tag=f"pre{g0}")
                nc.gpsimd.dma_gather(
                    gt[:, 0:n, :], p2_src(g0, n),
                    g2idx_t[:, g0 * 8:(g0 + n) * 8],
                    n * 128, n * 128, ch, queue_num=0)
                for j in range(n):
                    pre_tiles[g0 + j] = (gt, j)

        def p2_chunk(pw, sel, gt, j, w, cw, last):
            nc.tensor.matmul(pw[:], sel[:], gt[:, j, :],
                             start=(cw == 0), stop=last)

        def p2_win(pw, w):
            if post:
                awT = transpose_blocks(pw, "aw")
                po = psum.tile([128, ch], f32, tag="gemm", name="po")
                gemm_bias(awT, po)
                pw = po
            # weights pre-normalized on host: just relu + store
            ow = sbp.tile([128, ch], dt_out, tag="ow", name="ow")
            nc.vector.tensor_scalar(ow[:], pw[:], 1.0, 0.0, op0=mul, op1=mx)
            nc.sync.dma_start(OUT[w * 128:(w + 1) * 128, :], ow[:])

        # chunks that contain any dedup pair need the second sel pass; the
        # union over cores keeps the SPMD program identical on every core.
        is2_1 = np.zeros(P.C1, bool)
        is2_2 = np.zeros(P.C2, bool)
        maxrow2 = np.zeros(P.C2, np.int64)
        for k in range(P.ncores):
            is2_1 |= (_pack(P.p1[k][3], P.C1) >= 0).any(axis=0)
            is2_2 |= (_pack(P.p2[k][3], P.C2) >= 0).any(axis=0)
            maxrow2 = np.maximum(
                maxrow2, _pack(P.p2[k][0], P.C2).astype(np.int64).max(axis=0))

        piece = P.ncores * 128
        def p2_src(g0, n):
            # prefix slice of the edge table covering every row this gather
            # call touches, so it only waits on the AllGather pieces it needs
            pieces = int(maxrow2[g0:g0 + n].max()) // piece + 1
            if not AG_SLICED or pieces >= P.NW1:
                return CCOUT[:]
            return CCOUT[0:pieces * piece, :]

        # p2 prefetch schedule: at p1 window boundary w we may issue gathers
        # for p2 chunks that only need table pieces < w (their AllGather was
        # triggered a full window earlier).  Earliest-consumed chunks first.
        bound = (maxrow2 // piece + 1).astype(int)   # pieces needed per chunk
        pre_sched = {w: [] for w in range(1, P.NW1)}
        if AG_SLICED and PRE_PER_B > 0:
            taken = set()
            for w in range(1, P.NW1):
                for c in range(P.C2):
                    if len(pre_sched[w]) >= PRE_PER_B:
                        break
                    if c not in taken and bound[c] <= w:
                        pre_sched[w].append(c)
                        taken.add(c)

        for _rep in range(reps):
            pre_tiles.clear()
            agg_phase(XT[:], g1idx_t, eloc1_t, wsel1_t, eloc1b_t, wsel1b_t,
                      is2_1, P.W1_list, "g1", p1_chunk, p1_win, dt_p1)

            # phase 2: e2v aggregation (sel weights pre-normalized on host)
            agg_phase(CCOUT[:], g2idx_t, vloc2_t, w2raw_t, vloc2b_t, w2rawb_t,
                      is2_2, P.W2_list, "g2", p2_chunk, p2_win, dt_p2,
                      src_sel=p2_src, pre_tiles=pre_tiles, pool=g2pool,
                      cols=P.p2cols)

    nc.compile()
    return nc


# ------------------------------------------------------------------ runner ---
def make_in_maps(P, X, W, b, bf16=GATHER_BF16, p1_dt=P1_DT, fuse=FUSE):
    npdt = ml_dtypes.bfloat16 if bf16 else np.float32
    np_p1 = _npdt(p1_dt)
    s1 = P1_SCALE if p1_dt == "f8" else 1.0
    s2 = P2_SCALE if fuse == "post" else 1.0
    KT = P.ch // 128
    xt = np.ascontiguousarray((X * s1).astype(np_p1))
    wt = np.ascontiguousarray(
        W.T.reshape(KT, 128, P.ch).transpose(1, 0, 2).astype(npdt))
    bt = np.ascontiguousarray(b.astype(npdt).reshape(1, P.ch))
    iota = np.ascontiguousarray(
        np.broadcast_to(np.arange(128, dtype=npdt), (128, 128)))
    ident = np.eye(128, dtype=npdt)

    def tb(flat, C, s=1.0):
        return _pack(flat, C) / np.float32(s)

    in_maps = []
    for k in range(P.ncores):
        g1, l1, w1, l1b, w1b = P.p1[k]
        g2, l2, w2, l2b, w2b = P.p2[k]
        p1tab = np.ascontiguousarray(np.stack(
            [tb(l1, P.C1), tb(w1, P.C1, s1), tb(l1b, P.C1), tb(w1b, P.C1, s1)],
            axis=1))
        p2tab = np.ascontiguousarray(np.stack(
            [tb(l2, P.C2), tb(w2, P.C2, s2), tb(l2b, P.C2), tb(w2b, P.C2, s2)],
            axis=1))
        in_maps.append({
            "xt": xt, "wt": wt, "bt": bt, "iota": iota, "ident": ident,
            "g1idx": _wrap_idx(g1), "p1tab": p1tab,
            "g2idx": _wrap_idx(g2), "p2tab": p2tab,
        })
    return in_maps


def assemble(P, shards):
    out = np.zeros((P.nv, P.ch), np.float32)
    for k in range(P.ncores):
        vm = P.vmap[k]
        m = vm >= 0
        out[vm[m]] = shards[k][m].astype(np.float32)
    return out


_nc_cache = {}


def kernel(X, W, b, e2v_weight, v_idx, e_idx):
    global _last_results
    from concourse.bass_utils import run_bass_kernel_spmd

    P = make_plan(v_idx, e_idx, e2v_weight)
    key = (P.C1, P.C2, P.W1, P.W2, GATHER_BF16, P1_DT, P2_DT, OUT_DT, FUSE,
           AG_SLICED)
    if key not in _nc_cache:
        _nc_cache[key] = build_nc(P)
    nc = _nc_cache[key]
    in_maps = make_in_maps(P, X, W, b)
    res = run_bass_kernel_spmd(nc, in_maps, list(range(P.ncores)), trace=TRACE)
    _last_results = res
    shards = [res.results[k]["out"] for k in range(P.ncores)]
    return assemble(P, shards)



# revision 110
# speedup vs baseline: 1.0596x; 1.0085x over previous
"""HGNNPConv Trainium2 kernel (8 NeuronCores, SPMD).

Math (equivalent reformulation of the reference):
  Xe_raw[e] = mean_{i: e_idx[i]=e} X[v_idx[i]]              (v2e, softmax of ones = 1/deg)
  Xe_p      = Xe_raw @ W.T + b                              (GEMM on 4000 edges, not 20000 verts)
  Xv[v]     = sum_i wn_i * Xe_p[e_idx[i]],  wn_i = exp(w_i)/sum_{v} exp(w)
              (wn precomputed on host -> no on-chip denominator pass)
  out       = relu(Xv)
Deg-0 vertices are masked host-side in assemble(); empty edges get a spurious
+b in Xe_p but are never referenced downstream.

Sharding: edges and vertices are binpacked across the 8 cores (balancing
post-dedup gather slots).  Phase 1 aggregates by destination edge, runs the
edge-level GEMM per 128-edge window (bias folded in as a K=1 matmul), and
AllGathers each window's table slice as soon as it is ready (overlapping the
collective with the remaining phase-1 work).  Phase 2 aggregates by
destination vertex; each gather call's source AP is a prefix slice of the
table covering only the AllGather pieces it needs.

Per-destination-window weighted one-hot selection matrices (built on DVE
from a bf16 iota) reduce gathered rows on the PE into PSUM.  Incidences
sharing a source row within a window are deduplicated into one gathered slot
with a two-hot sel column (~11% fewer gather bytes; window binpacking is
rebalanced for post-dedup slot counts, phase 2 uses 21 windows x 7 chunks
instead of 20 x 8, and per-window chunk counts are variable — windows are
sorted largest-first so cross-core padding aligns and each phase ends on its
smallest window).

Dtypes: phase-1 gather table fp8-e3m4 (X pre-scaled by 2: worst-case quant
error halves vs e4m3 and the GEMM averages it out), phase-2 table bf16
(absmax-norm tolerance rules out fp8 post-GEMM), output bf16 (upcast on
host).  End-to-end rel err 1.35e-2 vs the 2e-2 gate.
"""

import os
from contextlib import ExitStack

import numpy as np
import ml_dtypes

# ---------------------------------------------------------------- config ---
NCORES = 8
NV, NE, NNZ, CH = 20000, 4000, 160000, 512
GATHER_BF16 = os.environ.get("KERNEL_F32", "") == ""  # bf16 tables+matmuls by default
P1_DT = os.environ.get("KERNEL_P1_DT", "f8")   # phase-1 gather table dtype
P2_DT = os.environ.get("KERNEL_P2_DT", "bf16")  # phase-2 gather table dtype
OUT_DT = os.environ.get("KERNEL_OUT_DT", "bf16")
FUSE = os.environ.get("KERNEL_FUSE", "pre")    # "post": GEMM after p2 agg
P1_SCALE = 2.0   # X pre-scale for fp8-e3m4 range use (exact power of 2)
P2_SCALE = 8.0   # Xe_raw pre-scale for the fp8 edge table (post mode)
GRP = 5          # gather chunks (of 128 idxs) per dma_gather call
AG_SLICED = os.environ.get("KERNEL_AG_SLICED", "1") != ""  # per-window AllGather
PRE_PER_B = int(os.environ.get("KERNEL_PRE_PER_B", "0"))  # p2 prefetches per p1 window
CW_MAJOR = os.environ.get("KERNEL_CW_MAJOR", "") != ""  # cw-major p2 table layout
TRACE = os.environ.get("BASS_TRACE", "") != ""


def _mydt(mybir, name):
    return {"f8": mybir.dt.float8e3, "bf16": mybir.dt.bfloat16,
            "f32": mybir.dt.float32}[name]


def _npdt(name):
    return {"f8": ml_dtypes.float8_e3m4, "bf16": ml_dtypes.bfloat16,
            "f32": np.float32}[name]

_last_results = None   # BassKernelResults of the most recent run (for test.py)


# ------------------------------------------------------------------- plan ---
class Plan:
    pass


def _binpack(ids, degs, nbins, cap=128):
    """Pack `ids` into `nbins` bins of <=cap items, balancing sum(degs)."""
    import heapq

    order = np.argsort(-degs, kind="stable")
    bins = [[] for _ in range(nbins)]
    loads = [0] * nbins
    heap = [(0, b) for b in range(nbins)]
    heapq.heapify(heap)
    for t in order:
        popped = []
        while True:
            load, b = heapq.heappop(heap)
            if len(bins[b]) < cap:
                break
            popped.append((load, b))
        for p in popped:
            heapq.heappush(heap, p)
        bins[b].append(int(ids[t]))
        loads[b] = load + int(degs[t])
        heapq.heappush(heap, (loads[b], b))
    return bins, loads


def _csr(idx, n):
    order = np.argsort(idx, kind="stable").astype(np.int64)
    deg = np.bincount(idx, minlength=n).astype(np.int64)
    starts = np.zeros(n + 1, np.int64)
    np.cumsum(deg, out=starts[1:])
    return order, deg, starts


def _pair_window(src, locs, ws):
    """Greedy within-window dedup: incidences sharing a source row become one
    gathered slot with two (loc, w) hots.  Returns (src', loc_a, w_a, loc_b,
    w_b) with pair slots FIRST; singles have loc_b = -1 / w_b = 0."""
    order = np.argsort(src, kind="stable")
    src, locs, ws = src[order], locs[order], ws[order]
    pa, pb, sg = [], [], []
    i, n = 0, len(src)
    while i < n:
        j = i
        while j < n and src[j] == src[i]:
            j += 1
        k = i
        while k + 1 < j:
            pa.append(k); pb.append(k + 1); k += 2
        if k < j:
            sg.append(k)
        i = j
    pa, pb, sg = np.array(pa, np.int64), np.array(pb, np.int64), np.array(sg, np.int64)
    src2 = np.concatenate([src[pa], src[sg]]) if len(pa) else src[sg]
    la = np.concatenate([locs[pa], locs[sg]]) if len(pa) else locs[sg]
    wa = np.concatenate([ws[pa], ws[sg]]) if len(pa) else ws[sg]
    lb = np.concatenate([locs[pb], np.full(len(sg), -1.0, locs.dtype)]) if len(pa) \
        else np.full(len(sg), -1.0, locs.dtype)
    wb = np.concatenate([ws[pb], np.zeros(len(sg), ws.dtype)]) if len(pa) \
        else np.zeros(len(sg), ws.dtype)
    return src2, la, wa, lb, wb, len(pa)


def _phase_windows(bins_per_core, order, starts, idx_of_inc, w_of_inc,
                   loc_dtype=np.float32, pair=True, piece_order=False):
    """Per-window slot lists for one core of one phase, after source dedup.

    Returns (wins, wmax): wins[w] = (src, loc_a, w_a, loc_b, w_b).  By
    default pair slots (loc_b >= 0) come first within each window; with
    piece_order=True, singles come first sorted by source id (so early
    chunks only reference low table pieces) and pair slots go last.
    """
    wins = []
    wmax = 0
    for bin_ids in bins_per_core:
        incs = []
        locs = []
        for j, d in enumerate(bin_ids):
            seg = order[starts[d]:starts[d + 1]]
            incs.append(seg)
            locs.append(np.full(len(seg), j, loc_dtype))
        incs = np.concatenate(incs) if incs else np.zeros(0, np.int64)
        locs = np.concatenate(locs) if locs else np.zeros(0, loc_dtype)
        src = idx_of_inc[incs]
        ws = w_of_inc[incs].astype(np.float32)
        if pair and len(src):
            src, la, wa, lb, wb, npair = _pair_window(src, locs, ws)
            if piece_order:
                ps = np.argsort(src[:npair], kind="stable")
                ss = npair + np.argsort(src[npair:], kind="stable")
                perm = np.concatenate([ss, ps])
                src, la, wa, lb, wb = (src[perm], la[perm], wa[perm],
                                       lb[perm], wb[perm])
        else:
            la, wa = locs, ws
            lb = np.full(len(src), -1.0, loc_dtype)
            wb = np.zeros(len(src), np.float32)
        wins.append((src, la, wa, lb, wb))
        wmax = max(wmax, len(src))
    return wins, wmax


def _layout(wins, W_list):
    """Flat slot arrays with per-window chunk counts.  Window w's cw-th
    chunk sits at table column offs[w] + cw."""
    offs = np.concatenate([[0], np.cumsum(W_list)])
    cols = []
    for w, Wx in enumerate(W_list):
        cols.extend((w, cw) for cw in range(Wx))
    return _layout_cols(wins, cols)


def _layout_cols(wins, cols):
    """Flat slot arrays for an explicit column list of (window, cw)."""
    L = len(cols) * 128
    gidx = np.zeros(L, np.int16)
    loc = np.full(L, -1.0, np.float32)
    wsel = np.zeros(L, np.float32)
    locb = np.full(L, -1.0, np.float32)
    wselb = np.zeros(L, np.float32)
    for t, (w, cw) in enumerate(cols):
        src, la, wa, lb, wb = wins[w]
        i0, i1 = cw * 128, min((cw + 1) * 128, len(src))
        if i0 >= len(src):
            continue
        o, n = t * 128, i1 - i0
        gidx[o:o + n] = src[i0:i1]
        loc[o:o + n] = la[i0:i1]
        wsel[o:o + n] = wa[i0:i1]
        locb[o:o + n] = lb[i0:i1]
        wselb[o:o + n] = wb[i0:i1]
    return gidx, loc, wsel, locb, wselb


def _wrap_idx(flat):
    """int16 flat[i] -> [128, len/16] with value i at [i%16, i//16], replicated."""
    a = flat.reshape(-1, 16).T  # [16, L/16]
    return np.ascontiguousarray(np.tile(a, (8, 1)))


def _pack(flat, C):
    """flat[c*128+p] -> [128, C]"""
    return np.ascontiguousarray(flat.reshape(C, 128).T)


def _dedup_slots(srcs):
    """#gather slots for a window's source list after pairing."""
    if not len(srcs):
        return 0
    _, cnt = np.unique(srcs, return_counts=True)
    return int(((cnt + 1) // 2).sum())


def _repair_bins(bins, order, starts, idx_of_inc, cap=128, iters=400):
    """Greedy rebalance: move members out of the window with the most
    post-dedup slots into the one with the fewest (respecting the member
    cap), to minimize max slots per window."""
    srcs = [
        [idx_of_inc[order[starts[d]:starts[d + 1]]] for d in b] for b in bins
    ]

    def slots(w):
        return _dedup_slots(np.concatenate(srcs[w]) if srcs[w] else
                            np.zeros(0, np.int64))

    cur = [slots(w) for w in range(len(bins))]
    for _ in range(iters):
        hot = int(np.argmax(cur))
        order_cold = np.argsort(cur)
        moved = False
        for cold in order_cold:
            if cold == hot or len(bins[cold]) >= cap:
                continue
            # move the member with the smallest segment out of `hot`
            j = int(np.argmin([len(s) for s in srcs[hot]]))
            bins[cold].append(bins[hot].pop(j))
            srcs[cold].append(srcs[hot].pop(j))
            new_hot, new_cold = slots(hot), slots(cold)
            if max(new_hot, new_cold) >= cur[hot]:
                # revert: no improvement
                bins[hot].append(bins[cold].pop())
                srcs[hot].append(srcs[cold].pop())
                continue
            cur[hot], cur[cold] = new_hot, new_cold
            moved = True
            break
        if not moved:
            break
    return bins


def make_plan(v_idx, e_idx, e2v_weight, nv=NV, ne=NE, ch=CH, ncores=NCORES):
    P = Plan()
    P.nv, P.ne, P.ch, P.ncores = nv, ne, ch, ncores
    epc, vpc = ne // ncores, nv // ncores
    P.epc, P.vpc = epc, vpc

    order_e, deg_e, starts_e = _csr(e_idx, ne)
    order_v, deg_v, starts_v = _csr(v_idx, nv)
    inv_deg = np.zeros(ne, np.float32)
    nz = deg_e > 0
    inv_deg[nz] = (np.float32(1.0) / deg_e[nz].astype(np.float32))

    nb1 = -(-epc // 128)
    nb2 = -(-vpc // 128)
    v_of_inc = v_idx.astype(np.int64)
    e_of_inc = e_idx.astype(np.int64)
    # balance destinations across cores globally (assignment is free — pos /
    # vmap carry it), then binpack windows within each core and rebalance for
    # post-dedup slot counts.
    cores_e, _ = _binpack(np.arange(ne), deg_e, ncores, cap=nb1 * 128)
    bins1 = []
    for k in range(ncores):
        eids = np.asarray(cores_e[k])
        b, _ = _binpack(eids, deg_e[eids], nb1)
        bins1.append(_repair_bins(b, order_e, starts_e, v_of_inc))

    # p2 window count: an extra window can admit a smaller W2 (less gather
    # padding) once dedup shrinks the per-window loads — pick the best.
    best = None
    for nb2c in (nb2, nb2 + 1):
        cores_v, _ = _binpack(np.arange(nv), deg_v, ncores, cap=nb2c * 128)
        cand = []
        sizes = []
        for k in range(ncores):
            vids = np.asarray(cores_v[k])
            b, _ = _binpack(vids, deg_v[vids], nb2c)
            b = _repair_bins(b, order_v, starts_v, e_of_inc)
            cand.append(b)
            sizes.append(sorted(
                (-(-_dedup_slots(np.concatenate(
                    [e_of_inc[order_v[starts_v[d]:starts_v[d + 1]]]
                     for d in bb]) if bb else np.zeros(0, np.int64)) // 128)
                 for bb in b), reverse=True))
        C2c = sum(max(sizes[k][w] for k in range(ncores))
                  for w in range(nb2c))
        if best is None or C2c < best[1]:
            best = (nb2c, C2c, cand)
    nb2, _, bins2 = best
    P.NW1, P.NW2 = nb1, nb2

    # phase-1 windows (dedup within window) + edge position map.  Positions
    # are window-major (w, core, row) so each window's table slice can be
    # AllGathered independently as soon as its GEMM finishes.  Windows are
    # sorted largest-first per core: cross-core maxes align (less padding)
    # and the smallest window lands last (shorter phase tail).
    pos = np.zeros(ne, np.int64)
    wins1 = []
    w1max = 0
    for k in range(ncores):
        wins, wmax = _phase_windows(
            bins1[k], order_e, starts_e, v_idx.astype(np.int64),
            inv_deg[e_idx.astype(np.int64)])
        order = np.argsort([-len(w[0]) for w in wins], kind="stable")
        wins = [wins[i] for i in order]
        bins1[k] = [bins1[k][i] for i in order]
        wins1.append(wins)
        w1max = max(w1max, wmax)
        for w, bin_ids in enumerate(bins1[k]):
            for j, e in enumerate(bin_ids):
                if AG_SLICED:   # window-major: (w, core, row)
                    pos[e] = (w * ncores + k) * 128 + j
                else:           # core-major: AllGather rank concatenation
                    pos[e] = (k * nb1 + w) * 128 + j
    assert pos.max() < 32768
    P.W1_list = [
        max(-(-len(wins1[k][w][0]) // 128) for k in range(ncores))
        for w in range(nb1)
    ]
    P.W1 = max(P.W1_list)
    P.C1 = int(sum(P.W1_list))
    P.p1 = [_layout(wins, P.W1_list) for wins in wins1]

    # phase-2 windows + output row map. Softmax weights are fully normalized
    # on the host (exp / per-vertex sum), so the kernel needs no denominator
    # pass.
    expw = np.exp(e2v_weight.astype(np.float64))
    den = np.zeros(nv, np.float64)
    np.add.at(den, v_idx, expw)
    wnorm = (expw / den[v_idx.astype(np.int64)]).astype(np.float32)
    wins2 = []
    P.vmap = []
    for k in range(ncores):
        wins, _ = _phase_windows(
            bins2[k], order_v, starts_v, pos[e_idx.astype(np.int64)], wnorm,
            piece_order=AG_SLICED)
        order = np.argsort([-len(w[0]) for w in wins], kind="stable")
        wins = [wins[i] for i in order]
        bins2[k] = [bins2[k][i] for i in order]
        wins2.append(wins)
        vm = np.full(P.NW2 * 128, -1, np.int64)
        for w, bin_ids in enumerate(bins2[k]):
            vm[w * 128:w * 128 + len(bin_ids)] = bin_ids
        # deg-0 vertices never receive contributions; drop them from the
        # output map so any on-chip garbage (e.g. a stray +b) is discarded.
        vme = vm[vm >= 0]
        vm[vm >= 0] = np.where(deg_v[vme] > 0, vme, -1)
        P.vmap.append(vm)
    P.W2_list = [
        max(-(-len(wins2[k][w][0]) // 128) for k in range(ncores))
        for w in range(nb2)
    ]
    P.W2 = max(P.W2_list)
    P.C2 = int(sum(P.W2_list))
    # pre-block: chunk-0 of the first windows, but only if they reference
    # table piece 0 alone (so a gather over them can run during phase 1,
    # right after the first window's AllGather).  They form the leading
    # contiguous columns of the table layout.
    piece = ncores * 128
    npre = 0 if not AG_SLICED else min(20, nb2)
    npre_cap, npre = npre, 0
    for w in range(npre_cap):
        b0max = max(
            int(wins2[k][w][0][:128].max()) if len(wins2[k][w][0]) else 0
            for k in range(ncores))
        if b0max >= piece or P.W2_list[w] < 2:
            break
        npre += 1
    npre -= npre % 5            # whole grp-5 calls only
    P.npre2 = npre
    cols = [(w, 0) for w in range(npre)]
    for w in range(nb2):
        for cw in (range(1, P.W2_list[w]) if w < npre
                   else range(P.W2_list[w])):
            cols.append((w, cw))
    P.p2cols = cols
    P.p2 = [_layout_cols(wins, cols) for wins in wins2]
    return P


# ---------------------------------------------------------------- builder ---
def build_nc(P, bf16=GATHER_BF16, spmd=True, reps=1, grp=GRP, gbufs=5,
             nqueues=1, p1_dt=P1_DT, p2_dt=P2_DT, out_dt=OUT_DT, fuse=FUSE):
    import concourse.bacc as bacc
    import concourse.mybir as mybir
    import concourse.tile as tile

    f32 = mybir.dt.float32
    dt_g = mybir.dt.bfloat16 if bf16 else f32   # sel matrices + GEMM operands
    dt_p1 = _mydt(mybir, p1_dt)
    dt_p2 = _mydt(mybir, p2_dt)
    dt_out = _mydt(mybir, out_dt)
    eq, mul, mx, add = (mybir.AluOpType.is_equal, mybir.AluOpType.mult,
                        mybir.AluOpType.max, mybir.AluOpType.add)
    ch, KT = P.ch, P.ch // 128
    post = fuse == "post"

    nc = bacc.Bacc("TRN2", target_bir_lowering=False, debug=False,
                   num_devices=P.ncores if spmd else 1,
                   num_swdge_queues=nqueues)

    XT = nc.dram_tensor("xt", [P.nv, ch], dt_p1, kind="ExternalInput")
    WT = nc.dram_tensor("wt", [128, KT, ch], dt_g, kind="ExternalInput")
    BT = nc.dram_tensor("bt", [1, ch], dt_g, kind="ExternalInput")
    IOTA = nc.dram_tensor("iota", [128, 128], dt_g, kind="ExternalInput")
    IDENT = nc.dram_tensor("ident", [128, 128], dt_g, kind="ExternalInput")
    G1IDX = nc.dram_tensor("g1idx", [128, P.C1 * 8], mybir.dt.int16, kind="ExternalInput")
    P1TAB = nc.dram_tensor("p1tab", [128, 4, P.C1], f32, kind="ExternalInput")
    G2IDX = nc.dram_tensor("g2idx", [128, P.C2 * 8], mybir.dt.int16, kind="ExternalInput")
    P2TAB = nc.dram_tensor("p2tab", [128, 4, P.C2], f32, kind="ExternalInput")

    ner1 = P.NW1 * 128
    CCIN = nc.dram_tensor("ccin", [ner1, ch], dt_p2)
    CCOUT = nc.dram_tensor("ccout", [P.ncores * ner1, ch], dt_p2, addr_space="Shared")
    OUT = nc.dram_tensor("out", [P.NW2 * 128, ch], dt_out, kind="ExternalOutput")

    with tile.TileContext(nc) as tc, ExitStack() as ctx:
        const = ctx.enter_context(tc.tile_pool(name="const", bufs=1))
        gpool = ctx.enter_context(tc.tile_pool(name="g", bufs=gbufs))
        g2pool = ctx.enter_context(tc.tile_pool(name="g2", bufs=gbufs))
        prep = ctx.enter_context(tc.tile_pool(name="prep", bufs=1))
        selp = ctx.enter_context(tc.tile_pool(name="selp", bufs=8))
        psum = ctx.enter_context(tc.tile_pool(name="ps", bufs=2, space="PSUM"))
        sbp = ctx.enter_context(tc.tile_pool(name="sbp", bufs=2))
        pre_tiles = {}

        def cload(dram, shape, dt, tag, eng=None):
            t = const.tile(shape, dt, tag=tag)
            (eng or nc.sync).dma_start(t[:], dram[:])
            return t

        # p1-critical tables first (SP ring); the rest go on the ACT ring so
        # they never delay the first gather.  g1idx leads: the first gather
        # needs only it, iota/p1tab are for the (later) first sel build.
        g1idx_t = cload(G1IDX, [128, P.C1 * 8], mybir.dt.int16, "g1idx")
        iota_t = cload(IOTA, [128, 128], dt_g, "iota")
        p1tab_t = cload(P1TAB, [128, 4, P.C1], f32, "p1tab")
        eloc1_t, wsel1_t = p1tab_t[:, 0, :], p1tab_t[:, 1, :]
        eloc1b_t, wsel1b_t = p1tab_t[:, 2, :], p1tab_t[:, 3, :]
        wt_t = cload(WT, [128, KT, ch], dt_g, "wt", eng=nc.scalar)
        bt_t = cload(BT, [1, ch], dt_g, "bt", eng=nc.scalar)
        ident_t = cload(IDENT, [128, 128], dt_g, "ident", eng=nc.scalar)
        g2idx_t = cload(G2IDX, [128, P.C2 * 8], mybir.dt.int16, "g2idx",
                        eng=nc.scalar)
        p2tab_t = cload(P2TAB, [128, 4, P.C2], f32, "p2tab", eng=nc.scalar)
        vloc2_t, w2raw_t = p2tab_t[:, 0, :], p2tab_t[:, 1, :]
        vloc2b_t, w2rawb_t = p2tab_t[:, 2, :], p2tab_t[:, 3, :]
        ones1_t = const.tile([1, 128], dt_g, tag="ones1")
        nc.vector.memset(ones1_t[:], 1.0)

        # ---------------- gather + one/two-hot reduce ----------------------
        def agg_phase(src_ap, gidx_t, loc_t, w_t, locb_t, wb_t, is2, W_list,
                      gtag, chunk_cb, win_cb, dt_tab, src_sel=None,
                      pre_tiles=None, pool=None, cols=None):
            """Consumption iterates (window, cw) order; `cols` gives each
            table column's (window, cw) — prefetched pre-block columns may
            sit outside their window's run."""
            pre_tiles = pre_tiles or {}
            pool = pool or gpool
            if cols is None:
                cols = [(w, cw) for w, Wx in enumerate(W_list)
                        for cw in range(Wx)]
            C = len(cols)
            pos_order = sorted(range(C), key=lambda t: cols[t])
            # calls = runs of consecutive non-prefetched table columns
            calls = []
            run = []
            for t in range(C):
                if t in pre_tiles or len(run) == grp:
                    if run:
                        calls.append((run[0], len(run)))
                    run = []
                if t not in pre_tiles:
                    run.append(t)
            if run:
                calls.append((run[0], len(run)))
            call_of = {}
            for g0, n in calls:
                for j in range(n):
                    call_of[g0 + j] = (g0, n)
            tiles = {}
            pw = None
            for t in pos_order:
                if t in call_of and call_of[t][0] == t:
                    g0, n = call_of[t]
                    gt_new = pool.tile([128, grp, ch], dt_tab, tag=gtag)
                    src = src_sel(g0, n) if src_sel is not None else src_ap
                    nc.gpsimd.dma_gather(
                        gt_new[:, 0:n, :], src,
                        gidx_t[:, g0 * 8:(g0 + n) * 8],
                        n * 128, n * 128, ch,
                        queue_num=(g0 // grp) % nqueues)
                    tiles[g0] = gt_new
                w, cw = cols[t]
                if t in pre_tiles:
                    gt, j = pre_tiles[t]
                else:
                    g0, n = call_of[t]
                    gt, j = tiles[g0], t - g0
                sel = selp.tile([128, 128], dt_g, tag="sel")
                nc.vector.tensor_scalar(
                    sel[:], iota_t[:], loc_t[:, t:t + 1], w_t[:, t:t + 1],
                    op0=eq, op1=mul)
                if is2[t]:  # dedup chunk: add the second hot
                    selb = selp.tile([128, 128], dt_g, tag="selb")
                    nc.vector.tensor_scalar(
                        selb[:], iota_t[:], locb_t[:, t:t + 1],
                        wb_t[:, t:t + 1], op0=eq, op1=mul)
                    sel2 = selp.tile([128, 128], dt_g, tag="sel2")
                    nc.vector.tensor_tensor(sel2[:], sel[:], selb[:], op=add)
                    sel = sel2
                if cw == 0:
                    pw = psum.tile([128, ch], f32, tag="win")
                last = cw == W_list[w] - 1
                chunk_cb(pw, sel, gt, j, w, cw, last)
                if last:
                    win_cb(pw, w)

        def p1_chunk(pw, sel, gt, j, w, cw, last):
            nc.tensor.matmul(pw[:], sel[:], gt[:, j, :],
                             start=(cw == 0), stop=last)

        def gemm_bias(src_t, dst_psum):
            """dst[v/e, co] = src^T blocks @ W.T + 1^T b (K=1 bias matmul)."""
            for k in range(KT):
                nc.tensor.matmul(dst_psum[:], src_t[:, k, :], wt_t[:, k, :],
                                 start=(k == 0), stop=False)
            nc.tensor.matmul(dst_psum[:], ones1_t[:], bt_t[:],
                             start=False, stop=True)

        def transpose_blocks(pw, tag):
            """psum [128, ch] f32 -> sbuf [128, KT, 128] dt_g transposed.
            Copies are per-128-block so transpose k pipelines with copy k+1."""
            tT_w = sbp.tile([128, KT, 128], dt_g, tag=tag + "T", name=tag + "T")
            for k in range(KT):
                twk = sbp.tile([128, 128], dt_g, tag=tag + "f", name=tag + "f")
                nc.vector.tensor_copy(twk[:], pw[:, k * 128:(k + 1) * 128])
                pt = psum.tile([128, 128], dt_g, tag="aux", name="pt")
                nc.tensor.transpose(pt[:], twk[:], ident_t[:])
                nc.vector.tensor_copy(tT_w[:, k, :], pt[:])
            return tT_w

        def p1_win(pw, w):
            # window w's edge rows are complete: ship its CCIN slice and
            # immediately AllGather that window's table piece, overlapping
            # the collective with the remaining p1 windows.
            xep = sbp.tile([128, ch], dt_p2, tag="xep", name="xep")
            if post:
                # raw table, scaled for fp8 range; GEMM happens after p2 agg
                nc.vector.tensor_scalar(xep[:], pw[:], float(P2_SCALE), None,
                                        op0=mul)
            else:
                xeT_w = transpose_blocks(pw, "xe")
                pg = psum.tile([128, ch], f32, tag="gemm", name="pg")
                gemm_bias(xeT_w, pg)
                nc.vector.tensor_copy(xep[:], pg[:])
            nc.sync.dma_start(CCIN[w * 128:(w + 1) * 128, :], xep[:])
            if AG_SLICED:
                o = w * P.ncores * 128
                if spmd:
                    nc.gpsimd.collective_compute(
                        "AllGather", mybir.AluOpType.bypass,
                        replica_groups=[list(range(P.ncores))],
                        ins=[CCIN[w * 128:(w + 1) * 128, :]],
                        outs=[CCOUT[o:o + P.ncores * 128, :]])
                else:  # single-core stand-in for the window AllGather
                    nc.sync.dma_start(CCOUT[o:o + 128, :],
                                      CCIN[w * 128:(w + 1) * 128, :])
            elif w == P.NW1 - 1:
                if spmd:
                    nc.gpsimd.collective_compute(
                        "AllGather", mybir.AluOpType.bypass,
                        replica_groups=[list(range(P.ncores))],
                        ins=[CCIN[:]], outs=[CCOUT[:]])
                else:
                    nc.sync.dma_start(CCOUT[0:ner1, :], CCIN[:])
            # prefetch the p2 pre-block (piece-0-only chunk-0 columns) during
            # p1, one window after its AllGather piece was triggered — soaks
            # p1's spare DMA bandwidth and thins p2's DMA-bound span
            if w >= 1:
                g0s = [(w - 1) * grp]
                if w == P.NW1 - 1:   # last boundary takes the leftovers
                    g0s += list(range(w * grp, P.npre2, grp))
                for g0 in g0s:
                    if g0 >= P.npre2:
                        continue
                    n = min(grp, P.npre2 - g0)
                    gt = prep.tile([128, grp, ch], dt_p2, tag=f"pre{g0}")
                    nc.gpsimd.dma_gather(
                        gt[:, 0:n, :], p2_src(g0, n),
                        g2idx_t[:, g0 * 8:(g0 + n) * 8],
                        n * 128, n * 128, ch, queue_num=0)
                    for j in range(n):
                        pre_tiles[g0 + j] = (gt, j)

        def p2_chunk(pw, sel, gt, j, w, cw, last):
            nc.tensor.matmul(pw[:], sel[:], gt[:, j, :],
                             start=(cw == 0), stop=last)

        def p2_win(pw, w):
            if post:
                awT = transpose_blocks(pw, "aw")
                po = psum.tile([128, ch], f32, tag="gemm", name="po")
                gemm_bias(awT, po)
                pw = po
            # weights pre-normalized on host: just relu + store
            ow = sbp.tile([128, ch], dt_out, tag="ow", name="ow")
            nc.vector.tensor_scalar(ow[:], pw[:], 1.0, 0.0, op0=mul, op1=mx)
            nc.sync.dma_start(OUT[w * 128:(w + 1) * 128, :], ow[:])

        # chunks that contain any dedup pair need the second sel pass; the
        # union over cores keeps the SPMD program identical on every core.
        is2_1 = np.zeros(P.C1, bool)
        is2_2 = np.zeros(P.C2, bool)
        maxrow2 = np.zeros(P.C2, np.int64)
        for k in range(P.ncores):
            is2_1 |= (_pack(P.p1[k][3], P.C1) >= 0).any(axis=0)
            is2_2 |= (_pack(P.p2[k][3], P.C2) >= 0).any(axis=0)
            maxrow2 = np.maximum(
                maxrow2, _pack(P.p2[k][0], P.C2).astype(np.int64).max(axis=0))

        piece = P.ncores * 128
        def p2_src(g0, n):
            # prefix slice of the edge table covering every row this gather
            # call touches, so it only waits on the AllGather pieces it needs
            pieces = int(maxrow2[g0:g0 + n].max()) // piece + 1
            if not AG_SLICED or pieces >= P.NW1:
                return CCOUT[:]
            return CCOUT[0:pieces * piece, :]

        # p2 prefetch schedule: at p1 window boundary w we may issue gathers
        # for p2 chunks that only need table pieces < w (their AllGather was
        # triggered a full window earlier).  Earliest-consumed chunks first.
        bound = (maxrow2 // piece + 1).astype(int)   # pieces needed per chunk
        pre_sched = {w: [] for w in range(1, P.NW1)}
        if AG_SLICED and PRE_PER_B > 0:
            taken = set()
            for w in range(1, P.NW1):
                for c in range(P.C2):
                    if len(pre_sched[w]) >= PRE_PER_B:
                        break
                    if c not in taken and bound[c] <= w:
                        pre_sched[w].append(c)
                        taken.add(c)

        for _rep in range(reps):
            pre_tiles.clear()
            agg_phase(XT[:], g1idx_t, eloc1_t, wsel1_t, eloc1b_t, wsel1b_t,
                      is2_1, P.W1_list, "g1", p1_chunk, p1_win, dt_p1)

            # phase 2: e2v aggregation (sel weights pre-normalized on host)
            agg_phase(CCOUT[:], g2idx_t, vloc2_t, w2raw_t, vloc2b_t, w2rawb_t,
                      is2_2, P.W2_list, "g2", p2_chunk, p2_win, dt_p2,
                      src_sel=p2_src, pre_tiles=pre_tiles, pool=g2pool,
                      cols=P.p2cols)

    nc.compile()
    return nc


# ------------------------------------------------------------------ runner ---
def make_in_maps(P, X, W, b, bf16=GATHER_BF16, p1_dt=P1_DT, fuse=FUSE):
    npdt = ml_dtypes.bfloat16 if bf16 else np.float32
    np_p1 = _npdt(p1_dt)
    s1 = P1_SCALE if p1_dt == "f8" else 1.0
    s2 = P2_SCALE if fuse == "post" else 1.0
    KT = P.ch // 128
    xt = np.ascontiguousarray((X * s1).astype(np_p1))
    wt = np.ascontiguousarray(
        W.T.reshape(KT, 128, P.ch).transpose(1, 0, 2).astype(npdt))
    bt = np.ascontiguousarray(b.astype(npdt).reshape(1, P.ch))
    iota = np.ascontiguousarray(
        np.broadcast_to(np.arange(128, dtype=npdt), (128, 128)))
    ident = np.eye(128, dtype=npdt)

    def tb(flat, C, s=1.0):
        return _pack(flat, C) / np.float32(s)

    in_maps = []
    for k in range(P.ncores):
        g1, l1, w1, l1b, w1b = P.p1[k]
        g2, l2, w2, l2b, w2b = P.p2[k]
        p1tab = np.ascontiguousarray(np.stack(
            [tb(l1, P.C1), tb(w1, P.C1, s1), tb(l1b, P.C1), tb(w1b, P.C1, s1)],
            axis=1))
        p2tab = np.ascontiguousarray(np.stack(
            [tb(l2, P.C2), tb(w2, P.C2, s2), tb(l2b, P.C2), tb(w2b, P.C2, s2)],
            axis=1))
        in_maps.append({
            "xt": xt, "wt": wt, "bt": bt, "iota": iota, "ident": ident,
            "g1idx": _wrap_idx(g1), "p1tab": p1tab,
            "g2idx": _wrap_idx(g2), "p2tab": p2tab,
        })
    return in_maps


def assemble(P, shards):
    out = np.zeros((P.nv, P.ch), np.float32)
    for k in range(P.ncores):
        vm = P.vmap[k]
        m = vm >= 0
        out[vm[m]] = shards[k][m].astype(np.float32)
    return out


_nc_cache = {}


def kernel(X, W, b, e2v_weight, v_idx, e_idx):
    global _last_results
    from concourse.bass_utils import run_bass_kernel_spmd

    P = make_plan(v_idx, e_idx, e2v_weight)
    key = (P.C1, P.C2, P.W1, P.W2, GATHER_BF16, P1_DT, P2_DT, OUT_DT, FUSE,
           AG_SLICED)
    if key not in _nc_cache:
        _nc_cache[key] = build_nc(P)
    nc = _nc_cache[key]
    in_maps = make_in_maps(P, X, W, b)
    res = run_bass_kernel_spmd(nc, in_maps, list(range(P.ncores)), trace=TRACE)
    _last_results = res
    shards = [res.results[k]["out"] for k in range(P.ncores)]
    return assemble(P, shards)



# revision 113
# speedup vs baseline: 1.0660x; 1.0061x over previous
"""HGNNPConv Trainium2 kernel (8 NeuronCores, SPMD).

Math (equivalent reformulation of the reference):
  Xe_raw[e] = mean_{i: e_idx[i]=e} X[v_idx[i]]              (v2e, softmax of ones = 1/deg)
  Xe_p      = Xe_raw @ W.T + b                              (GEMM on 4000 edges, not 20000 verts)
  Xv[v]     = sum_i wn_i * Xe_p[e_idx[i]],  wn_i = exp(w_i)/sum_{v} exp(w)
              (wn precomputed on host -> no on-chip denominator pass)
  out       = relu(Xv)
Deg-0 vertices are masked host-side in assemble(); empty edges get a spurious
+b in Xe_p but are never referenced downstream.

Sharding: edges and vertices are binpacked across the 8 cores (balancing
post-dedup gather slots).  Phase 1 aggregates by destination edge, runs the
edge-level GEMM per 128-edge window (bias folded in as a K=1 matmul), and
AllGathers each window's table slice as soon as it is ready (overlapping the
collective with the remaining phase-1 work).  Phase 2 aggregates by
destination vertex; each gather call's source AP is a prefix slice of the
table covering only the AllGather pieces it needs.

Per-destination-window weighted one-hot selection matrices (built on DVE
from a bf16 iota) reduce gathered rows on the PE into PSUM.  Incidences
sharing a source row within a window are deduplicated into one gathered slot
with a two-hot sel column (~11% fewer gather bytes; window binpacking is
rebalanced for post-dedup slot counts, phase 2 uses 21 windows x 7 chunks
instead of 20 x 8, and per-window chunk counts are variable — windows are
sorted largest-first so cross-core padding aligns and each phase ends on its
smallest window).

Dtypes: phase-1 gather table fp8-e3m4 (X pre-scaled by 2: worst-case quant
error halves vs e4m3 and the GEMM averages it out), phase-2 table bf16
(absmax-norm tolerance rules out fp8 post-GEMM), output bf16 (upcast on
host).  End-to-end rel err 1.35e-2 vs the 2e-2 gate.
"""

import os
from contextlib import ExitStack

import numpy as np
import ml_dtypes

# ---------------------------------------------------------------- config ---
NCORES = 8
NV, NE, NNZ, CH = 20000, 4000, 160000, 512
GATHER_BF16 = os.environ.get("KERNEL_F32", "") == ""  # bf16 tables+matmuls by default
P1_DT = os.environ.get("KERNEL_P1_DT", "f8")   # phase-1 gather table dtype
P2_DT = os.environ.get("KERNEL_P2_DT", "bf16")  # phase-2 gather table dtype
OUT_DT = os.environ.get("KERNEL_OUT_DT", "bf16")
FUSE = os.environ.get("KERNEL_FUSE", "pre")    # "post": GEMM after p2 agg
P1_SCALE = 2.0   # X pre-scale for fp8-e3m4 range use (exact power of 2)
P2_SCALE = 8.0   # Xe_raw pre-scale for the fp8 edge table (post mode)
GRP = 5          # gather chunks (of 128 idxs) per dma_gather call
AG_SLICED = os.environ.get("KERNEL_AG_SLICED", "1") != ""  # per-window AllGather
PRE_PER_B = int(os.environ.get("KERNEL_PRE_PER_B", "0"))  # p2 prefetches per p1 window
CW_MAJOR = os.environ.get("KERNEL_CW_MAJOR", "") != ""  # cw-major p2 table layout
TRACE = os.environ.get("BASS_TRACE", "") != ""


def _mydt(mybir, name):
    return {"f8": mybir.dt.float8e3, "bf16": mybir.dt.bfloat16,
            "f32": mybir.dt.float32}[name]


def _npdt(name):
    return {"f8": ml_dtypes.float8_e3m4, "bf16": ml_dtypes.bfloat16,
            "f32": np.float32}[name]

_last_results = None   # BassKernelResults of the most recent run (for test.py)


# ------------------------------------------------------------------- plan ---
class Plan:
    pass


def _binpack(ids, degs, nbins, cap=128):
    """Pack `ids` into `nbins` bins of <=cap items, balancing sum(degs)."""
    import heapq

    order = np.argsort(-degs, kind="stable")
    bins = [[] for _ in range(nbins)]
    loads = [0] * nbins
    heap = [(0, b) for b in range(nbins)]
    heapq.heapify(heap)
    for t in order:
        popped = []
        while True:
            load, b = heapq.heappop(heap)
            if len(bins[b]) < cap:
                break
            popped.append((load, b))
        for p in popped:
            heapq.heappush(heap, p)
        bins[b].append(int(ids[t]))
        loads[b] = load + int(degs[t])
        heapq.heappush(heap, (loads[b], b))
    return bins, loads


def _csr(idx, n):
    order = np.argsort(idx, kind="stable").astype(np.int64)
    deg = np.bincount(idx, minlength=n).astype(np.int64)
    starts = np.zeros(n + 1, np.int64)
    np.cumsum(deg, out=starts[1:])
    return order, deg, starts


def _pair_window(src, locs, ws):
    """Greedy within-window dedup: incidences sharing a source row become one
    gathered slot with two (loc, w) hots.  Returns (src', loc_a, w_a, loc_b,
    w_b) with pair slots FIRST; singles have loc_b = -1 / w_b = 0."""
    order = np.argsort(src, kind="stable")
    src, locs, ws = src[order], locs[order], ws[order]
    pa, pb, sg = [], [], []
    i, n = 0, len(src)
    while i < n:
        j = i
        while j < n and src[j] == src[i]:
            j += 1
        k = i
        while k + 1 < j:
            pa.append(k); pb.append(k + 1); k += 2
        if k < j:
            sg.append(k)
        i = j
    pa, pb, sg = np.array(pa, np.int64), np.array(pb, np.int64), np.array(sg, np.int64)
    src2 = np.concatenate([src[pa], src[sg]]) if len(pa) else src[sg]
    la = np.concatenate([locs[pa], locs[sg]]) if len(pa) else locs[sg]
    wa = np.concatenate([ws[pa], ws[sg]]) if len(pa) else ws[sg]
    lb = np.concatenate([locs[pb], np.full(len(sg), -1.0, locs.dtype)]) if len(pa) \
        else np.full(len(sg), -1.0, locs.dtype)
    wb = np.concatenate([ws[pb], np.zeros(len(sg), ws.dtype)]) if len(pa) \
        else np.zeros(len(sg), ws.dtype)
    return src2, la, wa, lb, wb, len(pa)


def _phase_windows(bins_per_core, order, starts, idx_of_inc, w_of_inc,
                   loc_dtype=np.float32, pair=True, piece_order=False):
    """Per-window slot lists for one core of one phase, after source dedup.

    Returns (wins, wmax): wins[w] = (src, loc_a, w_a, loc_b, w_b).  By
    default pair slots (loc_b >= 0) come first within each window; with
    piece_order=True, singles come first sorted by source id (so early
    chunks only reference low table pieces) and pair slots go last.
    """
    wins = []
    wmax = 0
    for bin_ids in bins_per_core:
        incs = []
        locs = []
        for j, d in enumerate(bin_ids):
            seg = order[starts[d]:starts[d + 1]]
            incs.append(seg)
            locs.append(np.full(len(seg), j, loc_dtype))
        incs = np.concatenate(incs) if incs else np.zeros(0, np.int64)
        locs = np.concatenate(locs) if locs else np.zeros(0, loc_dtype)
        src = idx_of_inc[incs]
        ws = w_of_inc[incs].astype(np.float32)
        if pair and len(src):
            src, la, wa, lb, wb, npair = _pair_window(src, locs, ws)
            if piece_order:
                ps = np.argsort(src[:npair], kind="stable")
                ss = npair + np.argsort(src[npair:], kind="stable")
                perm = np.concatenate([ss, ps])
                src, la, wa, lb, wb = (src[perm], la[perm], wa[perm],
                                       lb[perm], wb[perm])
        else:
            la, wa = locs, ws
            lb = np.full(len(src), -1.0, loc_dtype)
            wb = np.zeros(len(src), np.float32)
        wins.append((src, la, wa, lb, wb))
        wmax = max(wmax, len(src))
    return wins, wmax


def _layout(wins, W_list):
    """Flat slot arrays with per-window chunk counts.  Window w's cw-th
    chunk sits at table column offs[w] + cw."""
    offs = np.concatenate([[0], np.cumsum(W_list)])
    cols = []
    for w, Wx in enumerate(W_list):
        cols.extend((w, cw) for cw in range(Wx))
    return _layout_cols(wins, cols)


def _layout_cols(wins, cols):
    """Flat slot arrays for an explicit column list of (window, cw)."""
    L = len(cols) * 128
    gidx = np.zeros(L, np.int16)
    loc = np.full(L, -1.0, np.float32)
    wsel = np.zeros(L, np.float32)
    locb = np.full(L, -1.0, np.float32)
    wselb = np.zeros(L, np.float32)
    for t, (w, cw) in enumerate(cols):
        src, la, wa, lb, wb = wins[w]
        i0, i1 = cw * 128, min((cw + 1) * 128, len(src))
        if i0 >= len(src):
            continue
        o, n = t * 128, i1 - i0
        gidx[o:o + n] = src[i0:i1]
        loc[o:o + n] = la[i0:i1]
        wsel[o:o + n] = wa[i0:i1]
        locb[o:o + n] = lb[i0:i1]
        wselb[o:o + n] = wb[i0:i1]
    return gidx, loc, wsel, locb, wselb


def _wrap_idx(flat):
    """int16 flat[i] -> [128, len/16] with value i at [i%16, i//16], replicated."""
    a = flat.reshape(-1, 16).T  # [16, L/16]
    return np.ascontiguousarray(np.tile(a, (8, 1)))


def _pack(flat, C):
    """flat[c*128+p] -> [128, C]"""
    return np.ascontiguousarray(flat.reshape(C, 128).T)


def _dedup_slots(srcs):
    """#gather slots for a window's source list after pairing."""
    if not len(srcs):
        return 0
    _, cnt = np.unique(srcs, return_counts=True)
    return int(((cnt + 1) // 2).sum())


def _repair_bins(bins, order, starts, idx_of_inc, cap=128, iters=400):
    """Greedy rebalance: move members out of the window with the most
    post-dedup slots into the one with the fewest (respecting the member
    cap), to minimize max slots per window."""
    srcs = [
        [idx_of_inc[order[starts[d]:starts[d + 1]]] for d in b] for b in bins
    ]

    def slots(w):
        return _dedup_slots(np.concatenate(srcs[w]) if srcs[w] else
                            np.zeros(0, np.int64))

    cur = [slots(w) for w in range(len(bins))]
    for _ in range(iters):
        hot = int(np.argmax(cur))
        order_cold = np.argsort(cur)
        moved = False
        for cold in order_cold:
            if cold == hot or len(bins[cold]) >= cap:
                continue
            # move the member with the smallest segment out of `hot`
            j = int(np.argmin([len(s) for s in srcs[hot]]))
            bins[cold].append(bins[hot].pop(j))
            srcs[cold].append(srcs[hot].pop(j))
            new_hot, new_cold = slots(hot), slots(cold)
            if max(new_hot, new_cold) >= cur[hot]:
                # revert: no improvement
                bins[hot].append(bins[cold].pop())
                srcs[hot].append(srcs[cold].pop())
                continue
            cur[hot], cur[cold] = new_hot, new_cold
            moved = True
            break
        if not moved:
            break
    return bins


def make_plan(v_idx, e_idx, e2v_weight, nv=NV, ne=NE, ch=CH, ncores=NCORES):
    P = Plan()
    P.nv, P.ne, P.ch, P.ncores = nv, ne, ch, ncores
    epc, vpc = ne // ncores, nv // ncores
    P.epc, P.vpc = epc, vpc

    order_e, deg_e, starts_e = _csr(e_idx, ne)
    order_v, deg_v, starts_v = _csr(v_idx, nv)
    inv_deg = np.zeros(ne, np.float32)
    nz = deg_e > 0
    inv_deg[nz] = (np.float32(1.0) / deg_e[nz].astype(np.float32))

    nb1 = -(-epc // 128)
    nb2 = -(-vpc // 128)
    v_of_inc = v_idx.astype(np.int64)
    e_of_inc = e_idx.astype(np.int64)
    # balance destinations across cores globally (assignment is free — pos /
    # vmap carry it), then binpack windows within each core and rebalance for
    # post-dedup slot counts.
    cores_e, _ = _binpack(np.arange(ne), deg_e, ncores, cap=nb1 * 128)
    bins1 = []
    for k in range(ncores):
        eids = np.asarray(cores_e[k])
        b, _ = _binpack(eids, deg_e[eids], nb1)
        bins1.append(_repair_bins(b, order_e, starts_e, v_of_inc))

    # p2 window count: an extra window can admit a smaller W2 (less gather
    # padding) once dedup shrinks the per-window loads — pick the best.
    best = None
    for nb2c in (nb2, nb2 + 1):
        cores_v, _ = _binpack(np.arange(nv), deg_v, ncores, cap=nb2c * 128)
        cand = []
        sizes = []
        for k in range(ncores):
            vids = np.asarray(cores_v[k])
            b, _ = _binpack(vids, deg_v[vids], nb2c)
            b = _repair_bins(b, order_v, starts_v, e_of_inc)
            cand.append(b)
            sizes.append(sorted(
                (-(-_dedup_slots(np.concatenate(
                    [e_of_inc[order_v[starts_v[d]:starts_v[d + 1]]]
                     for d in bb]) if bb else np.zeros(0, np.int64)) // 128)
                 for bb in b), reverse=True))
        C2c = sum(max(sizes[k][w] for k in range(ncores))
                  for w in range(nb2c))
        if best is None or C2c < best[1]:
            best = (nb2c, C2c, cand)
    nb2, _, bins2 = best
    P.NW1, P.NW2 = nb1, nb2

    # phase-1 windows (dedup within window) + edge position map.  Positions
    # are window-major (w, core, row) so each window's table slice can be
    # AllGathered independently as soon as its GEMM finishes.  Windows are
    # sorted largest-first per core: cross-core maxes align (less padding)
    # and the smallest window lands last (shorter phase tail).
    pos = np.zeros(ne, np.int64)
    wins1 = []
    w1max = 0
    for k in range(ncores):
        wins, wmax = _phase_windows(
            bins1[k], order_e, starts_e, v_idx.astype(np.int64),
            inv_deg[e_idx.astype(np.int64)])
        order = np.argsort([-len(w[0]) for w in wins], kind="stable")
        wins = [wins[i] for i in order]
        bins1[k] = [bins1[k][i] for i in order]
        wins1.append(wins)
        w1max = max(w1max, wmax)
        for w, bin_ids in enumerate(bins1[k]):
            for j, e in enumerate(bin_ids):
                if AG_SLICED:   # window-major: (w, core, row)
                    pos[e] = (w * ncores + k) * 128 + j
                else:           # core-major: AllGather rank concatenation
                    pos[e] = (k * nb1 + w) * 128 + j
    assert pos.max() < 32768
    P.W1_list = [
        max(-(-len(wins1[k][w][0]) // 128) for k in range(ncores))
        for w in range(nb1)
    ]
    P.W1 = max(P.W1_list)
    P.C1 = int(sum(P.W1_list))
    P.p1 = [_layout(wins, P.W1_list) for wins in wins1]

    # phase-2 windows + output row map. Softmax weights are fully normalized
    # on the host (exp / per-vertex sum), so the kernel needs no denominator
    # pass.
    expw = np.exp(e2v_weight.astype(np.float64))
    den = np.zeros(nv, np.float64)
    np.add.at(den, v_idx, expw)
    wnorm = (expw / den[v_idx.astype(np.int64)]).astype(np.float32)
    wins2 = []
    P.vmap = []
    for k in range(ncores):
        wins, _ = _phase_windows(
            bins2[k], order_v, starts_v, pos[e_idx.astype(np.int64)], wnorm,
            piece_order=AG_SLICED)
        order = np.argsort([-len(w[0]) for w in wins], kind="stable")
        wins = [wins[i] for i in order]
        bins2[k] = [bins2[k][i] for i in order]
        wins2.append(wins)
        vm = np.full(P.NW2 * 128, -1, np.int64)
        for w, bin_ids in enumerate(bins2[k]):
            vm[w * 128:w * 128 + len(bin_ids)] = bin_ids
        # deg-0 vertices never receive contributions; drop them from the
        # output map so any on-chip garbage (e.g. a stray +b) is discarded.
        vme = vm[vm >= 0]
        vm[vm >= 0] = np.where(deg_v[vme] > 0, vme, -1)
        P.vmap.append(vm)
    P.W2_list = [
        max(-(-len(wins2[k][w][0]) // 128) for k in range(ncores))
        for w in range(nb2)
    ]
    P.W2 = max(P.W2_list)
    P.C2 = int(sum(P.W2_list))
    # pre-block: chunk-0 of the first windows, but only if they reference
    # table piece 0 alone (so a gather over them can run during phase 1,
    # right after the first window's AllGather).  They form the leading
    # contiguous columns of the table layout.
    piece = ncores * 128
    npre = 0 if not AG_SLICED else min(20, nb2)
    npre_cap, npre = npre, 0
    for w in range(npre_cap):
        b0max = max(
            int(wins2[k][w][0][:128].max()) if len(wins2[k][w][0]) else 0
            for k in range(ncores))
        if b0max >= piece or P.W2_list[w] < 2:
            break
        npre += 1
    npre -= npre % 5            # whole grp-5 calls only
    P.npre2 = npre
    cols = [(w, 0) for w in range(npre)]
    for w in range(nb2):
        for cw in (range(1, P.W2_list[w]) if w < npre
                   else range(P.W2_list[w])):
            cols.append((w, cw))
    P.p2cols = cols
    P.p2 = [_layout_cols(wins, cols) for wins in wins2]
    return P


# ---------------------------------------------------------------- builder ---
def build_nc(P, bf16=GATHER_BF16, spmd=True, reps=1, grp=GRP, gbufs=5,
             nqueues=1, p1_dt=P1_DT, p2_dt=P2_DT, out_dt=OUT_DT, fuse=FUSE):
    import concourse.bacc as bacc
    import concourse.mybir as mybir
    import concourse.tile as tile

    f32 = mybir.dt.float32
    dt_g = mybir.dt.bfloat16 if bf16 else f32   # sel matrices + GEMM operands
    dt_p1 = _mydt(mybir, p1_dt)
    dt_p2 = _mydt(mybir, p2_dt)
    dt_out = _mydt(mybir, out_dt)
    eq, mul, mx, add = (mybir.AluOpType.is_equal, mybir.AluOpType.mult,
                        mybir.AluOpType.max, mybir.AluOpType.add)
    ch, KT = P.ch, P.ch // 128
    post = fuse == "post"

    nc = bacc.Bacc("TRN2", target_bir_lowering=False, debug=False,
                   num_devices=P.ncores if spmd else 1,
                   num_swdge_queues=nqueues)

    XT = nc.dram_tensor("xt", [P.nv, ch], dt_p1, kind="ExternalInput")
    WT = nc.dram_tensor("wt", [128, KT, ch], dt_g, kind="ExternalInput")
    BT = nc.dram_tensor("bt", [1, ch], dt_g, kind="ExternalInput")
    IOTA = nc.dram_tensor("iota", [128, 128], dt_g, kind="ExternalInput")
    IDENT = nc.dram_tensor("ident", [128, 128], dt_g, kind="ExternalInput")
    G1IDX = nc.dram_tensor("g1idx", [128, P.C1 * 8], mybir.dt.int16, kind="ExternalInput")
    P1TAB = nc.dram_tensor("p1tab", [128, 4, P.C1], f32, kind="ExternalInput")
    G2IDX = nc.dram_tensor("g2idx", [128, P.C2 * 8], mybir.dt.int16, kind="ExternalInput")
    P2TAB = nc.dram_tensor("p2tab", [128, 4, P.C2], f32, kind="ExternalInput")

    ner1 = P.NW1 * 128
    CCIN = nc.dram_tensor("ccin", [ner1, ch], dt_p2)
    CCOUT = nc.dram_tensor("ccout", [P.ncores * ner1, ch], dt_p2, addr_space="Shared")
    OUT = nc.dram_tensor("out", [P.NW2 * 128, ch], dt_out, kind="ExternalOutput")

    with tile.TileContext(nc) as tc, ExitStack() as ctx:
        const = ctx.enter_context(tc.tile_pool(name="const", bufs=1))
        gpool = ctx.enter_context(tc.tile_pool(name="g", bufs=gbufs))
        g2pool = ctx.enter_context(tc.tile_pool(name="g2", bufs=gbufs))
        prep = ctx.enter_context(tc.tile_pool(name="prep", bufs=1))
        selp = ctx.enter_context(tc.tile_pool(name="selp", bufs=8))
        psum = ctx.enter_context(tc.tile_pool(name="ps", bufs=2, space="PSUM"))
        sbp = ctx.enter_context(tc.tile_pool(name="sbp", bufs=2))
        pre_tiles = {}

        def cload(dram, shape, dt, tag, eng=None):
            t = const.tile(shape, dt, tag=tag)
            (eng or nc.sync).dma_start(t[:], dram[:])
            return t

        # p1-critical tables first (SP ring); the rest go on the ACT ring so
        # they never delay the first gather.  g1idx leads: the first gather
        # needs only it, iota/p1tab are for the (later) first sel build.
        g1idx_t = cload(G1IDX, [128, P.C1 * 8], mybir.dt.int16, "g1idx")
        iota_t = cload(IOTA, [128, 128], dt_g, "iota")
        p1tab_t = cload(P1TAB, [128, 4, P.C1], f32, "p1tab")
        eloc1_t, wsel1_t = p1tab_t[:, 0, :], p1tab_t[:, 1, :]
        eloc1b_t, wsel1b_t = p1tab_t[:, 2, :], p1tab_t[:, 3, :]
        wt_t = cload(WT, [128, KT, ch], dt_g, "wt", eng=nc.scalar)
        bt_t = cload(BT, [1, ch], dt_g, "bt", eng=nc.scalar)
        ident_t = cload(IDENT, [128, 128], dt_g, "ident", eng=nc.scalar)
        g2idx_t = cload(G2IDX, [128, P.C2 * 8], mybir.dt.int16, "g2idx",
                        eng=nc.scalar)
        p2tab_t = cload(P2TAB, [128, 4, P.C2], f32, "p2tab", eng=nc.scalar)
        vloc2_t, w2raw_t = p2tab_t[:, 0, :], p2tab_t[:, 1, :]
        vloc2b_t, w2rawb_t = p2tab_t[:, 2, :], p2tab_t[:, 3, :]
        ones1_t = const.tile([1, 128], dt_g, tag="ones1")
        nc.vector.memset(ones1_t[:], 1.0)

        # ---------------- gather + one/two-hot reduce ----------------------
        def agg_phase(src_ap, gidx_t, loc_t, w_t, locb_t, wb_t, is2, W_list,
                      gtag, chunk_cb, win_cb, dt_tab, src_sel=None,
                      pre_tiles=None, pool=None, cols=None, split_bounds=None):
            """Consumption iterates (window, cw) order; `cols` gives each
            table column's (window, cw) — prefetched pre-block columns may
            sit outside their window's run."""
            pre_tiles = pre_tiles or {}
            pool = pool or gpool
            if cols is None:
                cols = [(w, cw) for w, Wx in enumerate(W_list)
                        for cw in range(Wx)]
            C = len(cols)
            pos_order = sorted(range(C), key=lambda t: cols[t])
            # calls = runs of consecutive non-prefetched table columns; in
            # the early columns a run also breaks where the source-prefix
            # bound rises, so the low-piece part never waits on later
            # AllGather pieces
            calls = []
            run = []
            for t in range(C):
                brk = t in pre_tiles or len(run) == grp
                if run and split_bounds is not None and t in split_bounds:
                    brk = True
                if brk:
                    if run:
                        calls.append((run[0], len(run)))
                    run = []
                if t not in pre_tiles:
                    run.append(t)
            if run:
                calls.append((run[0], len(run)))
            call_of = {}
            for g0, n in calls:
                for j in range(n):
                    call_of[g0 + j] = (g0, n)
            tiles = {}
            pw = None
            for t in pos_order:
                if t in call_of and call_of[t][0] == t:
                    g0, n = call_of[t]
                    gt_new = pool.tile([128, grp, ch], dt_tab, tag=gtag)
                    src = src_sel(g0, n) if src_sel is not None else src_ap
                    nc.gpsimd.dma_gather(
                        gt_new[:, 0:n, :], src,
                        gidx_t[:, g0 * 8:(g0 + n) * 8],
                        n * 128, n * 128, ch,
                        queue_num=(g0 // grp) % nqueues)
                    tiles[g0] = gt_new
                w, cw = cols[t]
                if t in pre_tiles:
                    gt, j = pre_tiles[t]
                else:
                    g0, n = call_of[t]
                    gt, j = tiles[g0], t - g0
                sel = selp.tile([128, 128], dt_g, tag="sel")
                nc.vector.tensor_scalar(
                    sel[:], iota_t[:], loc_t[:, t:t + 1], w_t[:, t:t + 1],
                    op0=eq, op1=mul)
                if is2[t]:  # dedup chunk: add the second hot
                    selb = selp.tile([128, 128], dt_g, tag="selb")
                    nc.vector.tensor_scalar(
                        selb[:], iota_t[:], locb_t[:, t:t + 1],
                        wb_t[:, t:t + 1], op0=eq, op1=mul)
                    sel2 = selp.tile([128, 128], dt_g, tag="sel2")
                    nc.vector.tensor_tensor(sel2[:], sel[:], selb[:], op=add)
                    sel = sel2
                if cw == 0:
                    pw = psum.tile([128, ch], f32, tag="win")
                last = cw == W_list[w] - 1
                chunk_cb(pw, sel, gt, j, w, cw, last)
                if last:
                    win_cb(pw, w)

        def p1_chunk(pw, sel, gt, j, w, cw, last):
            nc.tensor.matmul(pw[:], sel[:], gt[:, j, :],
                             start=(cw == 0), stop=last)

        def gemm_bias(src_t, dst_psum):
            """dst[v/e, co] = src^T blocks @ W.T + 1^T b (K=1 bias matmul)."""
            for k in range(KT):
                nc.tensor.matmul(dst_psum[:], src_t[:, k, :], wt_t[:, k, :],
                                 start=(k == 0), stop=False)
            nc.tensor.matmul(dst_psum[:], ones1_t[:], bt_t[:],
                             start=False, stop=True)

        def transpose_blocks(pw, tag):
            """psum [128, ch] f32 -> sbuf [128, KT, 128] dt_g transposed.
            Copies are per-128-block so transpose k pipelines with copy k+1."""
            tT_w = sbp.tile([128, KT, 128], dt_g, tag=tag + "T", name=tag + "T")
            for k in range(KT):
                twk = sbp.tile([128, 128], dt_g, tag=tag + "f", name=tag + "f")
                nc.vector.tensor_copy(twk[:], pw[:, k * 128:(k + 1) * 128])
                pt = psum.tile([128, 128], dt_g, tag="aux", name="pt")
                nc.tensor.transpose(pt[:], twk[:], ident_t[:])
                nc.vector.tensor_copy(tT_w[:, k, :], pt[:])
            return tT_w

        def p1_win(pw, w):
            # window w's edge rows are complete: ship its CCIN slice and
            # immediately AllGather that window's table piece, overlapping
            # the collective with the remaining p1 windows.
            xep = sbp.tile([128, ch], dt_p2, tag="xep", name="xep")
            if post:
                # raw table, scaled for fp8 range; GEMM happens after p2 agg
                nc.vector.tensor_scalar(xep[:], pw[:], float(P2_SCALE), None,
                                        op0=mul)
            else:
                xeT_w = transpose_blocks(pw, "xe")
                pg = psum.tile([128, ch], f32, tag="gemm", name="pg")
                gemm_bias(xeT_w, pg)
                nc.vector.tensor_copy(xep[:], pg[:])
            nc.sync.dma_start(CCIN[w * 128:(w + 1) * 128, :], xep[:])
            if AG_SLICED:
                o = w * P.ncores * 128
                if spmd:
                    nc.gpsimd.collective_compute(
                        "AllGather", mybir.AluOpType.bypass,
                        replica_groups=[list(range(P.ncores))],
                        ins=[CCIN[w * 128:(w + 1) * 128, :]],
                        outs=[CCOUT[o:o + P.ncores * 128, :]])
                else:  # single-core stand-in for the window AllGather
                    nc.sync.dma_start(CCOUT[o:o + 128, :],
                                      CCIN[w * 128:(w + 1) * 128, :])
            elif w == P.NW1 - 1:
                if spmd:
                    nc.gpsimd.collective_compute(
                        "AllGather", mybir.AluOpType.bypass,
                        replica_groups=[list(range(P.ncores))],
                        ins=[CCIN[:]], outs=[CCOUT[:]])
                else:
                    nc.sync.dma_start(CCOUT[0:ner1, :], CCIN[:])
            # prefetch the p2 pre-block (piece-0-only chunk-0 columns) during
            # p1, one window after its AllGather piece was triggered — soaks
            # p1's spare DMA bandwidth and thins p2's DMA-bound span
            if w >= 1:
                g0s = [(w - 1) * grp]
                if w == P.NW1 - 1:   # last boundary takes the leftovers
                    g0s += list(range(w * grp, P.npre2, grp))
                for g0 in g0s:
                    if g0 >= P.npre2:
                        continue
                    n = min(grp, P.npre2 - g0)
                    gt = prep.tile([128, grp, ch], dt_p2, tag=f"pre{g0}")
                    nc.gpsimd.dma_gather(
                        gt[:, 0:n, :], p2_src(g0, n),
                        g2idx_t[:, g0 * 8:(g0 + n) * 8],
                        n * 128, n * 128, ch, queue_num=0)
                    for j in range(n):
                        pre_tiles[g0 + j] = (gt, j)

        def p2_chunk(pw, sel, gt, j, w, cw, last):
            nc.tensor.matmul(pw[:], sel[:], gt[:, j, :],
                             start=(cw == 0), stop=last)

        def p2_win(pw, w):
            if post:
                awT = transpose_blocks(pw, "aw")
                po = psum.tile([128, ch], f32, tag="gemm", name="po")
                gemm_bias(awT, po)
                pw = po
            # weights pre-normalized on host: just relu + store
            ow = sbp.tile([128, ch], dt_out, tag="ow", name="ow")
            nc.vector.tensor_scalar(ow[:], pw[:], 1.0, 0.0, op0=mul, op1=mx)
            nc.sync.dma_start(OUT[w * 128:(w + 1) * 128, :], ow[:])

        # chunks that contain any dedup pair need the second sel pass; the
        # union over cores keeps the SPMD program identical on every core.
        is2_1 = np.zeros(P.C1, bool)
        is2_2 = np.zeros(P.C2, bool)
        maxrow2 = np.zeros(P.C2, np.int64)
        for k in range(P.ncores):
            is2_1 |= (_pack(P.p1[k][3], P.C1) >= 0).any(axis=0)
            is2_2 |= (_pack(P.p2[k][3], P.C2) >= 0).any(axis=0)
            maxrow2 = np.maximum(
                maxrow2, _pack(P.p2[k][0], P.C2).astype(np.int64).max(axis=0))

        piece = P.ncores * 128
        def p2_src(g0, n):
            # prefix slice of the edge table covering every row this gather
            # call touches, so it only waits on the AllGather pieces it needs
            pieces = int(maxrow2[g0:g0 + n].max()) // piece + 1
            if not AG_SLICED or pieces >= P.NW1:
                return CCOUT[:]
            return CCOUT[0:pieces * piece, :]

        # p2 prefetch schedule: at p1 window boundary w we may issue gathers
        # for p2 chunks that only need table pieces < w (their AllGather was
        # triggered a full window earlier).  Earliest-consumed chunks first.
        bound = (maxrow2 // piece + 1).astype(int)   # pieces needed per chunk
        pre_sched = {w: [] for w in range(1, P.NW1)}
        if AG_SLICED and PRE_PER_B > 0:
            taken = set()
            for w in range(1, P.NW1):
                for c in range(P.C2):
                    if len(pre_sched[w]) >= PRE_PER_B:
                        break
                    if c not in taken and bound[c] <= w:
                        pre_sched[w].append(c)
                        taken.add(c)

        for _rep in range(reps):
            pre_tiles.clear()
            agg_phase(XT[:], g1idx_t, eloc1_t, wsel1_t, eloc1b_t, wsel1b_t,
                      is2_1, P.W1_list, "g1", p1_chunk, p1_win, dt_p1)

            # phase 2: e2v aggregation (sel weights pre-normalized on host)
            sb = set()
            prev = None
            for t, (w_, cw_) in enumerate(P.p2cols):
                if t < P.npre2 or w_ >= 2:
                    prev = None
                    continue
                b = int(bound[t])
                if prev is not None and b > prev:
                    sb.add(t)
                prev = b
            agg_phase(CCOUT[:], g2idx_t, vloc2_t, w2raw_t, vloc2b_t, w2rawb_t,
                      is2_2, P.W2_list, "g2", p2_chunk, p2_win, dt_p2,
                      src_sel=p2_src, pre_tiles=pre_tiles, pool=g2pool,
                      cols=P.p2cols, split_bounds=sb)

    nc.compile()
    return nc


# ------------------------------------------------------------------ runner ---
def make_in_maps(P, X, W, b, bf16=GATHER_BF16, p1_dt=P1_DT, fuse=FUSE):
    npdt = ml_dtypes.bfloat16 if bf16 else np.float32
    np_p1 = _npdt(p1_dt)
    s1 = P1_SCALE if p1_dt == "f8" else 1.0
    s2 = P2_SCALE if fuse == "post" else 1.0
    KT = P.ch // 128
    xt = np.ascontiguousarray((X * s1).astype(np_p1))
    wt = np.ascontiguousarray(
        W.T.reshape(KT, 128, P.ch).transpose(1, 0, 2).astype(npdt))
    bt = np.ascontiguousarray(b.astype(npdt).reshape(1, P.ch))
    iota = np.ascontiguousarray(
        np.broadcast_to(np.arange(128, dtype=npdt), (128, 128)))
    ident = np.eye(128, dtype=npdt)

    def tb(flat, C, s=1.0):
        return _pack(flat, C) / np.float32(s)

    in_maps = []
    for k in range(P.ncores):
        g1, l1, w1, l1b, w1b = P.p1[k]
        g2, l2, w2, l2b, w2b = P.p2[k]
        p1tab = np.ascontiguousarray(np.stack(
            [tb(l1, P.C1), tb(w1, P.C1, s1), tb(l1b, P.C1), tb(w1b, P.C1, s1)],
            axis=1))
        p2tab = np.ascontiguousarray(np.stack(
            [tb(l2, P.C2), tb(w2, P.C2, s2), tb(l2b, P.C2), tb(w2b, P.C2, s2)],
            axis=1))
        in_maps.append({
            "xt": xt, "wt": wt, "bt": bt, "iota": iota, "ident": ident,
            "g1idx": _wrap_idx(g1), "p1tab": p1tab,
            "g2idx": _wrap_idx(g2), "p2tab": p2tab,
        })
    return in_maps


def assemble(P, shards):
    out = np.zeros((P.nv, P.ch), np.float32)
    for k in range(P.ncores):
        vm = P.vmap[k]
        m = vm >= 0
        out[vm[m]] = shards[k][m].astype(np.float32)
    return out


_nc_cache = {}


def kernel(X, W, b, e2v_weight, v_idx, e_idx):
    global _last_results
    from concourse.bass_utils import run_bass_kernel_spmd

    P = make_plan(v_idx, e_idx, e2v_weight)
    key = (P.C1, P.C2, P.W1, P.W2, GATHER_BF16, P1_DT, P2_DT, OUT_DT, FUSE,
           AG_SLICED)
    if key not in _nc_cache:
        _nc_cache[key] = build_nc(P)
    nc = _nc_cache[key]
    in_maps = make_in_maps(P, X, W, b)
    res = run_bass_kernel_spmd(nc, in_maps, list(range(P.ncores)), trace=TRACE)
    _last_results = res
    shards = [res.results[k]["out"] for k in range(P.ncores)]
    return assemble(P, shards)



# revision 114
# speedup vs baseline: 1.1080x; 1.0394x over previous
"""HGNNPConv Trainium2 kernel (8 NeuronCores, SPMD).

Math (equivalent reformulation of the reference):
  Xe_raw[e] = mean_{i: e_idx[i]=e} X[v_idx[i]]              (v2e, softmax of ones = 1/deg)
  Xe_p      = Xe_raw @ W.T + b                              (GEMM on 4000 edges, not 20000 verts)
  Xv[v]     = sum_i wn_i * Xe_p[e_idx[i]],  wn_i = exp(w_i)/sum_{v} exp(w)
              (wn precomputed on host -> no on-chip denominator pass)
  out       = relu(Xv)
Deg-0 vertices are masked host-side in assemble(); empty edges get a spurious
+b in Xe_p but are never referenced downstream.

Sharding: edges and vertices are binpacked across the 8 cores (balancing
post-dedup gather slots).  Phase 1 aggregates by destination edge, runs the
edge-level GEMM per 128-edge window (bias folded in as a K=1 matmul), and
AllGathers each window's table slice as soon as it is ready (overlapping the
collective with the remaining phase-1 work).  Phase 2 aggregates by
destination vertex; each gather call's source AP is a prefix slice of the
table covering only the AllGather pieces it needs.

Per-destination-window weighted one-hot selection matrices (built on DVE
from a bf16 iota) reduce gathered rows on the PE into PSUM.  Incidences
sharing a source row within a window are deduplicated into one gathered slot
with a two-hot sel column (~11% fewer gather bytes; window binpacking is
rebalanced for post-dedup slot counts, phase 2 uses 21 windows x 7 chunks
instead of 20 x 8, and per-window chunk counts are variable — windows are
sorted largest-first so cross-core padding aligns and each phase ends on its
smallest window).

Dtypes: phase-1 gather table fp8-e3m4 (X pre-scaled by 2: worst-case quant
error halves vs e4m3 and the GEMM averages it out), phase-2 table bf16
(absmax-norm tolerance rules out fp8 post-GEMM), output bf16 (upcast on
host).  End-to-end rel err 1.35e-2 vs the 2e-2 gate.
"""

import os
from contextlib import ExitStack

import numpy as np
import ml_dtypes

# ---------------------------------------------------------------- config ---
NCORES = 8
NV, NE, NNZ, CH = 20000, 4000, 160000, 512
GATHER_BF16 = os.environ.get("KERNEL_F32", "") == ""  # bf16 tables+matmuls by default
P1_DT = os.environ.get("KERNEL_P1_DT", "f8")   # phase-1 gather table dtype
P2_DT = os.environ.get("KERNEL_P2_DT", "bf16")  # phase-2 gather table dtype
OUT_DT = os.environ.get("KERNEL_OUT_DT", "bf16")
FUSE = os.environ.get("KERNEL_FUSE", "pre")    # "post": GEMM after p2 agg
P1_SCALE = 2.0   # X pre-scale for fp8-e3m4 range use (exact power of 2)
P2_SCALE = 8.0   # Xe_raw pre-scale for the fp8 edge table (post mode)
GRP = 5          # gather chunks (of 128 idxs) per dma_gather call
AG_SLICED = os.environ.get("KERNEL_AG_SLICED", "1") != ""  # per-window AllGather
PRE_PER_B = int(os.environ.get("KERNEL_PRE_PER_B", "0"))  # p2 prefetches per p1 window
CW_MAJOR = os.environ.get("KERNEL_CW_MAJOR", "") != ""  # cw-major p2 table layout
TRACE = os.environ.get("BASS_TRACE", "") != ""


def _mydt(mybir, name):
    return {"f8": mybir.dt.float8e3, "bf16": mybir.dt.bfloat16,
            "f32": mybir.dt.float32}[name]


def _npdt(name):
    return {"f8": ml_dtypes.float8_e3m4, "bf16": ml_dtypes.bfloat16,
            "f32": np.float32}[name]

_last_results = None   # BassKernelResults of the most recent run (for test.py)


# ------------------------------------------------------------------- plan ---
class Plan:
    pass


def _binpack(ids, degs, nbins, cap=128):
    """Pack `ids` into `nbins` bins of <=cap items, balancing sum(degs)."""
    import heapq

    order = np.argsort(-degs, kind="stable")
    bins = [[] for _ in range(nbins)]
    loads = [0] * nbins
    heap = [(0, b) for b in range(nbins)]
    heapq.heapify(heap)
    for t in order:
        popped = []
        while True:
            load, b = heapq.heappop(heap)
            if len(bins[b]) < cap:
                break
            popped.append((load, b))
        for p in popped:
            heapq.heappush(heap, p)
        bins[b].append(int(ids[t]))
        loads[b] = load + int(degs[t])
        heapq.heappush(heap, (loads[b], b))
    return bins, loads


def _csr(idx, n):
    order = np.argsort(idx, kind="stable").astype(np.int64)
    deg = np.bincount(idx, minlength=n).astype(np.int64)
    starts = np.zeros(n + 1, np.int64)
    np.cumsum(deg, out=starts[1:])
    return order, deg, starts


def _pair_window(src, locs, ws):
    """Greedy within-window dedup: incidences sharing a source row become one
    gathered slot with two (loc, w) hots.  Returns (src', loc_a, w_a, loc_b,
    w_b) with pair slots FIRST; singles have loc_b = -1 / w_b = 0."""
    order = np.argsort(src, kind="stable")
    src, locs, ws = src[order], locs[order], ws[order]
    pa, pb, sg = [], [], []
    i, n = 0, len(src)
    while i < n:
        j = i
        while j < n and src[j] == src[i]:
            j += 1
        k = i
        while k + 1 < j:
            pa.append(k); pb.append(k + 1); k += 2
        if k < j:
            sg.append(k)
        i = j
    pa, pb, sg = np.array(pa, np.int64), np.array(pb, np.int64), np.array(sg, np.int64)
    src2 = np.concatenate([src[pa], src[sg]]) if len(pa) else src[sg]
    la = np.concatenate([locs[pa], locs[sg]]) if len(pa) else locs[sg]
    wa = np.concatenate([ws[pa], ws[sg]]) if len(pa) else ws[sg]
    lb = np.concatenate([locs[pb], np.full(len(sg), -1.0, locs.dtype)]) if len(pa) \
        else np.full(len(sg), -1.0, locs.dtype)
    wb = np.concatenate([ws[pb], np.zeros(len(sg), ws.dtype)]) if len(pa) \
        else np.zeros(len(sg), ws.dtype)
    return src2, la, wa, lb, wb, len(pa)


def _phase_windows(bins_per_core, order, starts, idx_of_inc, w_of_inc,
                   loc_dtype=np.float32, pair=True, piece_order=False):
    """Per-window slot lists for one core of one phase, after source dedup.

    Returns (wins, wmax): wins[w] = (src, loc_a, w_a, loc_b, w_b).  By
    default pair slots (loc_b >= 0) come first within each window; with
    piece_order=True, singles come first sorted by source id (so early
    chunks only reference low table pieces) and pair slots go last.
    """
    wins = []
    wmax = 0
    for bin_ids in bins_per_core:
        incs = []
        locs = []
        for j, d in enumerate(bin_ids):
            seg = order[starts[d]:starts[d + 1]]
            incs.append(seg)
            locs.append(np.full(len(seg), j, loc_dtype))
        incs = np.concatenate(incs) if incs else np.zeros(0, np.int64)
        locs = np.concatenate(locs) if locs else np.zeros(0, loc_dtype)
        src = idx_of_inc[incs]
        ws = w_of_inc[incs].astype(np.float32)
        if pair and len(src):
            src, la, wa, lb, wb, npair = _pair_window(src, locs, ws)
            if piece_order:
                ps = np.argsort(src[:npair], kind="stable")
                ss = npair + np.argsort(src[npair:], kind="stable")
                perm = np.concatenate([ss, ps])
                src, la, wa, lb, wb = (src[perm], la[perm], wa[perm],
                                       lb[perm], wb[perm])
        else:
            la, wa = locs, ws
            lb = np.full(len(src), -1.0, loc_dtype)
            wb = np.zeros(len(src), np.float32)
        wins.append((src, la, wa, lb, wb))
        wmax = max(wmax, len(src))
    return wins, wmax


def _layout(wins, W_list):
    """Flat slot arrays with per-window chunk counts.  Window w's cw-th
    chunk sits at table column offs[w] + cw."""
    offs = np.concatenate([[0], np.cumsum(W_list)])
    cols = []
    for w, Wx in enumerate(W_list):
        cols.extend((w, cw) for cw in range(Wx))
    return _layout_cols(wins, cols)


def _layout_cols(wins, cols):
    """Flat slot arrays for an explicit column list of (window, cw)."""
    L = len(cols) * 128
    gidx = np.zeros(L, np.int16)
    loc = np.full(L, -1.0, np.float32)
    wsel = np.zeros(L, np.float32)
    locb = np.full(L, -1.0, np.float32)
    wselb = np.zeros(L, np.float32)
    for t, (w, cw) in enumerate(cols):
        src, la, wa, lb, wb = wins[w]
        i0, i1 = cw * 128, min((cw + 1) * 128, len(src))
        if i0 >= len(src):
            continue
        o, n = t * 128, i1 - i0
        gidx[o:o + n] = src[i0:i1]
        loc[o:o + n] = la[i0:i1]
        wsel[o:o + n] = wa[i0:i1]
        locb[o:o + n] = lb[i0:i1]
        wselb[o:o + n] = wb[i0:i1]
    return gidx, loc, wsel, locb, wselb


def _wrap_idx(flat):
    """int16 flat[i] -> [128, len/16] with value i at [i%16, i//16], replicated."""
    a = flat.reshape(-1, 16).T  # [16, L/16]
    return np.ascontiguousarray(np.tile(a, (8, 1)))


def _pack(flat, C):
    """flat[c*128+p] -> [128, C]"""
    return np.ascontiguousarray(flat.reshape(C, 128).T)


def _dedup_slots(srcs):
    """#gather slots for a window's source list after pairing."""
    if not len(srcs):
        return 0
    _, cnt = np.unique(srcs, return_counts=True)
    return int(((cnt + 1) // 2).sum())


def _repair_bins(bins, order, starts, idx_of_inc, cap=128, iters=400):
    """Greedy rebalance: move members out of the window with the most
    post-dedup slots into the one with the fewest (respecting the member
    cap), to minimize max slots per window."""
    srcs = [
        [idx_of_inc[order[starts[d]:starts[d + 1]]] for d in b] for b in bins
    ]

    def slots(w):
        return _dedup_slots(np.concatenate(srcs[w]) if srcs[w] else
                            np.zeros(0, np.int64))

    cur = [slots(w) for w in range(len(bins))]
    for _ in range(iters):
        hot = int(np.argmax(cur))
        order_cold = np.argsort(cur)
        moved = False
        for cold in order_cold:
            if cold == hot or len(bins[cold]) >= cap:
                continue
            # move the member with the smallest segment out of `hot`
            j = int(np.argmin([len(s) for s in srcs[hot]]))
            bins[cold].append(bins[hot].pop(j))
            srcs[cold].append(srcs[hot].pop(j))
            new_hot, new_cold = slots(hot), slots(cold)
            if max(new_hot, new_cold) >= cur[hot]:
                # revert: no improvement
                bins[hot].append(bins[cold].pop())
                srcs[hot].append(srcs[cold].pop())
                continue
            cur[hot], cur[cold] = new_hot, new_cold
            moved = True
            break
        if not moved:
            break
    return bins


def make_plan(v_idx, e_idx, e2v_weight, nv=NV, ne=NE, ch=CH, ncores=NCORES):
    P = Plan()
    P.nv, P.ne, P.ch, P.ncores = nv, ne, ch, ncores
    epc, vpc = ne // ncores, nv // ncores
    P.epc, P.vpc = epc, vpc

    order_e, deg_e, starts_e = _csr(e_idx, ne)
    order_v, deg_v, starts_v = _csr(v_idx, nv)
    inv_deg = np.zeros(ne, np.float32)
    nz = deg_e > 0
    inv_deg[nz] = (np.float32(1.0) / deg_e[nz].astype(np.float32))

    nb1 = -(-epc // 128)
    nb2 = -(-vpc // 128)
    v_of_inc = v_idx.astype(np.int64)
    e_of_inc = e_idx.astype(np.int64)
    # balance destinations across cores globally (assignment is free — pos /
    # vmap carry it), then binpack windows within each core and rebalance for
    # post-dedup slot counts.
    cores_e, _ = _binpack(np.arange(ne), deg_e, ncores, cap=nb1 * 128)
    bins1 = []
    for k in range(ncores):
        eids = np.asarray(cores_e[k])
        b, _ = _binpack(eids, deg_e[eids], nb1)
        bins1.append(_repair_bins(b, order_e, starts_e, v_of_inc))

    # p2 window count: an extra window can admit a smaller W2 (less gather
    # padding) once dedup shrinks the per-window loads — pick the best.
    best = None
    for nb2c in (nb2, nb2 + 1):
        cores_v, _ = _binpack(np.arange(nv), deg_v, ncores, cap=nb2c * 128)
        cand = []
        sizes = []
        for k in range(ncores):
            vids = np.asarray(cores_v[k])
            b, _ = _binpack(vids, deg_v[vids], nb2c)
            b = _repair_bins(b, order_v, starts_v, e_of_inc)
            cand.append(b)
            sizes.append(sorted(
                (-(-_dedup_slots(np.concatenate(
                    [e_of_inc[order_v[starts_v[d]:starts_v[d + 1]]]
                     for d in bb]) if bb else np.zeros(0, np.int64)) // 128)
                 for bb in b), reverse=True))
        C2c = sum(max(sizes[k][w] for k in range(ncores))
                  for w in range(nb2c))
        if best is None or C2c < best[1]:
            best = (nb2c, C2c, cand)
    nb2, _, bins2 = best
    P.NW1, P.NW2 = nb1, nb2

    # phase-1 windows (dedup within window) + edge position map.  Positions
    # are window-major (w, core, row) so each window's table slice can be
    # AllGathered independently as soon as its GEMM finishes.  Windows are
    # sorted largest-first per core: cross-core maxes align (less padding)
    # and the smallest window lands last (shorter phase tail).
    pos = np.zeros(ne, np.int64)
    wins1 = []
    w1max = 0
    for k in range(ncores):
        wins, wmax = _phase_windows(
            bins1[k], order_e, starts_e, v_idx.astype(np.int64),
            inv_deg[e_idx.astype(np.int64)])
        order = np.argsort([-len(w[0]) for w in wins], kind="stable")
        wins = [wins[i] for i in order]
        bins1[k] = [bins1[k][i] for i in order]
        wins1.append(wins)
        w1max = max(w1max, wmax)
        for w, bin_ids in enumerate(bins1[k]):
            for j, e in enumerate(bin_ids):
                if AG_SLICED:   # window-major: (w, core, row)
                    pos[e] = (w * ncores + k) * 128 + j
                else:           # core-major: AllGather rank concatenation
                    pos[e] = (k * nb1 + w) * 128 + j
    assert pos.max() < 32768
    P.W1_list = [
        max(-(-len(wins1[k][w][0]) // 128) for k in range(ncores))
        for w in range(nb1)
    ]
    P.W1 = max(P.W1_list)
    P.C1 = int(sum(P.W1_list))
    P.p1 = [_layout(wins, P.W1_list) for wins in wins1]

    # phase-2 windows + output row map. Softmax weights are fully normalized
    # on the host (exp / per-vertex sum), so the kernel needs no denominator
    # pass.
    expw = np.exp(e2v_weight.astype(np.float64))
    den = np.zeros(nv, np.float64)
    np.add.at(den, v_idx, expw)
    wnorm = (expw / den[v_idx.astype(np.int64)]).astype(np.float32)
    wins2 = []
    P.vmap = []
    for k in range(ncores):
        wins, _ = _phase_windows(
            bins2[k], order_v, starts_v, pos[e_idx.astype(np.int64)], wnorm,
            piece_order=AG_SLICED)
        order = np.argsort([-len(w[0]) for w in wins], kind="stable")
        wins = [wins[i] for i in order]
        bins2[k] = [bins2[k][i] for i in order]
        wins2.append(wins)
        vm = np.full(P.NW2 * 128, -1, np.int64)
        for w, bin_ids in enumerate(bins2[k]):
            vm[w * 128:w * 128 + len(bin_ids)] = bin_ids
        # deg-0 vertices never receive contributions; drop them from the
        # output map so any on-chip garbage (e.g. a stray +b) is discarded.
        vme = vm[vm >= 0]
        vm[vm >= 0] = np.where(deg_v[vme] > 0, vme, -1)
        P.vmap.append(vm)
    P.W2_list = [
        max(-(-len(wins2[k][w][0]) // 128) for k in range(ncores))
        for w in range(nb2)
    ]
    P.W2 = max(P.W2_list)
    P.C2 = int(sum(P.W2_list))
    # pre-block: chunk-0 of the first windows, but only if they reference
    # table piece 0 alone (so a gather over them can run during phase 1,
    # right after the first window's AllGather).  They form the leading
    # contiguous columns of the table layout.
    piece = ncores * 128
    npre = 0 if not AG_SLICED else min(20, nb2)
    npre_cap, npre = npre, 0
    for w in range(npre_cap):
        b0max = max(
            int(wins2[k][w][0][:128].max()) if len(wins2[k][w][0]) else 0
            for k in range(ncores))
        if b0max >= piece or P.W2_list[w] < 2:
            break
        npre += 1
    npre -= npre % 5            # whole grp-5 calls only
    P.npre2 = npre
    cols = [(w, 0) for w in range(npre)]
    for w in range(nb2):
        for cw in (range(1, P.W2_list[w]) if w < npre
                   else range(P.W2_list[w])):
            cols.append((w, cw))
    P.p2cols = cols
    P.p2 = [_layout_cols(wins, cols) for wins in wins2]
    return P


# ---------------------------------------------------------------- builder ---
def build_nc(P, bf16=GATHER_BF16, spmd=True, reps=1, grp=GRP, gbufs=5,
             nqueues=1, p1_dt=P1_DT, p2_dt=P2_DT, out_dt=OUT_DT, fuse=FUSE):
    import concourse.bacc as bacc
    import concourse.mybir as mybir
    import concourse.tile as tile

    f32 = mybir.dt.float32
    dt_g = mybir.dt.bfloat16 if bf16 else f32   # sel matrices + GEMM operands
    dt_p1 = _mydt(mybir, p1_dt)
    dt_p2 = _mydt(mybir, p2_dt)
    dt_out = _mydt(mybir, out_dt)
    eq, mul, mx, add = (mybir.AluOpType.is_equal, mybir.AluOpType.mult,
                        mybir.AluOpType.max, mybir.AluOpType.add)
    ch, KT = P.ch, P.ch // 128
    post = fuse == "post"

    nc = bacc.Bacc("TRN2", target_bir_lowering=False, debug=False,
                   num_devices=P.ncores if spmd else 1,
                   num_swdge_queues=nqueues)

    XT = nc.dram_tensor("xt", [P.nv, ch], dt_p1, kind="ExternalInput")
    WT = nc.dram_tensor("wt", [128, KT, ch], dt_g, kind="ExternalInput")
    BT = nc.dram_tensor("bt", [1, ch], dt_g, kind="ExternalInput")
    IOTA = nc.dram_tensor("iota", [128, 128], dt_g, kind="ExternalInput")
    IDENT = nc.dram_tensor("ident", [128, 128], dt_g, kind="ExternalInput")
    G1IDX = nc.dram_tensor("g1idx", [128, P.C1 * 8], mybir.dt.int16, kind="ExternalInput")
    P1TAB = nc.dram_tensor("p1tab", [128, 4, P.C1], f32, kind="ExternalInput")
    G2IDX = nc.dram_tensor("g2idx", [128, P.C2 * 8], mybir.dt.int16, kind="ExternalInput")
    P2TAB = nc.dram_tensor("p2tab", [128, 4, P.C2], f32, kind="ExternalInput")

    ner1 = P.NW1 * 128
    CCIN = nc.dram_tensor("ccin", [ner1, ch], dt_p2)
    CCOUT = nc.dram_tensor("ccout", [P.ncores * ner1, ch], dt_p2, addr_space="Shared")
    OUT = nc.dram_tensor("out", [P.NW2 * 128, ch], dt_out, kind="ExternalOutput")

    with tile.TileContext(nc) as tc, ExitStack() as ctx:
        const = ctx.enter_context(tc.tile_pool(name="const", bufs=1))
        gpool = ctx.enter_context(tc.tile_pool(name="g", bufs=gbufs))
        g2pool = ctx.enter_context(tc.tile_pool(name="g2", bufs=gbufs))
        prep = ctx.enter_context(tc.tile_pool(name="prep", bufs=1))
        selp = ctx.enter_context(tc.tile_pool(name="selp", bufs=8))
        psum = ctx.enter_context(tc.tile_pool(name="ps", bufs=2, space="PSUM"))
        sbp = ctx.enter_context(tc.tile_pool(name="sbp", bufs=3))
        pre_tiles = {}

        def cload(dram, shape, dt, tag, eng=None):
            t = const.tile(shape, dt, tag=tag)
            (eng or nc.sync).dma_start(t[:], dram[:])
            return t

        # p1-critical tables first (SP ring); the rest go on the ACT ring so
        # they never delay the first gather.  g1idx leads: the first gather
        # needs only it, iota/p1tab are for the (later) first sel build.
        g1idx_t = cload(G1IDX, [128, P.C1 * 8], mybir.dt.int16, "g1idx")
        iota_t = cload(IOTA, [128, 128], dt_g, "iota")
        p1tab_t = cload(P1TAB, [128, 4, P.C1], f32, "p1tab")
        eloc1_t, wsel1_t = p1tab_t[:, 0, :], p1tab_t[:, 1, :]
        eloc1b_t, wsel1b_t = p1tab_t[:, 2, :], p1tab_t[:, 3, :]
        wt_t = cload(WT, [128, KT, ch], dt_g, "wt", eng=nc.scalar)
        bt_t = cload(BT, [1, ch], dt_g, "bt", eng=nc.scalar)
        ident_t = cload(IDENT, [128, 128], dt_g, "ident", eng=nc.scalar)
        g2idx_t = cload(G2IDX, [128, P.C2 * 8], mybir.dt.int16, "g2idx",
                        eng=nc.scalar)
        p2tab_t = cload(P2TAB, [128, 4, P.C2], f32, "p2tab", eng=nc.scalar)
        vloc2_t, w2raw_t = p2tab_t[:, 0, :], p2tab_t[:, 1, :]
        vloc2b_t, w2rawb_t = p2tab_t[:, 2, :], p2tab_t[:, 3, :]
        ones1_t = const.tile([1, 128], dt_g, tag="ones1")
        nc.vector.memset(ones1_t[:], 1.0)

        # ---------------- gather + one/two-hot reduce ----------------------
        def agg_phase(src_ap, gidx_t, loc_t, w_t, locb_t, wb_t, is2, W_list,
                      gtag, chunk_cb, win_cb, dt_tab, src_sel=None,
                      pre_tiles=None, pool=None, cols=None, split_bounds=None):
            """Consumption iterates (window, cw) order; `cols` gives each
            table column's (window, cw) — prefetched pre-block columns may
            sit outside their window's run."""
            pre_tiles = pre_tiles or {}
            pool = pool or gpool
            if cols is None:
                cols = [(w, cw) for w, Wx in enumerate(W_list)
                        for cw in range(Wx)]
            C = len(cols)
            pos_order = sorted(range(C), key=lambda t: cols[t])
            # calls = runs of consecutive non-prefetched table columns; in
            # the early columns a run also breaks where the source-prefix
            # bound rises, so the low-piece part never waits on later
            # AllGather pieces
            calls = []
            run = []
            for t in range(C):
                brk = t in pre_tiles or len(run) == grp
                if run and split_bounds is not None and t in split_bounds:
                    brk = True
                if brk:
                    if run:
                        calls.append((run[0], len(run)))
                    run = []
                if t not in pre_tiles:
                    run.append(t)
            if run:
                calls.append((run[0], len(run)))
            call_of = {}
            for g0, n in calls:
                for j in range(n):
                    call_of[g0 + j] = (g0, n)
            tiles = {}
            pw = None
            for t in pos_order:
                if t in call_of and call_of[t][0] == t:
                    g0, n = call_of[t]
                    gt_new = pool.tile([128, grp, ch], dt_tab, tag=gtag)
                    src = src_sel(g0, n) if src_sel is not None else src_ap
                    nc.gpsimd.dma_gather(
                        gt_new[:, 0:n, :], src,
                        gidx_t[:, g0 * 8:(g0 + n) * 8],
                        n * 128, n * 128, ch,
                        queue_num=(g0 // grp) % nqueues)
                    tiles[g0] = gt_new
                w, cw = cols[t]
                if t in pre_tiles:
                    gt, j = pre_tiles[t]
                else:
                    g0, n = call_of[t]
                    gt, j = tiles[g0], t - g0
                sel = selp.tile([128, 128], dt_g, tag="sel")
                nc.vector.tensor_scalar(
                    sel[:], iota_t[:], loc_t[:, t:t + 1], w_t[:, t:t + 1],
                    op0=eq, op1=mul)
                if is2[t]:  # dedup chunk: add the second hot
                    selb = selp.tile([128, 128], dt_g, tag="selb")
                    nc.vector.tensor_scalar(
                        selb[:], iota_t[:], locb_t[:, t:t + 1],
                        wb_t[:, t:t + 1], op0=eq, op1=mul)
                    sel2 = selp.tile([128, 128], dt_g, tag="sel2")
                    nc.vector.tensor_tensor(sel2[:], sel[:], selb[:], op=add)
                    sel = sel2
                if cw == 0:
                    pw = psum.tile([128, ch], f32, tag="win")
                last = cw == W_list[w] - 1
                chunk_cb(pw, sel, gt, j, w, cw, last)
                if last:
                    win_cb(pw, w)

        def p1_chunk(pw, sel, gt, j, w, cw, last):
            nc.tensor.matmul(pw[:], sel[:], gt[:, j, :],
                             start=(cw == 0), stop=last)

        def gemm_bias(src_t, dst_psum):
            """dst[v/e, co] = src^T blocks @ W.T + 1^T b (K=1 bias matmul)."""
            for k in range(KT):
                nc.tensor.matmul(dst_psum[:], src_t[:, k, :], wt_t[:, k, :],
                                 start=(k == 0), stop=False)
            nc.tensor.matmul(dst_psum[:], ones1_t[:], bt_t[:],
                             start=False, stop=True)

        def transpose_blocks(pw, tag):
            """psum [128, ch] f32 -> sbuf [128, KT, 128] dt_g transposed.
            Copies are per-128-block so transpose k pipelines with copy k+1."""
            tT_w = sbp.tile([128, KT, 128], dt_g, tag=tag + "T", name=tag + "T")
            for k in range(KT):
                twk = sbp.tile([128, 128], dt_g, tag=tag + "f", name=tag + "f")
                nc.vector.tensor_copy(twk[:], pw[:, k * 128:(k + 1) * 128])
                pt = psum.tile([128, 128], dt_g, tag="aux", name="pt")
                nc.tensor.transpose(pt[:], twk[:], ident_t[:])
                nc.vector.tensor_copy(tT_w[:, k, :], pt[:])
            return tT_w

        def p1_win(pw, w):
            # window w's edge rows are complete: ship its CCIN slice and
            # immediately AllGather that window's table piece, overlapping
            # the collective with the remaining p1 windows.
            xep = sbp.tile([128, ch], dt_p2, tag="xep", name="xep")
            if post:
                # raw table, scaled for fp8 range; GEMM happens after p2 agg
                nc.vector.tensor_scalar(xep[:], pw[:], float(P2_SCALE), None,
                                        op0=mul)
            else:
                xeT_w = transpose_blocks(pw, "xe")
                pg = psum.tile([128, ch], f32, tag="gemm", name="pg")
                gemm_bias(xeT_w, pg)
                nc.vector.tensor_copy(xep[:], pg[:])
            nc.sync.dma_start(CCIN[w * 128:(w + 1) * 128, :], xep[:])
            if AG_SLICED:
                o = w * P.ncores * 128
                if spmd:
                    nc.gpsimd.collective_compute(
                        "AllGather", mybir.AluOpType.bypass,
                        replica_groups=[list(range(P.ncores))],
                        ins=[CCIN[w * 128:(w + 1) * 128, :]],
                        outs=[CCOUT[o:o + P.ncores * 128, :]])
                else:  # single-core stand-in for the window AllGather
                    nc.sync.dma_start(CCOUT[o:o + 128, :],
                                      CCIN[w * 128:(w + 1) * 128, :])
            elif w == P.NW1 - 1:
                if spmd:
                    nc.gpsimd.collective_compute(
                        "AllGather", mybir.AluOpType.bypass,
                        replica_groups=[list(range(P.ncores))],
                        ins=[CCIN[:]], outs=[CCOUT[:]])
                else:
                    nc.sync.dma_start(CCOUT[0:ner1, :], CCIN[:])
            # prefetch the p2 pre-block (piece-0-only chunk-0 columns) during
            # p1, one window after its AllGather piece was triggered — soaks
            # p1's spare DMA bandwidth and thins p2's DMA-bound span
            if w >= 1:
                g0s = [(w - 1) * grp]
                if w == P.NW1 - 1:   # last boundary takes the leftovers
                    g0s += list(range(w * grp, P.npre2, grp))
                for g0 in g0s:
                    if g0 >= P.npre2:
                        continue
                    n = min(grp, P.npre2 - g0)
                    gt = prep.tile([128, grp, ch], dt_p2, tag=f"pre{g0}")
                    nc.gpsimd.dma_gather(
                        gt[:, 0:n, :], p2_src(g0, n),
                        g2idx_t[:, g0 * 8:(g0 + n) * 8],
                        n * 128, n * 128, ch, queue_num=0)
                    for j in range(n):
                        pre_tiles[g0 + j] = (gt, j)

        def p2_chunk(pw, sel, gt, j, w, cw, last):
            nc.tensor.matmul(pw[:], sel[:], gt[:, j, :],
                             start=(cw == 0), stop=last)

        def p2_win(pw, w):
            if post:
                awT = transpose_blocks(pw, "aw")
                po = psum.tile([128, ch], f32, tag="gemm", name="po")
                gemm_bias(awT, po)
                pw = po
            # weights pre-normalized on host: just relu + store
            ow = sbp.tile([128, ch], dt_out, tag="ow", name="ow")
            nc.vector.tensor_scalar(ow[:], pw[:], 1.0, 0.0, op0=mul, op1=mx)
            nc.sync.dma_start(OUT[w * 128:(w + 1) * 128, :], ow[:])

        # chunks that contain any dedup pair need the second sel pass; the
        # union over cores keeps the SPMD program identical on every core.
        is2_1 = np.zeros(P.C1, bool)
        is2_2 = np.zeros(P.C2, bool)
        maxrow2 = np.zeros(P.C2, np.int64)
        for k in range(P.ncores):
            is2_1 |= (_pack(P.p1[k][3], P.C1) >= 0).any(axis=0)
            is2_2 |= (_pack(P.p2[k][3], P.C2) >= 0).any(axis=0)
            maxrow2 = np.maximum(
                maxrow2, _pack(P.p2[k][0], P.C2).astype(np.int64).max(axis=0))

        piece = P.ncores * 128
        def p2_src(g0, n):
            # prefix slice of the edge table covering every row this gather
            # call touches, so it only waits on the AllGather pieces it needs
            pieces = int(maxrow2[g0:g0 + n].max()) // piece + 1
            if not AG_SLICED or pieces >= P.NW1:
                return CCOUT[:]
            return CCOUT[0:pieces * piece, :]

        # p2 prefetch schedule: at p1 window boundary w we may issue gathers
        # for p2 chunks that only need table pieces < w (their AllGather was
        # triggered a full window earlier).  Earliest-consumed chunks first.
        bound = (maxrow2 // piece + 1).astype(int)   # pieces needed per chunk
        pre_sched = {w: [] for w in range(1, P.NW1)}
        if AG_SLICED and PRE_PER_B > 0:
            taken = set()
            for w in range(1, P.NW1):
                for c in range(P.C2):
                    if len(pre_sched[w]) >= PRE_PER_B:
                        break
                    if c not in taken and bound[c] <= w:
                        pre_sched[w].append(c)
                        taken.add(c)

        for _rep in range(reps):
            pre_tiles.clear()
            agg_phase(XT[:], g1idx_t, eloc1_t, wsel1_t, eloc1b_t, wsel1b_t,
                      is2_1, P.W1_list, "g1", p1_chunk, p1_win, dt_p1)

            # phase 2: e2v aggregation (sel weights pre-normalized on host)
            sb = set()
            prev = None
            for t, (w_, cw_) in enumerate(P.p2cols):
                if t < P.npre2 or w_ >= 2:
                    prev = None
                    continue
                b = int(bound[t])
                if prev is not None and b > prev:
                    sb.add(t)
                prev = b
            agg_phase(CCOUT[:], g2idx_t, vloc2_t, w2raw_t, vloc2b_t, w2rawb_t,
                      is2_2, P.W2_list, "g2", p2_chunk, p2_win, dt_p2,
                      src_sel=p2_src, pre_tiles=pre_tiles, pool=g2pool,
                      cols=P.p2cols, split_bounds=sb)

    nc.compile()
    return nc


# ------------------------------------------------------------------ runner ---
def make_in_maps(P, X, W, b, bf16=GATHER_BF16, p1_dt=P1_DT, fuse=FUSE):
    npdt = ml_dtypes.bfloat16 if bf16 else np.float32
    np_p1 = _npdt(p1_dt)
    s1 = P1_SCALE if p1_dt == "f8" else 1.0
    s2 = P2_SCALE if fuse == "post" else 1.0
    KT = P.ch // 128
    xt = np.ascontiguousarray((X * s1).astype(np_p1))
    wt = np.ascontiguousarray(
        W.T.reshape(KT, 128, P.ch).transpose(1, 0, 2).astype(npdt))
    bt = np.ascontiguousarray(b.astype(npdt).reshape(1, P.ch))
    iota = np.ascontiguousarray(
        np.broadcast_to(np.arange(128, dtype=npdt), (128, 128)))
    ident = np.eye(128, dtype=npdt)

    def tb(flat, C, s=1.0):
        return _pack(flat, C) / np.float32(s)

    in_maps = []
    for k in range(P.ncores):
        g1, l1, w1, l1b, w1b = P.p1[k]
        g2, l2, w2, l2b, w2b = P.p2[k]
        p1tab = np.ascontiguousarray(np.stack(
            [tb(l1, P.C1), tb(w1, P.C1, s1), tb(l1b, P.C1), tb(w1b, P.C1, s1)],
            axis=1))
        p2tab = np.ascontiguousarray(np.stack(
            [tb(l2, P.C2), tb(w2, P.C2, s2), tb(l2b, P.C2), tb(w2b, P.C2, s2)],
            axis=1))
        in_maps.append({
            "xt": xt, "wt": wt, "bt": bt, "iota": iota, "ident": ident,
            "g1idx": _wrap_idx(g1), "p1tab": p1tab,
            "g2idx": _wrap_idx(g2), "p2tab": p2tab,
        })
    return in_maps


def assemble(P, shards):
    out = np.zeros((P.nv, P.ch), np.float32)
    for k in range(P.ncores):
        vm = P.vmap[k]
        m = vm >= 0
        out[vm[m]] = shards[k][m].astype(np.float32)
    return out


_nc_cache = {}


def kernel(X, W, b, e2v_weight, v_idx, e_idx):
    global _last_results
    from concourse.bass_utils import run_bass_kernel_spmd

    P = make_plan(v_idx, e_idx, e2v_weight)
    key = (P.C1, P.C2, P.W1, P.W2, GATHER_BF16, P1_DT, P2_DT, OUT_DT, FUSE,
           AG_SLICED)
    if key not in _nc_cache:
        _nc_cache[key] = build_nc(P)
    nc = _nc_cache[key]
    in_maps = make_in_maps(P, X, W, b)
    res = run_bass_kernel_spmd(nc, in_maps, list(range(P.ncores)), trace=TRACE)
    _last_results = res
    shards = [res.results[k]["out"] for k in range(P.ncores)]
    return assemble(P, shards)

